# revision 42
# baseline (speedup 1.0000x reference)
"""Trainium2 Bass kernel for nn_InvDirectImageAlign (inverse-compositional image alignment).

v3: ONE compiled NEFF runs all 5 Gauss-Newton iterations on device
(hardware For_i loop). Per core: 2 batch elements. Device does warp,
bilinear grid_sample (GPSIMD ap_gather from fp16 pair-dup band tables),
the JtWJ/Rhs normal equations via TensorEngine matmuls of a per-pixel
fp16 factor matrix G (JtWJ = sum_c G_c^T G_c), the 6x6 Cholesky solve,
se3_exp (Taylor series - angles are <<1 here) and the pose composition.
Inputs upload once; output is just the final 4x4 poses.

Chunking: (batch, 16-row y-band, 224-col x-half) = 80 chunks/core; the 8
GPSIMD partition-groups each own one chunk per superstep; 10 supersteps.
Two pixel layouts, bridged only by PE transposes of gathered data:
  mod-128:    pixel j of chunk(g,s) at partition j%128, free col (g, j//128)
  wrapped-16: pixel j at partition 16g + j%16, free col j//16   (ap_gather's
              index layout)
"""
import numpy as np

B, C, H, W = 16, 3, 320, 448
HW = H * W
N_ITERS = 5
LAMBDA = 0.01
HUBER_DELTA = 0.1
EPS = 1e-6

BH = 16            # band rows per chunk
CW = 224           # band cols per chunk
N = BH * CW        # 3584 px per chunk
A = N // 128       # 28
M = N // 16        # 224
NS = 10            # supersteps
TR = 67            # table rows (16 + 25 + 26)
TC = 266           # table cols (224 + 20 + 21 + 1)
NELEM = TR * TC    # 17822 pairs
YPAD = 25
XPAD = 20


def skew3(w):
    x, y, z = w[..., 0], w[..., 1], w[..., 2]
    O = np.zeros_like(x)
    return np.stack([np.stack([O, -z, y], -1),
                     np.stack([z, O, -x], -1),
                     np.stack([-y, x, O], -1)], -2)


def se3_exp(xi):
    xi = np.asarray(xi, np.float64)
    v, w = xi[:, :3], xi[:, 3:]
    th2 = np.sum(w * w, -1)[:, None, None]
    th2c = np.maximum(th2, 1e-16)
    th = np.sqrt(th2c)
    small = th2 < 1e-10
    Aa = np.where(small, 1.0 - th2 / 6.0, np.sin(th) / th)
    Bc = np.where(small, 0.5 - th2 / 24.0, (1.0 - np.cos(th)) / th2c)
    Cc = np.where(small, 1.0 / 6.0 - th2 / 120.0, (1.0 - Aa) / th2c)
    K = skew3(w)
    K2 = K @ K
    I = np.eye(3)
    R = I + Aa * K + Bc * K2
    V = I + Bc * K + Cc * K2
    t = np.einsum('bij,bj->bi', V, v)
    T = np.zeros((xi.shape[0], 4, 4))
    T[:, :3, :3] = R
    T[:, :3, 3] = t
    T[:, 3, 3] = 1.0
    return T.astype(np.float32)


def feature_gradient(img):
    p = np.pad(img, ((0, 0), (0, 0), (0, 0), (1, 1)), mode='edge')
    dx = 0.5 * (p[..., 2:] - p[..., :-2])
    p = np.pad(img, ((0, 0), (0, 0), (1, 1), (0, 0)), mode='edge')
    dy = 0.5 * (p[..., 2:, :] - p[..., :-2, :])
    return dx.astype(np.float32), dy.astype(np.float32)


def chunk_of(g, s):
    b = g // 4
    local = (g % 4) * 10 + s
    return b, local // 2, local % 2


def bases_of(yb, xh):
    r0, c0 = yb * BH, xh * CW
    rbase = int(np.clip(r0 - YPAD, 0, H - TR))
    cbase = int(np.clip(c0 - XPAD, 0, W - (TC - 1)))
    return rbase, cbase


def mod128_cols_batch(x):
    """[2,K,H,W] -> [128, NS*8*A*K] vectorized (one core's 2 batches)."""
    K = x.shape[1]
    # chunk (b, yb, xh): local = yb*2+xh; g = b*4 + local//10; s = local%10
    a = x.reshape(2, K, 20, BH, 2, CW)          # b K yb row xh col
    a = a.transpose(0, 2, 4, 1, 3, 5)           # b yb xh K row col
    a = a.reshape(2, 40, K, N)                  # local = yb*2+xh
    a = a.reshape(2, 4, 10, K, A, 128)          # b g4 s K a p
    a = a.transpose(5, 2, 0, 1, 4, 3)           # p s b g4 a K
    return np.ascontiguousarray(a.reshape(128, NS, 8, A, K).reshape(128, -1))


def wrap16_cols_batch(x, K):
    """[2,K,H,W] -> [128, NS*M*K] (partition 16g + j%16, col (j//16)*K + k)."""
    a = x.reshape(2, K, 20, BH, 2, CW)
    a = a.transpose(0, 2, 4, 1, 3, 5).reshape(2, 40, K, N)
    a = a.reshape(2, 4, 10, K, M, 16)           # b g4 s K m plo
    a = a.transpose(0, 1, 5, 2, 4, 3)           # b g4 plo s m K
    # partition = 16*(b*4+g4) + plo
    return np.ascontiguousarray(a.reshape(128, NS, M, K).reshape(128, -1))


def host_precompute_all(pose_twist, I0, I1, invD0, invD1, intr):
    """Vectorized over all B=16; returns per-core input dicts + T0 per core."""
    T0 = se3_exp(pose_twist)
    fx = intr[:, 0][:, None, None]; fy = intr[:, 1][:, None, None]
    cx = intr[:, 2][:, None, None]; cy = intr[:, 3][:, None, None]
    uu = np.arange(W, dtype=np.float32)[None, None, :]
    vv = np.arange(H, dtype=np.float32)[None, :, None]
    iD = np.maximum(invD1[:, 0], EPS).astype(np.float32)
    z1 = (1.0 / iD).astype(np.float32)
    xn = ((uu - cx) / fx).astype(np.float32)     # [B,1,W]
    yn = ((vv - cy) / fy).astype(np.float32)     # [B,H,1]
    x1 = xn * z1
    y1 = yn * z1
    dI0x, dI0y = feature_gradient(I0)
    dD0x, dD0y = feature_gradient(invD0)
    planes12 = np.concatenate([dI0x, dI0y, dD0x, dD0y, I0, invD0], axis=1).astype(np.float16)
    flat = planes12.reshape(B, 12, HW)
    pds = np.zeros((B, 12, HW + 2), np.float16)
    pds[:, :, 1:HW + 1] = flat

    X1 = np.stack([x1, y1, z1], 1).astype(np.float32)       # [B, 3, H, W]
    I1f = np.asarray(I1, np.float32)

    bw = np.zeros((128, NS, 4), np.float32)
    for g in range(8):
        for s in range(NS):
            _, yb, xh2 = chunk_of(g, s)
            rbase, cbase = bases_of(yb, xh2)
            bw[16 * g:16 * g + 16, s, 0] = rbase
            bw[16 * g:16 * g + 16, s, 1] = cbase - 1          # xf min
            bw[16 * g:16 * g + 16, s, 2] = cbase + (TC - 2)   # xf max
            bw[16 * g:16 * g + 16, s, 3] = 1 - cbase          # kx offset
    bw = np.ascontiguousarray(bw.reshape(128, NS * 4))
    idn = np.eye(128, dtype=np.float16)

    I1h = I1f.astype(np.float16)
    core_inputs, T0s = [], []
    for core in range(8):
        sl = slice(2 * core, 2 * core + 2)
        inp = {}
        inp["pds"] = np.ascontiguousarray(pds[sl])
        inp["x1m"] = mod128_cols_batch(X1[sl])
        inp["x1w"] = wrap16_cols_batch(X1[sl], 3)
        inp["i1m"] = mod128_cols_batch(I1h[sl])
        inp["bw"] = bw
        inp["idn"] = idn
        q = np.zeros((2, 16), np.float32)
        q[:, :9] = T0[sl, :3, :3].reshape(2, 9)
        q[:, 9:12] = T0[sl, :3, 3]
        q[:, 12:16] = intr[sl]
        rtm = np.zeros((128, 16, 8), np.float32)
        rtw = np.zeros((128, 16), np.float32)
        for g in range(8):
            bb = g // 4
            rtm[:, :, g] = q[bb][None, :]
            rtw[16 * g:16 * g + 16, :] = q[bb][None, :]
        inp["rtm"] = np.ascontiguousarray(rtm.reshape(128, 16 * 8))
        inp["rtw"] = rtw
        inp["t0q"] = np.ascontiguousarray(T0[sl].reshape(2, 16).astype(np.float32))
        inp["intr2"] = np.ascontiguousarray(intr[sl].astype(np.float32))
        core_inputs.append(inp)
        T0s.append(T0[sl])
    return core_inputs, T0s


_NC_CACHE = {}
PROFILE = False
LAST_EXEC_NS = []
LAST_TRACES = []
LAST_WALL = []


def build_nc():
    import concourse.bacc as bacc
    import concourse.bass as bass
    import concourse.tile as tile
    from concourse import mybir

    fp32 = mybir.dt.float32
    fp16 = mybir.dt.float16
    i16 = mybir.dt.int16
    i32 = mybir.dt.int32
    AL = mybir.AluOpType
    ACT = mybir.ActivationFunctionType
    AX = mybir.AxisListType

    nc = bacc.Bacc("TRN2", target_bir_lowering=False, debug=False, num_devices=8)

    pd_in = nc.dram_tensor("pds", [2, 12, HW + 2], fp16, kind="ExternalInput")
    x1m_in = nc.dram_tensor("x1m", [128, NS * 8 * A * 3], fp32, kind="ExternalInput")
    x1w_in = nc.dram_tensor("x1w", [128, NS * M * 3], fp32, kind="ExternalInput")
    i1m_in = nc.dram_tensor("i1m", [128, NS * 8 * A * 3], fp16, kind="ExternalInput")
    bw_in = nc.dram_tensor("bw", [128, NS * 4], fp32, kind="ExternalInput")
    idn_in = nc.dram_tensor("idn", [128, 128], fp16, kind="ExternalInput")
    rtm_in = nc.dram_tensor("rtm", [128, 16 * 8], fp32, kind="ExternalInput")
    rtw_in = nc.dram_tensor("rtw", [128, 16], fp32, kind="ExternalInput")
    t0_in = nc.dram_tensor("t0q", [2, 16], fp32, kind="ExternalInput")
    intr_in = nc.dram_tensor("intr2", [2, 4], fp32, kind="ExternalInput")
    tout_ext = nc.dram_tensor("tout", [2, 16], fp32, kind="ExternalOutput")
    qscr = nc.dram_tensor("qscr", [2, 16], fp32, kind="Internal")

    with tile.TileContext(nc) as tc:
        with tc.tile_pool(name="cst", bufs=1) as cpool, \
             tc.tile_pool(name="tblp", bufs=1) as tpool, \
             tc.tile_pool(name="strm", bufs=2) as sp, \
             tc.tile_pool(name="scr", bufs=1) as sc, \
             tc.tile_pool(name="gath", bufs=1) as gp, \
             tc.tile_pool(name="ps", bufs=2, space="PSUM") as pp, \
             tc.tile_pool(name="jp", bufs=1, space="PSUM") as jp:

            rtm = cpool.tile([128, 16 * 8], fp32, tag="rtm")
            rtm0 = cpool.tile([128, 16 * 8], fp32, tag="rtm0")
            rtw = cpool.tile([128, 16], fp32, tag="rtw")
            bwc = cpool.tile([128, NS * 4], fp32, tag="bw")
            idn = cpool.tile([128, 128], fp16, tag="idn")
            Tq = cpool.tile([2, 16], fp32, tag="Tq")
            intr = cpool.tile([2, 4], fp32, tag="intr")
            nc.sync.dma_start(out=rtm[:, :], in_=rtm_in.ap())
            nc.sync.dma_start(out=rtm0[:, :], in_=rtm_in.ap())
            nc.sync.dma_start(out=rtw[:, :], in_=rtw_in.ap())
            nc.sync.dma_start(out=bwc[:, :], in_=bw_in.ap())
            nc.sync.dma_start(out=idn[:, :], in_=idn_in.ap())
            nc.sync.dma_start(out=Tq[:, :], in_=t0_in.ap())
            nc.sync.dma_start(out=intr[:, :], in_=intr_in.ap())

            psJ = [jp.tile([28, 28], fp32, name=f"psJ{b}", tag=f"psJ{b}") for b in range(2)]

            tbl0 = tpool.tile([128, NELEM * 2], fp16, tag="tbl")
            nc.vector.memset(tbl0[:, :], 0.0)
            stbl0 = tpool.tile([128, 34 * (TC + 1)], fp16, tag="stbl")
            nc.vector.memset(stbl0[:, :], 0.0)

            def rq(qi):   # mod-128 per-group broadcast: dims (g x8, a x A step0)
                sl = rtm[:, qi * 8:(qi + 1) * 8]
                return bass.AP(sl.tensor, sl.offset, [list(sl.ap[0]), [1, 8], [0, A]])

            def rqw(qi):  # wrapped per-partition scalar bcast over M
                sl = rtw[:, qi:qi + 1]
                return bass.AP(sl.tensor, sl.offset, [list(sl.ap[0]), [0, M]])

            def rtwS(qi):  # wrapped per-partition scalar [128,1]
                return rtw[:, qi:qi + 1]

            def bwq(s, j):
                sl = bwc[:, s * 4 + j:s * 4 + j + 1]
                return bass.AP(sl.tensor, sl.offset, [list(sl.ap[0]), [0, M]])

            def bwS(s, j):
                return bwc[:, s * 4 + j:s * 4 + j + 1]

            TT = nc.vector.tensor_tensor
            TS = lambda out, in0, s1, op: nc.vector.tensor_scalar(out, in0, s1, None, op)
            TS2 = lambda out, in0, s1, s2, op0, op1: nc.vector.tensor_scalar(out, in0, s1, s2, op0, op1)
            STT = nc.vector.scalar_tensor_tensor

            with tc.For_i(0, N_ITERS) as _it:
                for s in range(NS):
                    tbl = tbl0
                    for r0, nr in ((0, 34), (34, 33)):
                        for g in range(8):
                            b, yb, xh = chunk_of(g, s)
                            rbase, cbase = bases_of(yb, xh)
                            start = (rbase + r0) * W + cbase
                            src0 = pd_in.ap()
                            src = bass.AP(src0.tensor,
                                          src0.offset + b * 12 * (HW + 2) + start,
                                          [[HW + 2, 12], [W, nr], [1, TC + 1]])
                            dsl = stbl0[16 * g:16 * g + 12, :]
                            dst = bass.AP(dsl.tensor, dsl.offset,
                                          [[dsl.ap[0][0], 12], [TC + 1, nr], [1, TC + 1]])
                            nc.sync.dma_start(out=dst, in_=src)
                        for e in range(2):
                            pout = bass.AP(tbl.tensor, tbl.offset + e + r0 * 2 * TC,
                                           [list(tbl.ap[0]), [2 * TC, nr], [2, TC]])
                            pin = bass.AP(stbl0.tensor, stbl0.offset + e,
                                          [list(stbl0.ap[0]), [TC + 1, nr], [1, TC]])
                            nc.scalar.activation(pout, pin, ACT.Copy)

                    x1w = sp.tile([128, M * 3], fp32, tag="x1w")
                    nc.sync.dma_start(out=x1w[:, :], in_=x1w_in.ap()[:, s * M * 3:(s + 1) * M * 3])
                    x1m = sp.tile([128, 8 * A * 3], fp32, tag="x1m")
                    nc.sync.dma_start(out=x1m[:, :], in_=x1m_in.ap()[:, s * 8 * A * 3:(s + 1) * 8 * A * 3])
                    i1 = sp.tile([128, 8 * A * 3], fp16, tag="i1")
                    nc.sync.dma_start(out=i1[:, :], in_=i1m_in.ap()[:, s * 8 * A * 3:(s + 1) * 8 * A * 3])

                    # ---------- wrapped-16 idx pipeline ----------
                    def xw(k):
                        sl = x1w[:, :]
                        return bass.AP(sl.tensor, sl.offset + k, [list(sl.ap[0]), [3, M]])

                    def tw(name):
                        return sc.tile([128, M], fp32, name="w_" + name + f"_{s}", tag="w_" + name)

                    t1w = tw("t1")
                    X0zw = tw("X0z")
                    STT(X0zw[:, :], xw(0), rtwS(6), rqw(11), AL.mult, AL.add)
                    STT(X0zw[:, :], xw(1), rtwS(7), X0zw[:, :], AL.mult, AL.add)
                    STT(X0zw[:, :], xw(2), rtwS(8), X0zw[:, :], AL.mult, AL.add)
                    X0xw = tw("X0x")
                    STT(X0xw[:, :], xw(0), rtwS(0), rqw(9), AL.mult, AL.add)
                    STT(X0xw[:, :], xw(1), rtwS(1), X0xw[:, :], AL.mult, AL.add)
                    STT(X0xw[:, :], xw(2), rtwS(2), X0xw[:, :], AL.mult, AL.add)
                    X0yw = tw("X0y")
                    STT(X0yw[:, :], xw(0), rtwS(3), rqw(10), AL.mult, AL.add)
                    STT(X0yw[:, :], xw(1), rtwS(4), X0yw[:, :], AL.mult, AL.add)
                    STT(X0yw[:, :], xw(2), rtwS(5), X0yw[:, :], AL.mult, AL.add)

                    izw = tw("iz")
                    TS(t1w[:, :], X0zw[:, :], EPS, AL.max)
                    nc.vector.reciprocal_approx_fast(izw[:, :], t1w[:, :])
                    u0w = tw("u0"); v0w = tw("v0")
                    TT(u0w[:, :], X0xw[:, :], izw[:, :], op=AL.mult)
                    STT(u0w[:, :], u0w[:, :], rtwS(12), rqw(14), AL.mult, AL.add)
                    TT(v0w[:, :], X0yw[:, :], izw[:, :], op=AL.mult)
                    STT(v0w[:, :], v0w[:, :], rtwS(13), rqw(15), AL.mult, AL.add)
                    TS2(u0w[:, :], u0w[:, :], -0.5 * (W - 1), 1.5 * (W - 1), AL.max, AL.min)
                    TS2(v0w[:, :], v0w[:, :], -0.5 * (H - 1), 1.5 * (H - 1), AL.max, AL.min)
                    x0fw = tw("x0f"); y0fw = tw("y0f")
                    fi32w = sc.tile([128, M], i32, name=f"fi32w_{s}", tag="fi32w")
                    TS(t1w[:, :], u0w[:, :], 0.5, AL.subtract)
                    nc.vector.tensor_copy(fi32w[:, :], t1w[:, :])
                    nc.vector.tensor_copy(x0fw[:, :], fi32w[:, :])
                    TS(t1w[:, :], v0w[:, :], 0.5, AL.subtract)
                    nc.vector.tensor_copy(fi32w[:, :], t1w[:, :])
                    nc.vector.tensor_copy(y0fw[:, :], fi32w[:, :])
                    xfw = t1w; kxw = izw; yrw = X0zw
                    ktw = X0xw; kbw = X0yw
                    STT(xfw[:, :], x0fw[:, :], bwS(s, 1), bwq(s, 2), AL.max, AL.min)
                    nc.vector.tensor_scalar(kxw[:, :], xfw[:, :], bwS(s, 3), None, AL.add)
                    nc.vector.tensor_scalar(yrw[:, :], y0fw[:, :], bwS(s, 0), 0.0, AL.subtract, AL.max)
                    TS2(ktw[:, :], yrw[:, :], float(TR - 1), float(TC), AL.min, AL.mult)
                    TT(ktw[:, :], ktw[:, :], kxw[:, :], op=AL.add)
                    TS2(kbw[:, :], yrw[:, :], 1.0, float(TR - 1), AL.add, AL.min)
                    TS(kbw[:, :], kbw[:, :], float(TC), AL.mult)
                    TT(kbw[:, :], kbw[:, :], kxw[:, :], op=AL.add)
                    kidx = sc.tile([128, 2 * M], i16, name=f"kidx_{s}", tag="kidx")
                    nc.vector.tensor_copy(kidx[:, :M], ktw[:, :])
                    nc.vector.tensor_copy(kidx[:, M:], kbw[:, :])

                    gt2 = gp.tile([128, 2 * N * 2], fp16, tag="gt2")
                    nc.gpsimd.ap_gather(gt2[:, :], tbl[:, :], kidx[:, :],
                                        channels=128, num_elems=NELEM, d=2, num_idxs=2 * N)

                    # ---------- mod-128 warp pipeline ----------
                    def xm(k):
                        sl = x1m[:, :]
                        return bass.AP(sl.tensor, sl.offset + k, [list(sl.ap[0]), [3, 8 * A]])

                    def tm(name):
                        return sc.tile([128, 8 * A], fp32, name="m_" + name + f"_{s}", tag="m_" + name)

                    def matvec(dst, aps, qis, t1):
                        TT(dst[:, :], aps[0], qis[0], op=AL.mult)
                        TT(t1[:, :], aps[1], qis[1], op=AL.mult)
                        TT(dst[:, :], dst[:, :], t1[:, :], op=AL.add)
                        TT(t1[:, :], aps[2], qis[2], op=AL.mult)
                        TT(dst[:, :], dst[:, :], t1[:, :], op=AL.add)
                        TT(dst[:, :], dst[:, :], qis[3], op=AL.add)

                    # ---- on-device A6/B6/T6 at the initial pose (rtm0) ----
                    abt = sc.tile([128, 8 * A * 18], fp16, name=f"abt_{s}", tag="abt")

                    def acol(k):
                        sl = abt[:, :]
                        return bass.AP(sl.tensor, sl.offset + k, [list(sl.ap[0]), [18, 224]])

                    def rq0(qi):
                        sl = rtm0[:, qi * 8:(qi + 1) * 8]
                        return bass.AP(sl.tensor, sl.offset, [list(sl.ap[0]), [1, 8], [0, A]])

                    j1 = tm("j1"); j2 = tm("j2")
                    jx = tm("jx"); jy = tm("jy"); jz = tm("jz"); jiz = tm("jiz")
                    matvec(jz, [xm(0), xm(1), xm(2)], [rq0(6), rq0(7), rq0(8), rq0(11)], j1)
                    matvec(jx, [xm(0), xm(1), xm(2)], [rq0(0), rq0(1), rq0(2), rq0(9)], j1)
                    matvec(jy, [xm(0), xm(1), xm(2)], [rq0(3), rq0(4), rq0(5), rq0(10)], j1)
                    TS(j1[:, :], jz[:, :], EPS, AL.max)
                    nc.vector.reciprocal_approx_fast(jiz[:, :], j1[:, :])
                    fxiz = tm("fxiz"); fyiz = tm("fyiz"); zizt = tm("zizt")
                    A2t = tm("A2t"); B2t = tm("B2t")
                    TT(fxiz[:, :], jiz[:, :], rq0(12), op=AL.mult)
                    TT(fyiz[:, :], jiz[:, :], rq0(13), op=AL.mult)
                    TT(zizt[:, :], jz[:, :], jiz[:, :], op=AL.mult)
                    TT(j1[:, :], jx[:, :], jiz[:, :], op=AL.mult)
                    TT(A2t[:, :], fxiz[:, :], j1[:, :], op=AL.mult)
                    TT(j1[:, :], jy[:, :], jiz[:, :], op=AL.mult)
                    TT(B2t[:, :], fyiz[:, :], j1[:, :], op=AL.mult)
                    TS(acol(0), fxiz[:, :], -1.0, AL.mult)
                    TS(acol(1), fxiz[:, :], 0.0, AL.mult)
                    nc.vector.tensor_copy(acol(2), A2t[:, :])
                    TT(acol(3), A2t[:, :], jy[:, :], op=AL.mult)
                    TT(j1[:, :], zizt[:, :], rq0(12), op=AL.mult)
                    TT(j2[:, :], A2t[:, :], jx[:, :], op=AL.mult)
                    TT(j1[:, :], j1[:, :], j2[:, :], op=AL.add)
                    TS(acol(4), j1[:, :], -1.0, AL.mult)
                    TT(acol(5), fxiz[:, :], jy[:, :], op=AL.mult)
                    TS(acol(6), fxiz[:, :], 0.0, AL.mult)
                    TS(acol(7), fyiz[:, :], -1.0, AL.mult)
                    nc.vector.tensor_copy(acol(8), B2t[:, :])
                    TT(j1[:, :], zizt[:, :], rq0(13), op=AL.mult)
                    TT(j2[:, :], B2t[:, :], jy[:, :], op=AL.mult)
                    TT(acol(9), j1[:, :], j2[:, :], op=AL.add)
                    TT(j1[:, :], B2t[:, :], jx[:, :], op=AL.mult)
                    TS(acol(10), j1[:, :], -1.0, AL.mult)
                    TT(j1[:, :], fyiz[:, :], jx[:, :], op=AL.mult)
                    TS(acol(11), j1[:, :], -1.0, AL.mult)
                    TS(acol(12), fxiz[:, :], 0.0, AL.mult)
                    TS(acol(13), fxiz[:, :], 0.0, AL.mult)
                    TS2(acol(14), fxiz[:, :], 0.0, 1.0, AL.mult, AL.add)
                    nc.vector.tensor_copy(acol(15), jy[:, :])
                    TS(acol(16), jx[:, :], -1.0, AL.mult)
                    TS(acol(17), fxiz[:, :], 0.0, AL.mult)

                    m1 = j1; m2 = j2
                    X0z = jz
                    matvec(X0z, [xm(0), xm(1), xm(2)], [rq(6), rq(7), rq(8), rq(11)], m1)
                    X0x = jx
                    matvec(X0x, [xm(0), xm(1), xm(2)], [rq(0), rq(1), rq(2), rq(9)], m1)
                    X0y = jy
                    matvec(X0y, [xm(0), xm(1), xm(2)], [rq(3), rq(4), rq(5), rq(10)], m1)
                    iz = jiz
                    TS(m1[:, :], X0z[:, :], EPS, AL.max)
                    nc.vector.reciprocal_approx_fast(iz[:, :], m1[:, :])
                    u0 = fxiz; v0 = fyiz
                    TT(u0[:, :], X0x[:, :], iz[:, :], op=AL.mult)
                    TT(u0[:, :], u0[:, :], rq(12), op=AL.mult)
                    TT(u0[:, :], u0[:, :], rq(14), op=AL.add)
                    TT(v0[:, :], X0y[:, :], iz[:, :], op=AL.mult)
                    TT(v0[:, :], v0[:, :], rq(13), op=AL.mult)
                    TT(v0[:, :], v0[:, :], rq(15), op=AL.add)
                    vmask = zizt
                    TS(vmask[:, :], X0z[:, :], EPS, AL.is_gt)
                    STT(vmask[:, :], u0[:, :], 0.0, vmask[:, :], AL.is_gt, AL.mult)
                    STT(vmask[:, :], u0[:, :], float(W - 1), vmask[:, :], AL.is_lt, AL.mult)
                    STT(vmask[:, :], v0[:, :], 0.0, vmask[:, :], AL.is_gt, AL.mult)
                    STT(vmask[:, :], v0[:, :], float(H - 1), vmask[:, :], AL.is_lt, AL.mult)
                    TS2(u0[:, :], u0[:, :], -0.5 * (W - 1), 1.5 * (W - 1), AL.max, AL.min)
                    TS2(v0[:, :], v0[:, :], -0.5 * (H - 1), 1.5 * (H - 1), AL.max, AL.min)
                    wx = A2t; wy = B2t; x0f = tm("x0f"); y0f = tm("y0f")
                    fi32m = sc.tile([128, 8 * A], i32, name=f"fi32m_{s}", tag="fi32m")
                    TS(m1[:, :], u0[:, :], 0.5, AL.subtract)
                    nc.vector.tensor_copy(fi32m[:, :], m1[:, :])
                    nc.vector.tensor_copy(x0f[:, :], fi32m[:, :])
                    TT(wx[:, :], u0[:, :], x0f[:, :], op=AL.subtract)
                    TS(m1[:, :], v0[:, :], 0.5, AL.subtract)
                    nc.vector.tensor_copy(fi32m[:, :], m1[:, :])
                    nc.vector.tensor_copy(y0f[:, :], fi32m[:, :])
                    TT(wy[:, :], v0[:, :], y0f[:, :], op=AL.subtract)
                    mx0 = tm("mx0"); mx1 = tm("mx1"); my0 = tm("my0"); my1 = tm("my1")
                    TS(mx0[:, :], x0f[:, :], -0.5, AL.is_gt)
                    STT(mx0[:, :], x0f[:, :], float(W - 1) + 0.5, mx0[:, :], AL.is_lt, AL.mult)
                    TS(mx1[:, :], x0f[:, :], -1.5, AL.is_gt)
                    STT(mx1[:, :], x0f[:, :], float(W - 2) + 0.5, mx1[:, :], AL.is_lt, AL.mult)
                    TS(my0[:, :], y0f[:, :], -0.5, AL.is_gt)
                    STT(my0[:, :], y0f[:, :], float(H - 1) + 0.5, my0[:, :], AL.is_lt, AL.mult)
                    TS(my1[:, :], y0f[:, :], -1.5, AL.is_gt)
                    STT(my1[:, :], y0f[:, :], float(H - 2) + 0.5, my1[:, :], AL.is_lt, AL.mult)
                    W00 = tm("W00"); W01 = tm("W01"); W10 = tm("W10"); W11 = tm("W11")
                    TS2(m1[:, :], wx[:, :], 1.0, -1.0, AL.subtract, AL.mult)  # 1-wx
                    TS2(m2[:, :], wy[:, :], 1.0, -1.0, AL.subtract, AL.mult)  # 1-wy
                    TT(W00[:, :], m1[:, :], m2[:, :], op=AL.mult)
                    TT(W00[:, :], W00[:, :], mx0[:, :], op=AL.mult)
                    TT(W00[:, :], W00[:, :], my0[:, :], op=AL.mult)
                    TT(W01[:, :], wx[:, :], m2[:, :], op=AL.mult)
                    TT(W01[:, :], W01[:, :], mx1[:, :], op=AL.mult)
                    TT(W01[:, :], W01[:, :], my0[:, :], op=AL.mult)
                    TT(W10[:, :], m1[:, :], wy[:, :], op=AL.mult)
                    TT(W10[:, :], W10[:, :], mx0[:, :], op=AL.mult)
                    TT(W10[:, :], W10[:, :], my1[:, :], op=AL.mult)
                    TT(W11[:, :], wx[:, :], wy[:, :], op=AL.mult)
                    TT(W11[:, :], W11[:, :], mx1[:, :], op=AL.mult)
                    TT(W11[:, :], W11[:, :], my1[:, :], op=AL.mult)

                    # ---------- PE transpose + combine ----------
                    samp = sc.tile([128, A * 128], fp16, tag="samp")
                    ctmp = sc.tile([128, 512], fp16, tag="ctmp")
                    for a4 in range(A // 4):
                        ptall = pp.tile([128, 2048], fp16, tag="ptall")
                        for ci, base in enumerate((0, 1, 2 * N, 2 * N + 1)):
                            for aa in range(4):
                                a = a4 * 4 + aa
                                src = bass.AP(gt2.tensor, gt2.offset + base + a * 256,
                                              [list(gt2.ap[0]), [2, 128]])
                                nc.tensor.transpose(
                                    ptall[:, ci * 512 + aa * 128:ci * 512 + (aa + 1) * 128],
                                    src, idn[:, :])
                        for ci, wt_ in ((0, W00), (1, W01), (2, W10), (3, W11)):
                            pap = bass.AP(ptall.tensor, ptall.offset + ci * 512,
                                          [list(ptall.ap[0]), [128, 4], [16, 8], [1, 16]])
                            woff = wt_.offset + a4 * 4
                            wap = bass.AP(wt_.tensor, woff, [list(wt_.ap[0]), [1, 4], [A, 8], [0, 16]])
                            dst_off = samp.offset + a4 * 4 * 128
                            dap = bass.AP(samp.tensor, dst_off, [list(samp.ap[0]), [128, 4], [16, 8], [1, 16]])
                            if ci == 0:
                                TT(dap, pap, wap, op=AL.mult)
                            else:
                                tap = bass.AP(ctmp.tensor, ctmp.offset, [list(ctmp.ap[0]), [128, 4], [16, 8], [1, 16]])
                                TT(tap, pap, wap, op=AL.mult)
                                TT(dap, dap, tap, op=AL.add)

                    # ---------- residuals, huber weights, G build ----------
                    def sq(q):
                        sl = samp[:, :]
                        return bass.AP(sl.tensor, sl.offset + q, [list(sl.ap[0]), [16, 8], [128, A]])

                    def i1q(c):
                        sl = i1[:, :]
                        return bass.AP(sl.tensor, sl.offset + c, [list(sl.ap[0]), [3 * A, 8], [3, A]])

                    Gt = sc.tile([128, 28 * 224], fp16, tag="Gt")
                    g6a = sc.tile([128, 6 * 224], fp16, tag="g6a")
                    g6b = sc.tile([128, 6 * 224], fp16, tag="g6b")
                    one_m = tm("one_m")
                    TS2(one_m[:, :], vmask[:, :], 1.0, -1e-6, AL.subtract, AL.mult)  # (1-vm)*1e-6
                    rr = tm("rr"); bb_ = tm("bb"); ss = tm("ss")
                    ppv = tm("ppv"); qqv = tm("qqv")

                    def abt6(k0):  # [x(6) outer, chunk(224) inner], stride 18 per chunk
                        sl = abt[:, :]
                        return bass.AP(sl.tensor, sl.offset + k0, [list(sl.ap[0]), [1, 6], [18, 224]])

                    def gcols(c):  # G cols c*7 .. c*7+5: [x outer, chunk inner]
                        sl = Gt[:, :]
                        return bass.AP(sl.tensor, sl.offset + c * 7 * 224, [list(sl.ap[0]), [224, 6], [1, 224]])

                    def bc6(t):    # broadcast [128,224] over 6 x-cols
                        sl = t[:, :]
                        return bass.AP(sl.tensor, sl.offset, [list(sl.ap[0]), [0, 6], [1, 224]])

                    for c in range(3):
                        TT(rr[:, :], i1q(c), sq(8 + c), op=AL.subtract)
                        TT(rr[:, :], rr[:, :], vmask[:, :], op=AL.mult)
                        TT(rr[:, :], rr[:, :], one_m[:, :], op=AL.add)
                        nc.scalar.activation(bb_[:, :], rr[:, :], ACT.Abs)
                        TS(bb_[:, :], bb_[:, :], HUBER_DELTA, AL.max)
                        nc.vector.reciprocal_approx_fast(bb_[:, :], bb_[:, :])
                        nc.scalar.activation(ss[:, :], bb_[:, :], ACT.Sqrt, scale=HUBER_DELTA)
                        TT(ppv[:, :], ss[:, :], sq(0 + c), op=AL.mult)
                        TT(qqv[:, :], ss[:, :], sq(3 + c), op=AL.mult)
                        TT(g6a[:, :], abt6(0), bc6(ppv), op=AL.mult)
                        TT(g6b[:, :], abt6(6), bc6(qqv), op=AL.mult)
                        TT(gcols(c), g6a[:, :], g6b[:, :], op=AL.add)
                        TT(Gt[:, (c * 7 + 6) * 224:(c * 7 + 7) * 224], ss[:, :], rr[:, :], op=AL.mult)
                    # depth channel
                    TT(rr[:, :], iz[:, :], sq(11), op=AL.subtract)
                    TT(rr[:, :], rr[:, :], vmask[:, :], op=AL.mult)
                    TT(rr[:, :], rr[:, :], one_m[:, :], op=AL.add)
                    nc.scalar.activation(bb_[:, :], rr[:, :], ACT.Abs, scale=LAMBDA)
                    TS(bb_[:, :], bb_[:, :], HUBER_DELTA, AL.max)
                    nc.vector.reciprocal_approx_fast(bb_[:, :], bb_[:, :])
                    nc.scalar.activation(ss[:, :], bb_[:, :], ACT.Sqrt,
                                         scale=HUBER_DELTA * LAMBDA * LAMBDA)
                    TT(ppv[:, :], ss[:, :], sq(6), op=AL.mult)
                    TT(qqv[:, :], ss[:, :], sq(7), op=AL.mult)
                    TT(g6a[:, :], abt6(0), bc6(ppv), op=AL.mult)
                    TT(g6b[:, :], abt6(6), bc6(qqv), op=AL.mult)
                    TT(g6a[:, :], g6a[:, :], g6b[:, :], op=AL.add)
                    TT(g6b[:, :], abt6(12), bc6(ss), op=AL.mult)
                    TT(gcols(3), g6a[:, :], g6b[:, :], op=AL.add)
                    TT(Gt[:, (3 * 7 + 6) * 224:(3 * 7 + 7) * 224], ss[:, :], rr[:, :], op=AL.mult)

                    # ---------- PE: JtWJ accumulation ----------
                    for g in range(8):
                        b = g // 4
                        for a in range(A):
                            off = Gt.offset + g * A + a
                            gap = bass.AP(Gt.tensor, off, [list(Gt.ap[0]), [224, 28]])
                            first = (s == 0 and (g % 4) == 0 and a == 0)
                            last = (s == NS - 1 and (g % 4) == 3 and a == A - 1)
                            nc.tensor.matmul(psJ[b][:, :], gap, gap,
                                             start=first, stop=last,
                                             skip_group_check=True)

                # ---------- per-iteration: extract JtWJ/Rhs, solve, update pose ----------
                S28 = sc.tile([28, 56], fp32, tag="S28")
                for b in range(2):
                    nc.vector.tensor_copy(S28[:, b * 28:(b + 1) * 28], psJ[b][:, :])
                D28 = sc.tile([7, 56], fp32, tag="D28")
                for b in range(2):
                    for c in range(4):
                        src = S28[c * 7:(c + 1) * 7, b * 28 + c * 7:b * 28 + c * 7 + 7]
                        dsl = D28[:, b * 28 + c * 7:b * 28 + c * 7 + 7]
                        nc.sync.dma_start(out=dsl, in_=src)
                M7 = sc.tile([7, 14], fp32, tag="M7")
                for b in range(2):
                    din = bass.AP(D28.tensor, D28.offset + b * 28,
                                  [list(D28.ap[0]), [1, 7], [7, 4]])
                    nc.vector.tensor_reduce(M7[:, b * 7:(b + 1) * 7], din, axis=AX.X, op=AL.add)
                # Mb [2, 49]: row b = M7 block b flattened (x-major)
                Mb = sc.tile([2, 49], fp32, tag="Mb")
                for b in range(2):
                    msrc = bass.AP(M7.tensor, M7.offset + b * 7, [[M7.ap[0][0], 7], [1, 7]])
                    mdsl = Mb[b:b + 1, 0:1]
                    mdst = bass.AP(mdsl.tensor, mdsl.offset, [[Mb.ap[0][0], 1], [7, 7], [1, 7]])
                    nc.sync.dma_start(out=mdst, in_=msrc)
                # tr = sum diag(JtWJ); LM ridge on diag
                trt = sc.tile([2, 1], fp32, tag="trt")
                diag = bass.AP(Mb.tensor, Mb.offset, [list(Mb.ap[0]), [8, 6]])
                nc.vector.tensor_reduce(trt[:, :], diag, axis=AX.X, op=AL.add)
                trb = bass.AP(trt.tensor, trt.offset, [list(trt.ap[0]), [0, 6]])
                STT(diag, trb, 1e-6, diag, AL.mult, AL.add)

                # Cholesky LL^T = Hm (6x6, both batches in 2 partitions)
                Lt = sc.tile([2, 36], fp32, tag="Lt")
                lsrc = bass.AP(Mb.tensor, Mb.offset, [list(Mb.ap[0]), [7, 6], [1, 6]])
                nc.vector.tensor_copy(Lt[:, :], lsrc)
                rhs = sc.tile([2, 6], fp32, tag="rhs")
                rsrc = bass.AP(Mb.tensor, Mb.offset + 6, [list(Mb.ap[0]), [7, 6]])
                nc.vector.tensor_copy(rhs[:, :], rsrc)
                idg = sc.tile([2, 6], fp32, tag="idg")
                tmpj = sc.tile([2, 36], fp32, tag="tmpj")
                red = sc.tile([2, 6], fp32, tag="redj")
                for j in range(6):
                    jj = Lt[:, 6 * j + j:6 * j + j + 1]
                    if j > 0:
                        ljk = Lt[:, 6 * j:6 * j + j]
                        TT(tmpj[:, :j], ljk, ljk, op=AL.mult)
                        nc.vector.tensor_reduce(red[:, 0:1], tmpj[:, :j], axis=AX.X, op=AL.add)
                        TT(jj, jj, red[:, 0:1], op=AL.subtract)
                    nc.scalar.activation(jj, jj, ACT.Sqrt)
                    nc.vector.reciprocal(idg[:, j:j + 1], jj)
                    nr = 5 - j
                    if nr > 0:
                        colap = bass.AP(Lt.tensor, Lt.offset + 6 * (j + 1) + j, [list(Lt.ap[0]), [6, nr]])
                        if j > 0:
                            lik = bass.AP(Lt.tensor, Lt.offset + 6 * (j + 1), [list(Lt.ap[0]), [6, nr], [1, j]])
                            ljkb = bass.AP(Lt.tensor, Lt.offset + 6 * j, [list(Lt.ap[0]), [0, nr], [1, j]])
                            TT(tmpj[:, :nr * j], lik, ljkb, op=AL.mult)
                            tin = bass.AP(tmpj.tensor, tmpj.offset, [list(tmpj.ap[0]), [j, nr], [1, j]])
                            nc.vector.tensor_reduce(red[:, :nr], tin, axis=AX.X, op=AL.add)
                            TT(colap, colap, red[:, :nr], op=AL.subtract)
                        nc.vector.tensor_scalar(colap, colap, idg[:, j:j + 1], None, AL.mult)
                # forward substitution: L y = rhs (in place on rhs)
                for j in range(6):
                    yj = rhs[:, j:j + 1]
                    if j > 0:
                        ljk = Lt[:, 6 * j:6 * j + j]
                        TT(tmpj[:, :j], ljk, rhs[:, :j], op=AL.mult)
                        nc.vector.tensor_reduce(red[:, 0:1], tmpj[:, :j], axis=AX.X, op=AL.add)
                        TT(yj, yj, red[:, 0:1], op=AL.subtract)
                    nc.vector.tensor_scalar(yj, yj, idg[:, j:j + 1], None, AL.mult)
                # back substitution: L^T x = y -> xi = -x stored in xi tile
                for j in range(5, -1, -1):
                    xj = rhs[:, j:j + 1]
                    nk = 5 - j
                    if nk > 0:
                        lkj = bass.AP(Lt.tensor, Lt.offset + 6 * (j + 1) + j, [list(Lt.ap[0]), [6, nk]])
                        TT(tmpj[:, :nk], lkj, rhs[:, j + 1:6], op=AL.mult)
                        nc.vector.tensor_reduce(red[:, 0:1], tmpj[:, :nk], axis=AX.X, op=AL.add)
                        TT(xj, xj, red[:, 0:1], op=AL.subtract)
                    nc.vector.tensor_scalar(xj, xj, idg[:, j:j + 1], None, AL.mult)
                xi = sc.tile([2, 6], fp32, tag="xi")
                TS(xi[:, :], rhs[:, :], -1.0, AL.mult)

                # se3_exp(xi) via Taylor series (|w| << 1 in this regime)
                w3 = xi[:, 3:6]
                wsq = sc.tile([2, 3], fp32, tag="wsq")
                TT(wsq[:, :], w3, w3, op=AL.mult)
                th2 = sc.tile([2, 1], fp32, tag="th2")
                nc.vector.tensor_reduce(th2[:, :], wsq[:, :], axis=AX.X, op=AL.add)
                coA = sc.tile([2, 1], fp32, tag="coA")
                coB = sc.tile([2, 1], fp32, tag="coB")
                coC = sc.tile([2, 1], fp32, tag="coC")
                hh = sc.tile([2, 1], fp32, tag="hh")
                TS2(hh[:, :], th2[:, :], 1.0 / 120.0, -1.0 / 6.0, AL.mult, AL.add)
                nc.vector.tensor_scalar(coA[:, :], th2[:, :], hh[:, :], 1.0, AL.mult, AL.add)
                TS2(hh[:, :], th2[:, :], 1.0 / 720.0, -1.0 / 24.0, AL.mult, AL.add)
                nc.vector.tensor_scalar(coB[:, :], th2[:, :], hh[:, :], 0.5, AL.mult, AL.add)
                TS2(hh[:, :], th2[:, :], 1.0 / 5040.0, -1.0 / 120.0, AL.mult, AL.add)
                nc.vector.tensor_scalar(coC[:, :], th2[:, :], hh[:, :], 1.0 / 6.0, AL.mult, AL.add)
                # K, K2
                Kt = sc.tile([2, 9], fp32, tag="Kt")
                nc.vector.memset(Kt[:, :], 0.0)
                TS(Kt[:, 1:2], xi[:, 5:6], -1.0, AL.mult)   # -z
                nc.vector.tensor_copy(Kt[:, 2:3], xi[:, 4:5])  # y
                nc.vector.tensor_copy(Kt[:, 3:4], xi[:, 5:6])  # z
                TS(Kt[:, 5:6], xi[:, 3:4], -1.0, AL.mult)   # -x
                TS(Kt[:, 6:7], xi[:, 4:5], -1.0, AL.mult)   # -y
                nc.vector.tensor_copy(Kt[:, 7:8], xi[:, 3:4])  # x
                K2t = sc.tile([2, 9], fp32, tag="K2t")
                wiap = bass.AP(xi.tensor, xi.offset + 3, [list(xi.ap[0]), [1, 3], [0, 3]])
                wjap = bass.AP(xi.tensor, xi.offset + 3, [list(xi.ap[0]), [0, 3], [1, 3]])
                TT(K2t[:, :], wiap, wjap, op=AL.mult)
                k2diag = bass.AP(K2t.tensor, K2t.offset, [list(K2t.ap[0]), [4, 3]])
                nc.vector.tensor_scalar(k2diag, k2diag, th2[:, :], None, AL.subtract)
                Rt = sc.tile([2, 9], fp32, tag="Rt")
                Vt = sc.tile([2, 9], fp32, tag="Vt")
                t9 = sc.tile([2, 9], fp32, tag="t9")
                nc.vector.tensor_scalar(Rt[:, :], Kt[:, :], coA[:, :], None, AL.mult)
                nc.vector.tensor_scalar(t9[:, :], K2t[:, :], coB[:, :], None, AL.mult)
                TT(Rt[:, :], Rt[:, :], t9[:, :], op=AL.add)
                rdiag = bass.AP(Rt.tensor, Rt.offset, [list(Rt.ap[0]), [4, 3]])
                TS(rdiag, rdiag, 1.0, AL.add)
                nc.vector.tensor_scalar(Vt[:, :], Kt[:, :], coB[:, :], None, AL.mult)
                nc.vector.tensor_scalar(t9[:, :], K2t[:, :], coC[:, :], None, AL.mult)
                TT(Vt[:, :], Vt[:, :], t9[:, :], op=AL.add)
                vdiag = bass.AP(Vt.tensor, Vt.offset, [list(Vt.ap[0]), [4, 3]])
                TS(vdiag, vdiag, 1.0, AL.add)
                # t = V @ v
                vbc = bass.AP(xi.tensor, xi.offset, [list(xi.ap[0]), [0, 3], [1, 3]])
                TT(t9[:, :], Vt[:, :], vbc, op=AL.mult)
                tv = sc.tile([2, 3], fp32, tag="tv")
                t9v = bass.AP(t9.tensor, t9.offset, [list(t9.ap[0]), [3, 3], [1, 3]])
                nc.vector.tensor_reduce(tv[:, :], t9v, axis=AX.X, op=AL.add)
                # E = [[R, t],[0,0,0,1]] as [2,16]
                Et = sc.tile([2, 16], fp32, tag="Et")
                nc.vector.memset(Et[:, :], 0.0)
                edst = bass.AP(Et.tensor, Et.offset, [list(Et.ap[0]), [4, 3], [1, 3]])
                esrc = bass.AP(Rt.tensor, Rt.offset, [list(Rt.ap[0]), [3, 3], [1, 3]])
                nc.vector.tensor_copy(edst, esrc)
                edst2 = bass.AP(Et.tensor, Et.offset + 3, [list(Et.ap[0]), [4, 3]])
                nc.vector.tensor_copy(edst2, tv[:, :])
                TS(Et[:, 15:16], Et[:, 15:16], 1.0, AL.add)
                # newT = T @ E
                nT = sc.tile([2, 16], fp32, tag="nT")
                for k in range(4):
                    tcol = bass.AP(Tq.tensor, Tq.offset + k, [list(Tq.ap[0]), [4, 4], [0, 4]])
                    erow = bass.AP(Et.tensor, Et.offset + 4 * k, [list(Et.ap[0]), [0, 4], [1, 4]])
                    if k == 0:
                        TT(nT[:, :], tcol, erow, op=AL.mult)
                    else:
                        TT(tmpj[:, :16], tcol, erow, op=AL.mult)
                        TT(nT[:, :], nT[:, :], tmpj[:, :16], op=AL.add)
                nc.vector.tensor_copy(Tq[:, :], nT[:, :])
                # rebuild q = [R(9) | t(3) | intr(4)] and broadcast to rtm/rtw
                qt = sc.tile([2, 16], fp32, tag="qt")
                qr = bass.AP(Tq.tensor, Tq.offset, [list(Tq.ap[0]), [4, 3], [1, 3]])
                nc.vector.tensor_copy(qt[:, 0:9], qr)
                qtcol = bass.AP(Tq.tensor, Tq.offset + 3, [list(Tq.ap[0]), [4, 3]])
                nc.vector.tensor_copy(qt[:, 9:12], qtcol)
                nc.vector.tensor_copy(qt[:, 12:16], intr[:, :])
                nc.sync.dma_start(out=qscr.ap(), in_=qt[:, :])
                qsap = qscr.ap()
                for b in range(2):
                    qsrc = bass.AP(qsap.tensor, qsap.offset + b * 16, [[0, 64], [1, 16]])
                    nc.sync.dma_start(out=rtw[b * 64:(b + 1) * 64, :], in_=qsrc)
                for g in range(8):
                    b = g // 4
                    qsrc = bass.AP(qsap.tensor, qsap.offset + b * 16, [[0, 128], [1, 16]])
                    rdst = bass.AP(rtm.tensor, rtm.offset + g, [list(rtm.ap[0]), [8, 16]])
                    nc.sync.dma_start(out=rdst, in_=qsrc)

            nc.sync.dma_start(out=tout_ext.ap(), in_=Tq[:, :])

    nc.finalize()
    return nc


def kernel(pose_twist, I0, I1, invD0, invD1, intrinsics):
    from concourse.bass_utils import run_bass_kernel_spmd

    nc = _NC_CACHE.get("nc")
    if nc is None:
        nc = build_nc()
        _NC_CACHE["nc"] = nc

    pose_twist = np.asarray(pose_twist, np.float32)
    I0 = np.asarray(I0, np.float32); I1 = np.asarray(I1, np.float32)
    invD0 = np.asarray(invD0, np.float32); invD1 = np.asarray(invD1, np.float32)
    intrinsics = np.asarray(intrinsics, np.float32)

    import time as _time
    LAST_WALL.clear(); LAST_EXEC_NS.clear(); LAST_TRACES.clear()
    t0 = _time.time()
    in_maps, _ = host_precompute_all(pose_twist, I0, I1, invD0, invD1, intrinsics)
    t1 = _time.time()
    res = run_bass_kernel_spmd(nc, in_maps, list(range(8)), trace=PROFILE)
    t2 = _time.time()
    LAST_WALL.extend([round(t1 - t0, 3), round(t2 - t1, 3)])
    if PROFILE:
        if res.exec_time_ns is not None:
            LAST_EXEC_NS.append(res.exec_time_ns)
        if res.instructions_and_trace is not None:
            LAST_TRACES.append(res.instructions_and_trace[1])

    outs = []
    for core in range(8):
        outs.append(res.results[core]["tout"].reshape(2, 4, 4))
    return np.concatenate(outs, axis=0).astype(np.float32)


# revision 43
# speedup vs baseline: 1.2780x; 1.2780x over previous
"""Trainium2 Bass kernel for nn_InvDirectImageAlign (inverse-compositional image alignment).

v3: ONE compiled NEFF runs all 5 Gauss-Newton iterations on device
(hardware For_i loop). Per core: 2 batch elements. Device does warp,
bilinear grid_sample (GPSIMD ap_gather from fp16 pair-dup band tables),
the JtWJ/Rhs normal equations via TensorEngine matmuls of a per-pixel
fp16 factor matrix G (JtWJ = sum_c G_c^T G_c), the 6x6 Cholesky solve,
se3_exp (Taylor series - angles are <<1 here) and the pose composition.
Inputs upload once; output is just the final 4x4 poses.

Chunking: (batch, 16-row y-band, 224-col x-half) = 80 chunks/core; the 8
GPSIMD partition-groups each own one chunk per superstep; 10 supersteps.
Two pixel layouts, bridged only by PE transposes of gathered data:
  mod-128:    pixel j of chunk(g,s) at partition j%128, free col (g, j//128)
  wrapped-16: pixel j at partition 16g + j%16, free col j//16   (ap_gather's
              index layout)
"""
import numpy as np

B, C, H, W = 16, 3, 320, 448
HW = H * W
N_ITERS = 5
LAMBDA = 0.01
HUBER_DELTA = 0.1
EPS = 1e-6

BH = 16            # band rows per chunk
CW = 224           # band cols per chunk
N = BH * CW        # 3584 px per chunk
A = N // 128       # 28
M = N // 16        # 224
NS = 10            # supersteps
TR = 67            # table rows (16 + 25 + 26)
TC = 266           # table cols (224 + 20 + 21 + 1)
NELEM = TR * TC    # 17822 pairs
YPAD = 25
XPAD = 20


def skew3(w):
    x, y, z = w[..., 0], w[..., 1], w[..., 2]
    O = np.zeros_like(x)
    return np.stack([np.stack([O, -z, y], -1),
                     np.stack([z, O, -x], -1),
                     np.stack([-y, x, O], -1)], -2)


def se3_exp(xi):
    xi = np.asarray(xi, np.float64)
    v, w = xi[:, :3], xi[:, 3:]
    th2 = np.sum(w * w, -1)[:, None, None]
    th2c = np.maximum(th2, 1e-16)
    th = np.sqrt(th2c)
    small = th2 < 1e-10
    Aa = np.where(small, 1.0 - th2 / 6.0, np.sin(th) / th)
    Bc = np.where(small, 0.5 - th2 / 24.0, (1.0 - np.cos(th)) / th2c)
    Cc = np.where(small, 1.0 / 6.0 - th2 / 120.0, (1.0 - Aa) / th2c)
    K = skew3(w)
    K2 = K @ K
    I = np.eye(3)
    R = I + Aa * K + Bc * K2
    V = I + Bc * K + Cc * K2
    t = np.einsum('bij,bj->bi', V, v)
    T = np.zeros((xi.shape[0], 4, 4))
    T[:, :3, :3] = R
    T[:, :3, 3] = t
    T[:, 3, 3] = 1.0
    return T.astype(np.float32)


def feature_gradient(img):
    p = np.pad(img, ((0, 0), (0, 0), (0, 0), (1, 1)), mode='edge')
    dx = 0.5 * (p[..., 2:] - p[..., :-2])
    p = np.pad(img, ((0, 0), (0, 0), (1, 1), (0, 0)), mode='edge')
    dy = 0.5 * (p[..., 2:, :] - p[..., :-2, :])
    return dx.astype(np.float32), dy.astype(np.float32)


def chunk_of(g, s):
    b = g // 4
    local = (g % 4) * 10 + s
    return b, local // 2, local % 2


def bases_of(yb, xh):
    r0, c0 = yb * BH, xh * CW
    rbase = int(np.clip(r0 - YPAD, 0, H - TR))
    cbase = int(np.clip(c0 - XPAD, 0, W - (TC - 1)))
    return rbase, cbase


def mod128_cols_batch(x):
    """[2,K,H,W] -> [128, NS*8*A*K] vectorized (one core's 2 batches)."""
    K = x.shape[1]
    # chunk (b, yb, xh): local = yb*2+xh; g = b*4 + local//10; s = local%10
    a = x.reshape(2, K, 20, BH, 2, CW)          # b K yb row xh col
    a = a.transpose(0, 2, 4, 1, 3, 5)           # b yb xh K row col
    a = a.reshape(2, 40, K, N)                  # local = yb*2+xh
    a = a.reshape(2, 4, 10, K, A, 128)          # b g4 s K a p
    a = a.transpose(5, 2, 0, 1, 4, 3)           # p s b g4 a K
    return np.ascontiguousarray(a.reshape(128, NS, 8, A, K).reshape(128, -1))


def wrap16_cols_batch(x, K):
    """[2,K,H,W] -> [128, NS*M*K] (partition 16g + j%16, col (j//16)*K + k)."""
    a = x.reshape(2, K, 20, BH, 2, CW)
    a = a.transpose(0, 2, 4, 1, 3, 5).reshape(2, 40, K, N)
    a = a.reshape(2, 4, 10, K, M, 16)           # b g4 s K m plo
    a = a.transpose(0, 1, 5, 2, 4, 3)           # b g4 plo s m K
    # partition = 16*(b*4+g4) + plo
    return np.ascontiguousarray(a.reshape(128, NS, M, K).reshape(128, -1))


def host_precompute_all(pose_twist, I0, I1, invD0, invD1, intr):
    """Vectorized over all B=16; returns per-core input dicts + T0 per core."""
    T0 = se3_exp(pose_twist)
    fx = intr[:, 0][:, None, None]; fy = intr[:, 1][:, None, None]
    cx = intr[:, 2][:, None, None]; cy = intr[:, 3][:, None, None]
    uu = np.arange(W, dtype=np.float32)[None, None, :]
    vv = np.arange(H, dtype=np.float32)[None, :, None]
    iD = np.maximum(invD1[:, 0], EPS).astype(np.float32)
    z1 = (1.0 / iD).astype(np.float32)
    xn = ((uu - cx) / fx).astype(np.float32)     # [B,1,W]
    yn = ((vv - cy) / fy).astype(np.float32)     # [B,H,1]
    x1 = xn * z1
    y1 = yn * z1
    dI0x, dI0y = feature_gradient(I0)
    dD0x, dD0y = feature_gradient(invD0)
    planes12 = np.concatenate([dI0x, dI0y, dD0x, dD0y, I0, invD0], axis=1).astype(np.float16)
    flat = planes12.reshape(B, 12, HW)
    pds = np.zeros((B, 12, HW + 2), np.float16)
    pds[:, :, 1:HW + 1] = flat

    X1 = np.stack([x1, y1, z1], 1).astype(np.float16)       # [B, 3, H, W]
    I1f = np.asarray(I1, np.float32)

    bw = np.zeros((128, NS, 4), np.float32)
    for g in range(8):
        for s in range(NS):
            _, yb, xh2 = chunk_of(g, s)
            rbase, cbase = bases_of(yb, xh2)
            bw[16 * g:16 * g + 16, s, 0] = rbase
            bw[16 * g:16 * g + 16, s, 1] = cbase - 1          # xf min
            bw[16 * g:16 * g + 16, s, 2] = cbase + (TC - 2)   # xf max
            bw[16 * g:16 * g + 16, s, 3] = 1 - cbase          # kx offset
    bw = np.ascontiguousarray(bw.reshape(128, NS * 4))
    idn = np.eye(128, dtype=np.float16)

    I1h = I1f.astype(np.float16)
    core_inputs, T0s = [], []
    for core in range(8):
        sl = slice(2 * core, 2 * core + 2)
        inp = {}
        inp["pds"] = np.ascontiguousarray(pds[sl])
        inp["x1m"] = mod128_cols_batch(X1[sl])
        inp["x1w"] = wrap16_cols_batch(X1[sl], 3)
        inp["i1m"] = mod128_cols_batch(I1h[sl])
        inp["bw"] = bw
        inp["idn"] = idn
        q = np.zeros((2, 16), np.float32)
        q[:, :9] = T0[sl, :3, :3].reshape(2, 9)
        q[:, 9:12] = T0[sl, :3, 3]
        q[:, 12:16] = intr[sl]
        rtm = np.zeros((128, 16, 8), np.float32)
        rtw = np.zeros((128, 16), np.float32)
        for g in range(8):
            bb = g // 4
            rtm[:, :, g] = q[bb][None, :]
            rtw[16 * g:16 * g + 16, :] = q[bb][None, :]
        inp["rtm"] = np.ascontiguousarray(rtm.reshape(128, 16 * 8))
        inp["rtw"] = rtw
        inp["t0q"] = np.ascontiguousarray(T0[sl].reshape(2, 16).astype(np.float32))
        inp["intr2"] = np.ascontiguousarray(intr[sl].astype(np.float32))
        core_inputs.append(inp)
        T0s.append(T0[sl])
    return core_inputs, T0s


_NC_CACHE = {}
PROFILE = False
LAST_EXEC_NS = []
LAST_TRACES = []
LAST_WALL = []


def build_nc():
    import concourse.bacc as bacc
    import concourse.bass as bass
    import concourse.tile as tile
    from concourse import mybir

    fp32 = mybir.dt.float32
    fp16 = mybir.dt.float16
    i16 = mybir.dt.int16
    i32 = mybir.dt.int32
    AL = mybir.AluOpType
    ACT = mybir.ActivationFunctionType
    AX = mybir.AxisListType

    nc = bacc.Bacc("TRN2", target_bir_lowering=False, debug=False, num_devices=8)

    pd_in = nc.dram_tensor("pds", [2, 12, HW + 2], fp16, kind="ExternalInput")
    x1m_in = nc.dram_tensor("x1m", [128, NS * 8 * A * 3], fp16, kind="ExternalInput")
    x1w_in = nc.dram_tensor("x1w", [128, NS * M * 3], fp16, kind="ExternalInput")
    i1m_in = nc.dram_tensor("i1m", [128, NS * 8 * A * 3], fp16, kind="ExternalInput")
    bw_in = nc.dram_tensor("bw", [128, NS * 4], fp32, kind="ExternalInput")
    idn_in = nc.dram_tensor("idn", [128, 128], fp16, kind="ExternalInput")
    rtm_in = nc.dram_tensor("rtm", [128, 16 * 8], fp32, kind="ExternalInput")
    rtw_in = nc.dram_tensor("rtw", [128, 16], fp32, kind="ExternalInput")
    t0_in = nc.dram_tensor("t0q", [2, 16], fp32, kind="ExternalInput")
    intr_in = nc.dram_tensor("intr2", [2, 4], fp32, kind="ExternalInput")
    tout_ext = nc.dram_tensor("tout", [2, 16], fp32, kind="ExternalOutput")
    qscr = nc.dram_tensor("qscr", [2, 16], fp32, kind="Internal")

    with tile.TileContext(nc) as tc:
        with tc.tile_pool(name="cst", bufs=1) as cpool, \
             tc.tile_pool(name="tblp", bufs=1) as tpool, \
             tc.tile_pool(name="strm", bufs=2) as sp, \
             tc.tile_pool(name="scr", bufs=1) as sc, \
             tc.tile_pool(name="gath", bufs=1) as gp, \
             tc.tile_pool(name="ps", bufs=2, space="PSUM") as pp, \
             tc.tile_pool(name="jp", bufs=1, space="PSUM") as jp:

            rtm = cpool.tile([128, 16 * 8], fp32, tag="rtm")
            rtm0 = cpool.tile([128, 16 * 8], fp32, tag="rtm0")
            rtw = cpool.tile([128, 16], fp32, tag="rtw")
            bwc = cpool.tile([128, NS * 4], fp32, tag="bw")
            idn = cpool.tile([128, 128], fp16, tag="idn")
            Tq = cpool.tile([2, 16], fp32, tag="Tq")
            intr = cpool.tile([2, 4], fp32, tag="intr")
            nc.sync.dma_start(out=rtm[:, :], in_=rtm_in.ap())
            nc.sync.dma_start(out=rtm0[:, :], in_=rtm_in.ap())
            nc.sync.dma_start(out=rtw[:, :], in_=rtw_in.ap())
            nc.sync.dma_start(out=bwc[:, :], in_=bw_in.ap())
            nc.sync.dma_start(out=idn[:, :], in_=idn_in.ap())
            nc.sync.dma_start(out=Tq[:, :], in_=t0_in.ap())
            nc.sync.dma_start(out=intr[:, :], in_=intr_in.ap())

            psJ = [jp.tile([28, 28], fp32, name=f"psJ{b}", tag=f"psJ{b}") for b in range(2)]

            tbl0 = tpool.tile([128, NELEM * 2], fp16, tag="tbl")
            nc.vector.memset(tbl0[:, :], 0.0)
            stbl0 = tpool.tile([128, 34 * (TC + 1)], fp16, tag="stbl")
            nc.vector.memset(stbl0[:, :], 0.0)

            def rq(qi):   # mod-128 per-group broadcast: dims (g x8, a x A step0)
                sl = rtm[:, qi * 8:(qi + 1) * 8]
                return bass.AP(sl.tensor, sl.offset, [list(sl.ap[0]), [1, 8], [0, A]])

            def rqw(qi):  # wrapped per-partition scalar bcast over M
                sl = rtw[:, qi:qi + 1]
                return bass.AP(sl.tensor, sl.offset, [list(sl.ap[0]), [0, M]])

            def rtwS(qi):  # wrapped per-partition scalar [128,1]
                return rtw[:, qi:qi + 1]

            def bwq(s, j):
                sl = bwc[:, s * 4 + j:s * 4 + j + 1]
                return bass.AP(sl.tensor, sl.offset, [list(sl.ap[0]), [0, M]])

            def bwS(s, j):
                return bwc[:, s * 4 + j:s * 4 + j + 1]

            TT = nc.vector.tensor_tensor
            TS = lambda out, in0, s1, op: nc.vector.tensor_scalar(out, in0, s1, None, op)
            TS2 = lambda out, in0, s1, s2, op0, op1: nc.vector.tensor_scalar(out, in0, s1, s2, op0, op1)
            STT = nc.vector.scalar_tensor_tensor

            with tc.For_i(0, N_ITERS) as _it:
                for s in range(NS):
                    tbl = tbl0
                    for r0, nr in ((0, 34), (34, 33)):
                        for g in range(8):
                            b, yb, xh = chunk_of(g, s)
                            rbase, cbase = bases_of(yb, xh)
                            start = (rbase + r0) * W + cbase
                            src0 = pd_in.ap()
                            src = bass.AP(src0.tensor,
                                          src0.offset + b * 12 * (HW + 2) + start,
                                          [[HW + 2, 12], [W, nr], [1, TC + 1]])
                            dsl = stbl0[16 * g:16 * g + 12, :]
                            dst = bass.AP(dsl.tensor, dsl.offset,
                                          [[dsl.ap[0][0], 12], [TC + 1, nr], [1, TC + 1]])
                            nc.sync.dma_start(out=dst, in_=src)
                        for e in range(2):
                            pout = bass.AP(tbl.tensor, tbl.offset + e + r0 * 2 * TC,
                                           [list(tbl.ap[0]), [2 * TC, nr], [2, TC]])
                            pin = bass.AP(stbl0.tensor, stbl0.offset + e,
                                          [list(stbl0.ap[0]), [TC + 1, nr], [1, TC]])
                            nc.scalar.activation(pout, pin, ACT.Copy)

                    x1w = sp.tile([128, M * 3], fp16, tag="x1w")
                    nc.sync.dma_start(out=x1w[:, :], in_=x1w_in.ap()[:, s * M * 3:(s + 1) * M * 3])
                    x1m = sp.tile([128, 8 * A * 3], fp16, tag="x1m")
                    nc.sync.dma_start(out=x1m[:, :], in_=x1m_in.ap()[:, s * 8 * A * 3:(s + 1) * 8 * A * 3])
                    i1 = sp.tile([128, 8 * A * 3], fp16, tag="i1")
                    nc.sync.dma_start(out=i1[:, :], in_=i1m_in.ap()[:, s * 8 * A * 3:(s + 1) * 8 * A * 3])

                    # ---------- wrapped-16 idx pipeline ----------
                    def xw(k):
                        sl = x1w[:, :]
                        return bass.AP(sl.tensor, sl.offset + k, [list(sl.ap[0]), [3, M]])

                    def tw(name):
                        return sc.tile([128, M], fp32, name="w_" + name + f"_{s}", tag="w_" + name)

                    t1w = tw("t1")
                    X0zw = tw("X0z")
                    STT(X0zw[:, :], xw(0), rtwS(6), rqw(11), AL.mult, AL.add)
                    STT(X0zw[:, :], xw(1), rtwS(7), X0zw[:, :], AL.mult, AL.add)
                    STT(X0zw[:, :], xw(2), rtwS(8), X0zw[:, :], AL.mult, AL.add)
                    X0xw = tw("X0x")
                    STT(X0xw[:, :], xw(0), rtwS(0), rqw(9), AL.mult, AL.add)
                    STT(X0xw[:, :], xw(1), rtwS(1), X0xw[:, :], AL.mult, AL.add)
                    STT(X0xw[:, :], xw(2), rtwS(2), X0xw[:, :], AL.mult, AL.add)
                    X0yw = tw("X0y")
                    STT(X0yw[:, :], xw(0), rtwS(3), rqw(10), AL.mult, AL.add)
                    STT(X0yw[:, :], xw(1), rtwS(4), X0yw[:, :], AL.mult, AL.add)
                    STT(X0yw[:, :], xw(2), rtwS(5), X0yw[:, :], AL.mult, AL.add)

                    izw = tw("iz")
                    TS(t1w[:, :], X0zw[:, :], EPS, AL.max)
                    nc.vector.reciprocal_approx_fast(izw[:, :], t1w[:, :])
                    u0w = tw("u0"); v0w = tw("v0")
                    TT(u0w[:, :], X0xw[:, :], izw[:, :], op=AL.mult)
                    STT(u0w[:, :], u0w[:, :], rtwS(12), rqw(14), AL.mult, AL.add)
                    TT(v0w[:, :], X0yw[:, :], izw[:, :], op=AL.mult)
                    STT(v0w[:, :], v0w[:, :], rtwS(13), rqw(15), AL.mult, AL.add)
                    TS2(u0w[:, :], u0w[:, :], -0.5 * (W - 1), 1.5 * (W - 1), AL.max, AL.min)
                    TS2(v0w[:, :], v0w[:, :], -0.5 * (H - 1), 1.5 * (H - 1), AL.max, AL.min)
                    x0fw = tw("x0f"); y0fw = tw("y0f")
                    fi32w = sc.tile([128, M], i32, name=f"fi32w_{s}", tag="fi32w")
                    TS(t1w[:, :], u0w[:, :], 0.5, AL.subtract)
                    nc.vector.tensor_copy(fi32w[:, :], t1w[:, :])
                    nc.vector.tensor_copy(x0fw[:, :], fi32w[:, :])
                    TS(t1w[:, :], v0w[:, :], 0.5, AL.subtract)
                    nc.vector.tensor_copy(fi32w[:, :], t1w[:, :])
                    nc.vector.tensor_copy(y0fw[:, :], fi32w[:, :])
                    xfw = t1w; kxw = izw; yrw = X0zw
                    ktw = X0xw; kbw = X0yw
                    STT(xfw[:, :], x0fw[:, :], bwS(s, 1), bwq(s, 2), AL.max, AL.min)
                    nc.vector.tensor_scalar(kxw[:, :], xfw[:, :], bwS(s, 3), None, AL.add)
                    nc.vector.tensor_scalar(yrw[:, :], y0fw[:, :], bwS(s, 0), 0.0, AL.subtract, AL.max)
                    TS2(ktw[:, :], yrw[:, :], float(TR - 1), float(TC), AL.min, AL.mult)
                    TT(ktw[:, :], ktw[:, :], kxw[:, :], op=AL.add)
                    TS2(kbw[:, :], yrw[:, :], 1.0, float(TR - 1), AL.add, AL.min)
                    TS(kbw[:, :], kbw[:, :], float(TC), AL.mult)
                    TT(kbw[:, :], kbw[:, :], kxw[:, :], op=AL.add)
                    kidx = sc.tile([128, 2 * M], i16, name=f"kidx_{s}", tag="kidx")
                    nc.vector.tensor_copy(kidx[:, :M], ktw[:, :])
                    nc.vector.tensor_copy(kidx[:, M:], kbw[:, :])

                    gt2 = gp.tile([128, 2 * N * 2], fp16, tag="gt2")
                    nc.gpsimd.ap_gather(gt2[:, :], tbl[:, :], kidx[:, :],
                                        channels=128, num_elems=NELEM, d=2, num_idxs=2 * N)

                    # ---------- mod-128 warp pipeline ----------
                    def xm(k):
                        sl = x1m[:, :]
                        return bass.AP(sl.tensor, sl.offset + k, [list(sl.ap[0]), [3, 8 * A]])

                    def tm(name):
                        return sc.tile([128, 8 * A], fp32, name="m_" + name + f"_{s}", tag="m_" + name)

                    def matvec(dst, aps, qis, t1):
                        TT(dst[:, :], aps[0], qis[0], op=AL.mult)
                        TT(t1[:, :], aps[1], qis[1], op=AL.mult)
                        TT(dst[:, :], dst[:, :], t1[:, :], op=AL.add)
                        TT(t1[:, :], aps[2], qis[2], op=AL.mult)
                        TT(dst[:, :], dst[:, :], t1[:, :], op=AL.add)
                        TT(dst[:, :], dst[:, :], qis[3], op=AL.add)

                    # ---- on-device A6/B6/T6 at the initial pose (rtm0) ----
                    abt = sc.tile([128, 8 * A * 18], fp16, name=f"abt_{s}", tag="abt")

                    def acol(k):
                        sl = abt[:, :]
                        return bass.AP(sl.tensor, sl.offset + k, [list(sl.ap[0]), [18, 224]])

                    def rq0(qi):
                        sl = rtm0[:, qi * 8:(qi + 1) * 8]
                        return bass.AP(sl.tensor, sl.offset, [list(sl.ap[0]), [1, 8], [0, A]])

                    j1 = tm("j1"); j2 = tm("j2")
                    jx = tm("jx"); jy = tm("jy"); jz = tm("jz"); jiz = tm("jiz")
                    matvec(jz, [xm(0), xm(1), xm(2)], [rq0(6), rq0(7), rq0(8), rq0(11)], j1)
                    matvec(jx, [xm(0), xm(1), xm(2)], [rq0(0), rq0(1), rq0(2), rq0(9)], j1)
                    matvec(jy, [xm(0), xm(1), xm(2)], [rq0(3), rq0(4), rq0(5), rq0(10)], j1)
                    TS(j1[:, :], jz[:, :], EPS, AL.max)
                    nc.vector.reciprocal_approx_fast(jiz[:, :], j1[:, :])
                    fxiz = tm("fxiz"); fyiz = tm("fyiz"); zizt = tm("zizt")
                    A2t = tm("A2t"); B2t = tm("B2t")
                    TT(fxiz[:, :], jiz[:, :], rq0(12), op=AL.mult)
                    TT(fyiz[:, :], jiz[:, :], rq0(13), op=AL.mult)
                    TT(zizt[:, :], jz[:, :], jiz[:, :], op=AL.mult)
                    TT(j1[:, :], jx[:, :], jiz[:, :], op=AL.mult)
                    TT(A2t[:, :], fxiz[:, :], j1[:, :], op=AL.mult)
                    TT(j1[:, :], jy[:, :], jiz[:, :], op=AL.mult)
                    TT(B2t[:, :], fyiz[:, :], j1[:, :], op=AL.mult)
                    TS(acol(0), fxiz[:, :], -1.0, AL.mult)
                    TS(acol(1), fxiz[:, :], 0.0, AL.mult)
                    nc.vector.tensor_copy(acol(2), A2t[:, :])
                    TT(acol(3), A2t[:, :], jy[:, :], op=AL.mult)
                    TT(j1[:, :], zizt[:, :], rq0(12), op=AL.mult)
                    TT(j2[:, :], A2t[:, :], jx[:, :], op=AL.mult)
                    TT(j1[:, :], j1[:, :], j2[:, :], op=AL.add)
                    TS(acol(4), j1[:, :], -1.0, AL.mult)
                    TT(acol(5), fxiz[:, :], jy[:, :], op=AL.mult)
                    TS(acol(6), fxiz[:, :], 0.0, AL.mult)
                    TS(acol(7), fyiz[:, :], -1.0, AL.mult)
                    nc.vector.tensor_copy(acol(8), B2t[:, :])
                    TT(j1[:, :], zizt[:, :], rq0(13), op=AL.mult)
                    TT(j2[:, :], B2t[:, :], jy[:, :], op=AL.mult)
                    TT(acol(9), j1[:, :], j2[:, :], op=AL.add)
                    TT(j1[:, :], B2t[:, :], jx[:, :], op=AL.mult)
                    TS(acol(10), j1[:, :], -1.0, AL.mult)
                    TT(j1[:, :], fyiz[:, :], jx[:, :], op=AL.mult)
                    TS(acol(11), j1[:, :], -1.0, AL.mult)
                    TS(acol(12), fxiz[:, :], 0.0, AL.mult)
                    TS(acol(13), fxiz[:, :], 0.0, AL.mult)
                    TS2(acol(14), fxiz[:, :], 0.0, 1.0, AL.mult, AL.add)
                    nc.vector.tensor_copy(acol(15), jy[:, :])
                    TS(acol(16), jx[:, :], -1.0, AL.mult)
                    TS(acol(17), fxiz[:, :], 0.0, AL.mult)

                    m1 = j1; m2 = j2
                    X0z = jz
                    matvec(X0z, [xm(0), xm(1), xm(2)], [rq(6), rq(7), rq(8), rq(11)], m1)
                    X0x = jx
                    matvec(X0x, [xm(0), xm(1), xm(2)], [rq(0), rq(1), rq(2), rq(9)], m1)
                    X0y = jy
                    matvec(X0y, [xm(0), xm(1), xm(2)], [rq(3), rq(4), rq(5), rq(10)], m1)
                    iz = jiz
                    TS(m1[:, :], X0z[:, :], EPS, AL.max)
                    nc.vector.reciprocal_approx_fast(iz[:, :], m1[:, :])
                    u0 = fxiz; v0 = fyiz
                    TT(u0[:, :], X0x[:, :], iz[:, :], op=AL.mult)
                    TT(u0[:, :], u0[:, :], rq(12), op=AL.mult)
                    TT(u0[:, :], u0[:, :], rq(14), op=AL.add)
                    TT(v0[:, :], X0y[:, :], iz[:, :], op=AL.mult)
                    TT(v0[:, :], v0[:, :], rq(13), op=AL.mult)
                    TT(v0[:, :], v0[:, :], rq(15), op=AL.add)
                    vmask = zizt
                    TS(vmask[:, :], X0z[:, :], EPS, AL.is_gt)
                    STT(vmask[:, :], u0[:, :], 0.0, vmask[:, :], AL.is_gt, AL.mult)
                    STT(vmask[:, :], u0[:, :], float(W - 1), vmask[:, :], AL.is_lt, AL.mult)
                    STT(vmask[:, :], v0[:, :], 0.0, vmask[:, :], AL.is_gt, AL.mult)
                    STT(vmask[:, :], v0[:, :], float(H - 1), vmask[:, :], AL.is_lt, AL.mult)
                    TS2(u0[:, :], u0[:, :], -0.5 * (W - 1), 1.5 * (W - 1), AL.max, AL.min)
                    TS2(v0[:, :], v0[:, :], -0.5 * (H - 1), 1.5 * (H - 1), AL.max, AL.min)
                    wx = A2t; wy = B2t; x0f = tm("x0f"); y0f = tm("y0f")
                    fi32m = sc.tile([128, 8 * A], i32, name=f"fi32m_{s}", tag="fi32m")
                    TS(m1[:, :], u0[:, :], 0.5, AL.subtract)
                    nc.vector.tensor_copy(fi32m[:, :], m1[:, :])
                    nc.vector.tensor_copy(x0f[:, :], fi32m[:, :])
                    TT(wx[:, :], u0[:, :], x0f[:, :], op=AL.subtract)
                    TS(m1[:, :], v0[:, :], 0.5, AL.subtract)
                    nc.vector.tensor_copy(fi32m[:, :], m1[:, :])
                    nc.vector.tensor_copy(y0f[:, :], fi32m[:, :])
                    TT(wy[:, :], v0[:, :], y0f[:, :], op=AL.subtract)
                    mx0 = tm("mx0"); mx1 = tm("mx1"); my0 = tm("my0"); my1 = tm("my1")
                    TS(mx0[:, :], x0f[:, :], -0.5, AL.is_gt)
                    STT(mx0[:, :], x0f[:, :], float(W - 1) + 0.5, mx0[:, :], AL.is_lt, AL.mult)
                    TS(mx1[:, :], x0f[:, :], -1.5, AL.is_gt)
                    STT(mx1[:, :], x0f[:, :], float(W - 2) + 0.5, mx1[:, :], AL.is_lt, AL.mult)
                    TS(my0[:, :], y0f[:, :], -0.5, AL.is_gt)
                    STT(my0[:, :], y0f[:, :], float(H - 1) + 0.5, my0[:, :], AL.is_lt, AL.mult)
                    TS(my1[:, :], y0f[:, :], -1.5, AL.is_gt)
                    STT(my1[:, :], y0f[:, :], float(H - 2) + 0.5, my1[:, :], AL.is_lt, AL.mult)
                    W00 = tm("W00"); W01 = tm("W01"); W10 = tm("W10"); W11 = tm("W11")
                    TS2(m1[:, :], wx[:, :], 1.0, -1.0, AL.subtract, AL.mult)  # 1-wx
                    TS2(m2[:, :], wy[:, :], 1.0, -1.0, AL.subtract, AL.mult)  # 1-wy
                    TT(W00[:, :], m1[:, :], m2[:, :], op=AL.mult)
                    TT(W00[:, :], W00[:, :], mx0[:, :], op=AL.mult)
                    TT(W00[:, :], W00[:, :], my0[:, :], op=AL.mult)
                    TT(W01[:, :], wx[:, :], m2[:, :], op=AL.mult)
                    TT(W01[:, :], W01[:, :], mx1[:, :], op=AL.mult)
                    TT(W01[:, :], W01[:, :], my0[:, :], op=AL.mult)
                    TT(W10[:, :], m1[:, :], wy[:, :], op=AL.mult)
                    TT(W10[:, :], W10[:, :], mx0[:, :], op=AL.mult)
                    TT(W10[:, :], W10[:, :], my1[:, :], op=AL.mult)
                    TT(W11[:, :], wx[:, :], wy[:, :], op=AL.mult)
                    TT(W11[:, :], W11[:, :], mx1[:, :], op=AL.mult)
                    TT(W11[:, :], W11[:, :], my1[:, :], op=AL.mult)

                    # ---------- PE transpose + combine ----------
                    samp = sc.tile([128, A * 128], fp16, tag="samp")
                    ctmp = sc.tile([128, 512], fp16, tag="ctmp")
                    for a4 in range(A // 4):
                        ptall = pp.tile([128, 2048], fp16, tag="ptall")
                        for ci, base in enumerate((0, 1, 2 * N, 2 * N + 1)):
                            for aa in range(4):
                                a = a4 * 4 + aa
                                src = bass.AP(gt2.tensor, gt2.offset + base + a * 256,
                                              [list(gt2.ap[0]), [2, 128]])
                                nc.tensor.transpose(
                                    ptall[:, ci * 512 + aa * 128:ci * 512 + (aa + 1) * 128],
                                    src, idn[:, :])
                        for ci, wt_ in ((0, W00), (1, W01), (2, W10), (3, W11)):
                            pap = bass.AP(ptall.tensor, ptall.offset + ci * 512,
                                          [list(ptall.ap[0]), [128, 4], [16, 8], [1, 16]])
                            woff = wt_.offset + a4 * 4
                            wap = bass.AP(wt_.tensor, woff, [list(wt_.ap[0]), [1, 4], [A, 8], [0, 16]])
                            dst_off = samp.offset + a4 * 4 * 128
                            dap = bass.AP(samp.tensor, dst_off, [list(samp.ap[0]), [128, 4], [16, 8], [1, 16]])
                            if ci == 0:
                                TT(dap, pap, wap, op=AL.mult)
                            else:
                                tap = bass.AP(ctmp.tensor, ctmp.offset, [list(ctmp.ap[0]), [128, 4], [16, 8], [1, 16]])
                                TT(tap, pap, wap, op=AL.mult)
                                TT(dap, dap, tap, op=AL.add)

                    # ---------- residuals, huber weights, G build ----------
                    def sq(q):
                        sl = samp[:, :]
                        return bass.AP(sl.tensor, sl.offset + q, [list(sl.ap[0]), [16, 8], [128, A]])

                    def i1q(c):
                        sl = i1[:, :]
                        return bass.AP(sl.tensor, sl.offset + c, [list(sl.ap[0]), [3 * A, 8], [3, A]])

                    Gt = sc.tile([128, 28 * 224], fp16, tag="Gt")
                    g6a = sc.tile([128, 6 * 224], fp16, tag="g6a")
                    g6b = sc.tile([128, 6 * 224], fp16, tag="g6b")
                    one_m = tm("one_m")
                    TS2(one_m[:, :], vmask[:, :], 1.0, -1e-6, AL.subtract, AL.mult)  # (1-vm)*1e-6
                    rr = tm("rr"); bb_ = tm("bb"); ss = tm("ss")
                    ppv = tm("ppv"); qqv = tm("qqv")

                    def abt6(k0):  # [x(6) outer, chunk(224) inner], stride 18 per chunk
                        sl = abt[:, :]
                        return bass.AP(sl.tensor, sl.offset + k0, [list(sl.ap[0]), [1, 6], [18, 224]])

                    def gcols(c):  # G cols c*7 .. c*7+5: [x outer, chunk inner]
                        sl = Gt[:, :]
                        return bass.AP(sl.tensor, sl.offset + c * 7 * 224, [list(sl.ap[0]), [224, 6], [1, 224]])

                    def bc6(t):    # broadcast [128,224] over 6 x-cols
                        sl = t[:, :]
                        return bass.AP(sl.tensor, sl.offset, [list(sl.ap[0]), [0, 6], [1, 224]])

                    for c in range(3):
                        TT(rr[:, :], i1q(c), sq(8 + c), op=AL.subtract)
                        TT(rr[:, :], rr[:, :], vmask[:, :], op=AL.mult)
                        TT(rr[:, :], rr[:, :], one_m[:, :], op=AL.add)
                        nc.scalar.activation(bb_[:, :], rr[:, :], ACT.Abs)
                        TS(bb_[:, :], bb_[:, :], HUBER_DELTA, AL.max)
                        nc.vector.reciprocal_approx_fast(bb_[:, :], bb_[:, :])
                        nc.scalar.activation(ss[:, :], bb_[:, :], ACT.Sqrt, scale=HUBER_DELTA)
                        TT(ppv[:, :], ss[:, :], sq(0 + c), op=AL.mult)
                        TT(qqv[:, :], ss[:, :], sq(3 + c), op=AL.mult)
                        TT(g6a[:, :], abt6(0), bc6(ppv), op=AL.mult)
                        TT(g6b[:, :], abt6(6), bc6(qqv), op=AL.mult)
                        TT(gcols(c), g6a[:, :], g6b[:, :], op=AL.add)
                        TT(Gt[:, (c * 7 + 6) * 224:(c * 7 + 7) * 224], ss[:, :], rr[:, :], op=AL.mult)
                    # depth channel
                    TT(rr[:, :], iz[:, :], sq(11), op=AL.subtract)
                    TT(rr[:, :], rr[:, :], vmask[:, :], op=AL.mult)
                    TT(rr[:, :], rr[:, :], one_m[:, :], op=AL.add)
                    nc.scalar.activation(bb_[:, :], rr[:, :], ACT.Abs, scale=LAMBDA)
                    TS(bb_[:, :], bb_[:, :], HUBER_DELTA, AL.max)
                    nc.vector.reciprocal_approx_fast(bb_[:, :], bb_[:, :])
                    nc.scalar.activation(ss[:, :], bb_[:, :], ACT.Sqrt,
                                         scale=HUBER_DELTA * LAMBDA * LAMBDA)
                    TT(ppv[:, :], ss[:, :], sq(6), op=AL.mult)
                    TT(qqv[:, :], ss[:, :], sq(7), op=AL.mult)
                    TT(g6a[:, :], abt6(0), bc6(ppv), op=AL.mult)
                    TT(g6b[:, :], abt6(6), bc6(qqv), op=AL.mult)
                    TT(g6a[:, :], g6a[:, :], g6b[:, :], op=AL.add)
                    TT(g6b[:, :], abt6(12), bc6(ss), op=AL.mult)
                    TT(gcols(3), g6a[:, :], g6b[:, :], op=AL.add)
                    TT(Gt[:, (3 * 7 + 6) * 224:(3 * 7 + 7) * 224], ss[:, :], rr[:, :], op=AL.mult)

                    # ---------- PE: JtWJ accumulation ----------
                    for g in range(8):
                        b = g // 4
                        for a in range(A):
                            off = Gt.offset + g * A + a
                            gap = bass.AP(Gt.tensor, off, [list(Gt.ap[0]), [224, 28]])
                            first = (s == 0 and (g % 4) == 0 and a == 0)
                            last = (s == NS - 1 and (g % 4) == 3 and a == A - 1)
                            nc.tensor.matmul(psJ[b][:, :], gap, gap,
                                             start=first, stop=last,
                                             skip_group_check=True)

                # ---------- per-iteration: extract JtWJ/Rhs, solve, update pose ----------
                S28 = sc.tile([28, 56], fp32, tag="S28")
                for b in range(2):
                    nc.vector.tensor_copy(S28[:, b * 28:(b + 1) * 28], psJ[b][:, :])
                D28 = sc.tile([7, 56], fp32, tag="D28")
                for b in range(2):
                    for c in range(4):
                        src = S28[c * 7:(c + 1) * 7, b * 28 + c * 7:b * 28 + c * 7 + 7]
                        dsl = D28[:, b * 28 + c * 7:b * 28 + c * 7 + 7]
                        nc.sync.dma_start(out=dsl, in_=src)
                M7 = sc.tile([7, 14], fp32, tag="M7")
                for b in range(2):
                    din = bass.AP(D28.tensor, D28.offset + b * 28,
                                  [list(D28.ap[0]), [1, 7], [7, 4]])
                    nc.vector.tensor_reduce(M7[:, b * 7:(b + 1) * 7], din, axis=AX.X, op=AL.add)
                # Mb [2, 49]: row b = M7 block b flattened (x-major)
                Mb = sc.tile([2, 49], fp32, tag="Mb")
                for b in range(2):
                    msrc = bass.AP(M7.tensor, M7.offset + b * 7, [[M7.ap[0][0], 7], [1, 7]])
                    mdsl = Mb[b:b + 1, 0:1]
                    mdst = bass.AP(mdsl.tensor, mdsl.offset, [[Mb.ap[0][0], 1], [7, 7], [1, 7]])
                    nc.sync.dma_start(out=mdst, in_=msrc)
                # tr = sum diag(JtWJ); LM ridge on diag
                trt = sc.tile([2, 1], fp32, tag="trt")
                diag = bass.AP(Mb.tensor, Mb.offset, [list(Mb.ap[0]), [8, 6]])
                nc.vector.tensor_reduce(trt[:, :], diag, axis=AX.X, op=AL.add)
                trb = bass.AP(trt.tensor, trt.offset, [list(trt.ap[0]), [0, 6]])
                STT(diag, trb, 1e-6, diag, AL.mult, AL.add)

                # Cholesky LL^T = Hm (6x6, both batches in 2 partitions)
                Lt = sc.tile([2, 36], fp32, tag="Lt")
                lsrc = bass.AP(Mb.tensor, Mb.offset, [list(Mb.ap[0]), [7, 6], [1, 6]])
                nc.vector.tensor_copy(Lt[:, :], lsrc)
                rhs = sc.tile([2, 6], fp32, tag="rhs")
                rsrc = bass.AP(Mb.tensor, Mb.offset + 6, [list(Mb.ap[0]), [7, 6]])
                nc.vector.tensor_copy(rhs[:, :], rsrc)
                idg = sc.tile([2, 6], fp32, tag="idg")
                tmpj = sc.tile([2, 36], fp32, tag="tmpj")
                red = sc.tile([2, 6], fp32, tag="redj")
                for j in range(6):
                    jj = Lt[:, 6 * j + j:6 * j + j + 1]
                    if j > 0:
                        ljk = Lt[:, 6 * j:6 * j + j]
                        TT(tmpj[:, :j], ljk, ljk, op=AL.mult)
                        nc.vector.tensor_reduce(red[:, 0:1], tmpj[:, :j], axis=AX.X, op=AL.add)
                        TT(jj, jj, red[:, 0:1], op=AL.subtract)
                    nc.scalar.activation(jj, jj, ACT.Sqrt)
                    nc.vector.reciprocal(idg[:, j:j + 1], jj)
                    nr = 5 - j
                    if nr > 0:
                        colap = bass.AP(Lt.tensor, Lt.offset + 6 * (j + 1) + j, [list(Lt.ap[0]), [6, nr]])
                        if j > 0:
                            lik = bass.AP(Lt.tensor, Lt.offset + 6 * (j + 1), [list(Lt.ap[0]), [6, nr], [1, j]])
                            ljkb = bass.AP(Lt.tensor, Lt.offset + 6 * j, [list(Lt.ap[0]), [0, nr], [1, j]])
                            TT(tmpj[:, :nr * j], lik, ljkb, op=AL.mult)
                            tin = bass.AP(tmpj.tensor, tmpj.offset, [list(tmpj.ap[0]), [j, nr], [1, j]])
                            nc.vector.tensor_reduce(red[:, :nr], tin, axis=AX.X, op=AL.add)
                            TT(colap, colap, red[:, :nr], op=AL.subtract)
                        nc.vector.tensor_scalar(colap, colap, idg[:, j:j + 1], None, AL.mult)
                # forward substitution: L y = rhs (in place on rhs)
                for j in range(6):
                    yj = rhs[:, j:j + 1]
                    if j > 0:
                        ljk = Lt[:, 6 * j:6 * j + j]
                        TT(tmpj[:, :j], ljk, rhs[:, :j], op=AL.mult)
                        nc.vector.tensor_reduce(red[:, 0:1], tmpj[:, :j], axis=AX.X, op=AL.add)
                        TT(yj, yj, red[:, 0:1], op=AL.subtract)
                    nc.vector.tensor_scalar(yj, yj, idg[:, j:j + 1], None, AL.mult)
                # back substitution: L^T x = y -> xi = -x stored in xi tile
                for j in range(5, -1, -1):
                    xj = rhs[:, j:j + 1]
                    nk = 5 - j
                    if nk > 0:
                        lkj = bass.AP(Lt.tensor, Lt.offset + 6 * (j + 1) + j, [list(Lt.ap[0]), [6, nk]])
                        TT(tmpj[:, :nk], lkj, rhs[:, j + 1:6], op=AL.mult)
                        nc.vector.tensor_reduce(red[:, 0:1], tmpj[:, :nk], axis=AX.X, op=AL.add)
                        TT(xj, xj, red[:, 0:1], op=AL.subtract)
                    nc.vector.tensor_scalar(xj, xj, idg[:, j:j + 1], None, AL.mult)
                xi = sc.tile([2, 6], fp32, tag="xi")
                TS(xi[:, :], rhs[:, :], -1.0, AL.mult)

                # se3_exp(xi) via Taylor series (|w| << 1 in this regime)
                w3 = xi[:, 3:6]
                wsq = sc.tile([2, 3], fp32, tag="wsq")
                TT(wsq[:, :], w3, w3, op=AL.mult)
                th2 = sc.tile([2, 1], fp32, tag="th2")
                nc.vector.tensor_reduce(th2[:, :], wsq[:, :], axis=AX.X, op=AL.add)
                coA = sc.tile([2, 1], fp32, tag="coA")
                coB = sc.tile([2, 1], fp32, tag="coB")
                coC = sc.tile([2, 1], fp32, tag="coC")
                hh = sc.tile([2, 1], fp32, tag="hh")
                TS2(hh[:, :], th2[:, :], 1.0 / 120.0, -1.0 / 6.0, AL.mult, AL.add)
                nc.vector.tensor_scalar(coA[:, :], th2[:, :], hh[:, :], 1.0, AL.mult, AL.add)
                TS2(hh[:, :], th2[:, :], 1.0 / 720.0, -1.0 / 24.0, AL.mult, AL.add)
                nc.vector.tensor_scalar(coB[:, :], th2[:, :], hh[:, :], 0.5, AL.mult, AL.add)
                TS2(hh[:, :], th2[:, :], 1.0 / 5040.0, -1.0 / 120.0, AL.mult, AL.add)
                nc.vector.tensor_scalar(coC[:, :], th2[:, :], hh[:, :], 1.0 / 6.0, AL.mult, AL.add)
                # K, K2
                Kt = sc.tile([2, 9], fp32, tag="Kt")
                nc.vector.memset(Kt[:, :], 0.0)
                TS(Kt[:, 1:2], xi[:, 5:6], -1.0, AL.mult)   # -z
                nc.vector.tensor_copy(Kt[:, 2:3], xi[:, 4:5])  # y
                nc.vector.tensor_copy(Kt[:, 3:4], xi[:, 5:6])  # z
                TS(Kt[:, 5:6], xi[:, 3:4], -1.0, AL.mult)   # -x
                TS(Kt[:, 6:7], xi[:, 4:5], -1.0, AL.mult)   # -y
                nc.vector.tensor_copy(Kt[:, 7:8], xi[:, 3:4])  # x
                K2t = sc.tile([2, 9], fp32, tag="K2t")
                wiap = bass.AP(xi.tensor, xi.offset + 3, [list(xi.ap[0]), [1, 3], [0, 3]])
                wjap = bass.AP(xi.tensor, xi.offset + 3, [list(xi.ap[0]), [0, 3], [1, 3]])
                TT(K2t[:, :], wiap, wjap, op=AL.mult)
                k2diag = bass.AP(K2t.tensor, K2t.offset, [list(K2t.ap[0]), [4, 3]])
                nc.vector.tensor_scalar(k2diag, k2diag, th2[:, :], None, AL.subtract)
                Rt = sc.tile([2, 9], fp32, tag="Rt")
                Vt = sc.tile([2, 9], fp32, tag="Vt")
                t9 = sc.tile([2, 9], fp32, tag="t9")
                nc.vector.tensor_scalar(Rt[:, :], Kt[:, :], coA[:, :], None, AL.mult)
                nc.vector.tensor_scalar(t9[:, :], K2t[:, :], coB[:, :], None, AL.mult)
                TT(Rt[:, :], Rt[:, :], t9[:, :], op=AL.add)
                rdiag = bass.AP(Rt.tensor, Rt.offset, [list(Rt.ap[0]), [4, 3]])
                TS(rdiag, rdiag, 1.0, AL.add)
                nc.vector.tensor_scalar(Vt[:, :], Kt[:, :], coB[:, :], None, AL.mult)
                nc.vector.tensor_scalar(t9[:, :], K2t[:, :], coC[:, :], None, AL.mult)
                TT(Vt[:, :], Vt[:, :], t9[:, :], op=AL.add)
                vdiag = bass.AP(Vt.tensor, Vt.offset, [list(Vt.ap[0]), [4, 3]])
                TS(vdiag, vdiag, 1.0, AL.add)
                # t = V @ v
                vbc = bass.AP(xi.tensor, xi.offset, [list(xi.ap[0]), [0, 3], [1, 3]])
                TT(t9[:, :], Vt[:, :], vbc, op=AL.mult)
                tv = sc.tile([2, 3], fp32, tag="tv")
                t9v = bass.AP(t9.tensor, t9.offset, [list(t9.ap[0]), [3, 3], [1, 3]])
                nc.vector.tensor_reduce(tv[:, :], t9v, axis=AX.X, op=AL.add)
                # E = [[R, t],[0,0,0,1]] as [2,16]
                Et = sc.tile([2, 16], fp32, tag="Et")
                nc.vector.memset(Et[:, :], 0.0)
                edst = bass.AP(Et.tensor, Et.offset, [list(Et.ap[0]), [4, 3], [1, 3]])
                esrc = bass.AP(Rt.tensor, Rt.offset, [list(Rt.ap[0]), [3, 3], [1, 3]])
                nc.vector.tensor_copy(edst, esrc)
                edst2 = bass.AP(Et.tensor, Et.offset + 3, [list(Et.ap[0]), [4, 3]])
                nc.vector.tensor_copy(edst2, tv[:, :])
                TS(Et[:, 15:16], Et[:, 15:16], 1.0, AL.add)
                # newT = T @ E
                nT = sc.tile([2, 16], fp32, tag="nT")
                for k in range(4):
                    tcol = bass.AP(Tq.tensor, Tq.offset + k, [list(Tq.ap[0]), [4, 4], [0, 4]])
                    erow = bass.AP(Et.tensor, Et.offset + 4 * k, [list(Et.ap[0]), [0, 4], [1, 4]])
                    if k == 0:
                        TT(nT[:, :], tcol, erow, op=AL.mult)
                    else:
                        TT(tmpj[:, :16], tcol, erow, op=AL.mult)
                        TT(nT[:, :], nT[:, :], tmpj[:, :16], op=AL.add)
                nc.vector.tensor_copy(Tq[:, :], nT[:, :])
                # rebuild q = [R(9) | t(3) | intr(4)] and broadcast to rtm/rtw
                qt = sc.tile([2, 16], fp32, tag="qt")
                qr = bass.AP(Tq.tensor, Tq.offset, [list(Tq.ap[0]), [4, 3], [1, 3]])
                nc.vector.tensor_copy(qt[:, 0:9], qr)
                qtcol = bass.AP(Tq.tensor, Tq.offset + 3, [list(Tq.ap[0]), [4, 3]])
                nc.vector.tensor_copy(qt[:, 9:12], qtcol)
                nc.vector.tensor_copy(qt[:, 12:16], intr[:, :])
                nc.sync.dma_start(out=qscr.ap(), in_=qt[:, :])
                qsap = qscr.ap()
                for b in range(2):
                    qsrc = bass.AP(qsap.tensor, qsap.offset + b * 16, [[0, 64], [1, 16]])
                    nc.sync.dma_start(out=rtw[b * 64:(b + 1) * 64, :], in_=qsrc)
                for g in range(8):
                    b = g // 4
                    qsrc = bass.AP(qsap.tensor, qsap.offset + b * 16, [[0, 128], [1, 16]])
                    rdst = bass.AP(rtm.tensor, rtm.offset + g, [list(rtm.ap[0]), [8, 16]])
                    nc.sync.dma_start(out=rdst, in_=qsrc)

            nc.sync.dma_start(out=tout_ext.ap(), in_=Tq[:, :])

    nc.finalize()
    return nc


def kernel(pose_twist, I0, I1, invD0, invD1, intrinsics):
    from concourse.bass_utils import run_bass_kernel_spmd

    nc = _NC_CACHE.get("nc")
    if nc is None:
        nc = build_nc()
        _NC_CACHE["nc"] = nc

    pose_twist = np.asarray(pose_twist, np.float32)
    I0 = np.asarray(I0, np.float32); I1 = np.asarray(I1, np.float32)
    invD0 = np.asarray(invD0, np.float32); invD1 = np.asarray(invD1, np.float32)
    intrinsics = np.asarray(intrinsics, np.float32)

    import time as _time
    LAST_WALL.clear(); LAST_EXEC_NS.clear(); LAST_TRACES.clear()
    t0 = _time.time()
    in_maps, _ = host_precompute_all(pose_twist, I0, I1, invD0, invD1, intrinsics)
    t1 = _time.time()
    res = run_bass_kernel_spmd(nc, in_maps, list(range(8)), trace=PROFILE)
    t2 = _time.time()
    LAST_WALL.extend([round(t1 - t0, 3), round(t2 - t1, 3)])
    if PROFILE:
        if res.exec_time_ns is not None:
            LAST_EXEC_NS.append(res.exec_time_ns)
        if res.instructions_and_trace is not None:
            LAST_TRACES.append(res.instructions_and_trace[1])

    outs = []
    for core in range(8):
        outs.append(res.results[core]["tout"].reshape(2, 4, 4))
    return np.concatenate(outs, axis=0).astype(np.float32)


# revision 45
# speedup vs baseline: 1.5375x; 1.2030x over previous
"""Trainium2 Bass kernel for nn_InvDirectImageAlign (inverse-compositional image alignment).

v3: ONE compiled NEFF runs all 5 Gauss-Newton iterations on device
(hardware For_i loop). Per core: 2 batch elements. Device does warp,
bilinear grid_sample (GPSIMD ap_gather from fp16 pair-dup band tables),
the JtWJ/Rhs normal equations via TensorEngine matmuls of a per-pixel
fp16 factor matrix G (JtWJ = sum_c G_c^T G_c), the 6x6 Cholesky solve,
se3_exp (Taylor series - angles are <<1 here) and the pose composition.
Inputs upload once; output is just the final 4x4 poses.

Chunking: (batch, 16-row y-band, 224-col x-half) = 80 chunks/core; the 8
GPSIMD partition-groups each own one chunk per superstep; 10 supersteps.
Two pixel layouts, bridged only by PE transposes of gathered data:
  mod-128:    pixel j of chunk(g,s) at partition j%128, free col (g, j//128)
  wrapped-16: pixel j at partition 16g + j%16, free col j//16   (ap_gather's
              index layout)
"""
import numpy as np

B, C, H, W = 16, 3, 320, 448
HW = H * W
N_ITERS = 5
LAMBDA = 0.01
HUBER_DELTA = 0.1
EPS = 1e-6

BH = 16            # band rows per chunk
CW = 224           # band cols per chunk
N = BH * CW        # 3584 px per chunk
A = N // 128       # 28
M = N // 16        # 224
NS = 10            # supersteps
TR = 67            # table rows (16 + 25 + 26)
TC = 266           # table cols (224 + 20 + 21 + 1)
NELEM = TR * TC    # 17822 pairs
YPAD = 25
XPAD = 20


def skew3(w):
    x, y, z = w[..., 0], w[..., 1], w[..., 2]
    O = np.zeros_like(x)
    return np.stack([np.stack([O, -z, y], -1),
                     np.stack([z, O, -x], -1),
                     np.stack([-y, x, O], -1)], -2)


def se3_exp(xi):
    xi = np.asarray(xi, np.float64)
    v, w = xi[:, :3], xi[:, 3:]
    th2 = np.sum(w * w, -1)[:, None, None]
    th2c = np.maximum(th2, 1e-16)
    th = np.sqrt(th2c)
    small = th2 < 1e-10
    Aa = np.where(small, 1.0 - th2 / 6.0, np.sin(th) / th)
    Bc = np.where(small, 0.5 - th2 / 24.0, (1.0 - np.cos(th)) / th2c)
    Cc = np.where(small, 1.0 / 6.0 - th2 / 120.0, (1.0 - Aa) / th2c)
    K = skew3(w)
    K2 = K @ K
    I = np.eye(3)
    R = I + Aa * K + Bc * K2
    V = I + Bc * K + Cc * K2
    t = np.einsum('bij,bj->bi', V, v)
    T = np.zeros((xi.shape[0], 4, 4))
    T[:, :3, :3] = R
    T[:, :3, 3] = t
    T[:, 3, 3] = 1.0
    return T.astype(np.float32)


def feature_gradient(img):
    p = np.pad(img, ((0, 0), (0, 0), (0, 0), (1, 1)), mode='edge')
    dx = 0.5 * (p[..., 2:] - p[..., :-2])
    p = np.pad(img, ((0, 0), (0, 0), (1, 1), (0, 0)), mode='edge')
    dy = 0.5 * (p[..., 2:, :] - p[..., :-2, :])
    return dx.astype(np.float32), dy.astype(np.float32)


def chunk_of(g, s):
    b = g // 4
    local = (g % 4) * 10 + s
    return b, local // 2, local % 2


def bases_of(yb, xh):
    r0, c0 = yb * BH, xh * CW
    rbase = int(np.clip(r0 - YPAD, 0, H - TR))
    cbase = int(np.clip(c0 - XPAD, 0, W - (TC - 1)))
    return rbase, cbase


def mod128_cols_batch(x):
    """[2,K,H,W] -> [128, NS*8*A*K] vectorized (one core's 2 batches)."""
    K = x.shape[1]
    # chunk (b, yb, xh): local = yb*2+xh; g = b*4 + local//10; s = local%10
    a = x.reshape(2, K, 20, BH, 2, CW)          # b K yb row xh col
    a = a.transpose(0, 2, 4, 1, 3, 5)           # b yb xh K row col
    a = a.reshape(2, 40, K, N)                  # local = yb*2+xh
    a = a.reshape(2, 4, 10, K, A, 128)          # b g4 s K a p
    a = a.transpose(5, 2, 0, 1, 4, 3)           # p s b g4 a K
    return np.ascontiguousarray(a.reshape(128, NS, 8, A, K).reshape(128, -1))


def wrap16_cols_batch(x, K):
    """[2,K,H,W] -> [128, NS*M*K] (partition 16g + j%16, col (j//16)*K + k)."""
    a = x.reshape(2, K, 20, BH, 2, CW)
    a = a.transpose(0, 2, 4, 1, 3, 5).reshape(2, 40, K, N)
    a = a.reshape(2, 4, 10, K, M, 16)           # b g4 s K m plo
    a = a.transpose(0, 1, 5, 2, 4, 3)           # b g4 plo s m K
    # partition = 16*(b*4+g4) + plo
    return np.ascontiguousarray(a.reshape(128, NS, M, K).reshape(128, -1))


def host_precompute_all(pose_twist, I0, I1, invD0, invD1, intr):
    """Vectorized over all B=16; returns per-core input dicts + T0 per core."""
    T0 = se3_exp(pose_twist)
    fx = intr[:, 0][:, None, None]; fy = intr[:, 1][:, None, None]
    cx = intr[:, 2][:, None, None]; cy = intr[:, 3][:, None, None]
    uu = np.arange(W, dtype=np.float32)[None, None, :]
    vv = np.arange(H, dtype=np.float32)[None, :, None]
    iD = np.maximum(invD1[:, 0], EPS).astype(np.float32)
    z1 = (1.0 / iD).astype(np.float32)
    xn = ((uu - cx) / fx).astype(np.float32)     # [B,1,W]
    yn = ((vv - cy) / fy).astype(np.float32)     # [B,H,1]
    x1 = xn * z1
    y1 = yn * z1
    # central-difference gradients with replicated edges, written pad-free
    # directly into the fp16 plane stack (same float ops as feature_gradient)
    P = np.empty((B, 12, H, W), np.float32)
    for dst, srcp, axis in ((0, I0, 'x'), (3, I0, 'y'),
                            (6, invD0, 'x'), (7, invD0, 'y')):
        nch = srcp.shape[1]
        v = P[:, dst:dst + nch]
        if axis == 'x':
            v[..., 1:-1] = 0.5 * (srcp[..., 2:] - srcp[..., :-2])
            v[..., 0] = 0.5 * (srcp[..., 1] - srcp[..., 0])
            v[..., -1] = 0.5 * (srcp[..., -1] - srcp[..., -2])
        else:
            v[..., 1:-1, :] = 0.5 * (srcp[..., 2:, :] - srcp[..., :-2, :])
            v[..., 0, :] = 0.5 * (srcp[..., 1, :] - srcp[..., 0, :])
            v[..., -1, :] = 0.5 * (srcp[..., -1, :] - srcp[..., -2, :])
    P[:, 8:11] = I0
    P[:, 11:12] = invD0
    pds = np.zeros((B, 12, HW + 2), np.float16)
    pds[:, :, 1:HW + 1] = P.reshape(B, 12, HW)

    X1 = np.stack([x1, y1, z1], 1).astype(np.float16)       # [B, 3, H, W]
    I1f = np.asarray(I1, np.float32)

    bw = np.zeros((128, NS, 4), np.float32)
    for g in range(8):
        for s in range(NS):
            _, yb, xh2 = chunk_of(g, s)
            rbase, cbase = bases_of(yb, xh2)
            bw[16 * g:16 * g + 16, s, 0] = rbase
            bw[16 * g:16 * g + 16, s, 1] = cbase - 1          # xf min
            bw[16 * g:16 * g + 16, s, 2] = cbase + (TC - 2)   # xf max
            bw[16 * g:16 * g + 16, s, 3] = 1 - cbase          # kx offset
    bw = np.ascontiguousarray(bw.reshape(128, NS * 4))
    idn = np.eye(128, dtype=np.float16)

    I1h = I1f.astype(np.float16)
    core_inputs, T0s = [], []
    for core in range(8):
        sl = slice(2 * core, 2 * core + 2)
        inp = {}
        inp["pds"] = np.ascontiguousarray(pds[sl])
        inp["x1m"] = mod128_cols_batch(X1[sl])
        inp["x1w"] = wrap16_cols_batch(X1[sl], 3)
        inp["i1m"] = mod128_cols_batch(I1h[sl])
        inp["bw"] = bw
        inp["idn"] = idn
        q = np.zeros((2, 16), np.float32)
        q[:, :9] = T0[sl, :3, :3].reshape(2, 9)
        q[:, 9:12] = T0[sl, :3, 3]
        q[:, 12:16] = intr[sl]
        rtm = np.zeros((128, 16, 8), np.float32)
        rtw = np.zeros((128, 16), np.float32)
        for g in range(8):
            bb = g // 4
            rtm[:, :, g] = q[bb][None, :]
            rtw[16 * g:16 * g + 16, :] = q[bb][None, :]
        inp["rtm"] = np.ascontiguousarray(rtm.reshape(128, 16 * 8))
        inp["rtw"] = rtw
        inp["t0q"] = np.ascontiguousarray(T0[sl].reshape(2, 16).astype(np.float32))
        inp["intr2"] = np.ascontiguousarray(intr[sl].astype(np.float32))
        core_inputs.append(inp)
        T0s.append(T0[sl])
    return core_inputs, T0s


_NC_CACHE = {}
PROFILE = False
LAST_EXEC_NS = []
LAST_TRACES = []
LAST_WALL = []


def build_nc():
    import concourse.bacc as bacc
    import concourse.bass as bass
    import concourse.tile as tile
    from concourse import mybir

    fp32 = mybir.dt.float32
    fp16 = mybir.dt.float16
    i16 = mybir.dt.int16
    i32 = mybir.dt.int32
    AL = mybir.AluOpType
    ACT = mybir.ActivationFunctionType
    AX = mybir.AxisListType

    nc = bacc.Bacc("TRN2", target_bir_lowering=False, debug=False, num_devices=8)

    pd_in = nc.dram_tensor("pds", [2, 12, HW + 2], fp16, kind="ExternalInput")
    x1m_in = nc.dram_tensor("x1m", [128, NS * 8 * A * 3], fp16, kind="ExternalInput")
    x1w_in = nc.dram_tensor("x1w", [128, NS * M * 3], fp16, kind="ExternalInput")
    i1m_in = nc.dram_tensor("i1m", [128, NS * 8 * A * 3], fp16, kind="ExternalInput")
    bw_in = nc.dram_tensor("bw", [128, NS * 4], fp32, kind="ExternalInput")
    idn_in = nc.dram_tensor("idn", [128, 128], fp16, kind="ExternalInput")
    rtm_in = nc.dram_tensor("rtm", [128, 16 * 8], fp32, kind="ExternalInput")
    rtw_in = nc.dram_tensor("rtw", [128, 16], fp32, kind="ExternalInput")
    t0_in = nc.dram_tensor("t0q", [2, 16], fp32, kind="ExternalInput")
    intr_in = nc.dram_tensor("intr2", [2, 4], fp32, kind="ExternalInput")
    tout_ext = nc.dram_tensor("tout", [2, 16], fp32, kind="ExternalOutput")
    qscr = nc.dram_tensor("qscr", [2, 16], fp32, kind="Internal")

    with tile.TileContext(nc) as tc:
        with tc.tile_pool(name="cst", bufs=1) as cpool, \
             tc.tile_pool(name="tblp", bufs=1) as tpool, \
             tc.tile_pool(name="strm", bufs=2) as sp, \
             tc.tile_pool(name="scr", bufs=1) as sc, \
             tc.tile_pool(name="gath", bufs=1) as gp, \
             tc.tile_pool(name="ps", bufs=2, space="PSUM") as pp, \
             tc.tile_pool(name="jp", bufs=1, space="PSUM") as jp:

            rtm = cpool.tile([128, 16 * 8], fp32, tag="rtm")
            rtm0 = cpool.tile([128, 16 * 8], fp32, tag="rtm0")
            rtw = cpool.tile([128, 16], fp32, tag="rtw")
            bwc = cpool.tile([128, NS * 4], fp32, tag="bw")
            idn = cpool.tile([128, 128], fp16, tag="idn")
            Tq = cpool.tile([2, 16], fp32, tag="Tq")
            intr = cpool.tile([2, 4], fp32, tag="intr")
            nc.sync.dma_start(out=rtm[:, :], in_=rtm_in.ap())
            nc.sync.dma_start(out=rtm0[:, :], in_=rtm_in.ap())
            nc.sync.dma_start(out=rtw[:, :], in_=rtw_in.ap())
            nc.sync.dma_start(out=bwc[:, :], in_=bw_in.ap())
            nc.sync.dma_start(out=idn[:, :], in_=idn_in.ap())
            nc.sync.dma_start(out=Tq[:, :], in_=t0_in.ap())
            nc.sync.dma_start(out=intr[:, :], in_=intr_in.ap())

            psJ = [jp.tile([28, 28], fp32, name=f"psJ{b}", tag=f"psJ{b}") for b in range(2)]

            tbl0 = tpool.tile([128, NELEM * 2], fp16, tag="tbl")
            nc.vector.memset(tbl0[:, :], 0.0)
            stbl0 = tpool.tile([128, 34 * (TC + 1)], fp16, tag="stbl")
            nc.vector.memset(stbl0[:, :], 0.0)

            def rq(qi):   # mod-128 per-group broadcast: dims (g x8, a x A step0)
                sl = rtm[:, qi * 8:(qi + 1) * 8]
                return bass.AP(sl.tensor, sl.offset, [list(sl.ap[0]), [1, 8], [0, A]])

            def rqw(qi):  # wrapped per-partition scalar bcast over M
                sl = rtw[:, qi:qi + 1]
                return bass.AP(sl.tensor, sl.offset, [list(sl.ap[0]), [0, M]])

            def rtwS(qi):  # wrapped per-partition scalar [128,1]
                return rtw[:, qi:qi + 1]

            def bwq(s, j):
                sl = bwc[:, s * 4 + j:s * 4 + j + 1]
                return bass.AP(sl.tensor, sl.offset, [list(sl.ap[0]), [0, M]])

            def bwS(s, j):
                return bwc[:, s * 4 + j:s * 4 + j + 1]

            TT = nc.vector.tensor_tensor
            TS = lambda out, in0, s1, op: nc.vector.tensor_scalar(out, in0, s1, None, op)
            TS2 = lambda out, in0, s1, s2, op0, op1: nc.vector.tensor_scalar(out, in0, s1, s2, op0, op1)
            STT = nc.vector.scalar_tensor_tensor

            with tc.For_i(0, N_ITERS) as _it:
                for s in range(NS):
                    tbl = tbl0
                    for r0, nr in ((0, 34), (34, 33)):
                        for g in range(8):
                            b, yb, xh = chunk_of(g, s)
                            rbase, cbase = bases_of(yb, xh)
                            start = (rbase + r0) * W + cbase
                            src0 = pd_in.ap()
                            src = bass.AP(src0.tensor,
                                          src0.offset + b * 12 * (HW + 2) + start,
                                          [[HW + 2, 12], [W, nr], [1, TC + 1]])
                            dsl = stbl0[16 * g:16 * g + 12, :]
                            dst = bass.AP(dsl.tensor, dsl.offset,
                                          [[dsl.ap[0][0], 12], [TC + 1, nr], [1, TC + 1]])
                            nc.sync.dma_start(out=dst, in_=src)
                        for e in range(2):
                            pout = bass.AP(tbl.tensor, tbl.offset + e + r0 * 2 * TC,
                                           [list(tbl.ap[0]), [2 * TC, nr], [2, TC]])
                            pin = bass.AP(stbl0.tensor, stbl0.offset + e,
                                          [list(stbl0.ap[0]), [TC + 1, nr], [1, TC]])
                            nc.scalar.activation(pout, pin, ACT.Copy)

                    x1w = sp.tile([128, M * 3], fp16, tag="x1w")
                    nc.sync.dma_start(out=x1w[:, :], in_=x1w_in.ap()[:, s * M * 3:(s + 1) * M * 3])
                    x1m = sp.tile([128, 8 * A * 3], fp16, tag="x1m")
                    nc.sync.dma_start(out=x1m[:, :], in_=x1m_in.ap()[:, s * 8 * A * 3:(s + 1) * 8 * A * 3])
                    i1 = sp.tile([128, 8 * A * 3], fp16, tag="i1")
                    nc.sync.dma_start(out=i1[:, :], in_=i1m_in.ap()[:, s * 8 * A * 3:(s + 1) * 8 * A * 3])

                    # ---------- wrapped-16 idx pipeline ----------
                    def xw(k):
                        sl = x1w[:, :]
                        return bass.AP(sl.tensor, sl.offset + k, [list(sl.ap[0]), [3, M]])

                    def tw(name):
                        return sc.tile([128, M], fp32, name="w_" + name + f"_{s}", tag="w_" + name)

                    t1w = tw("t1")
                    X0zw = tw("X0z")
                    STT(X0zw[:, :], xw(0), rtwS(6), rqw(11), AL.mult, AL.add)
                    STT(X0zw[:, :], xw(1), rtwS(7), X0zw[:, :], AL.mult, AL.add)
                    STT(X0zw[:, :], xw(2), rtwS(8), X0zw[:, :], AL.mult, AL.add)
                    X0xw = tw("X0x")
                    STT(X0xw[:, :], xw(0), rtwS(0), rqw(9), AL.mult, AL.add)
                    STT(X0xw[:, :], xw(1), rtwS(1), X0xw[:, :], AL.mult, AL.add)
                    STT(X0xw[:, :], xw(2), rtwS(2), X0xw[:, :], AL.mult, AL.add)
                    X0yw = tw("X0y")
                    STT(X0yw[:, :], xw(0), rtwS(3), rqw(10), AL.mult, AL.add)
                    STT(X0yw[:, :], xw(1), rtwS(4), X0yw[:, :], AL.mult, AL.add)
                    STT(X0yw[:, :], xw(2), rtwS(5), X0yw[:, :], AL.mult, AL.add)

                    izw = tw("iz")
                    TS(t1w[:, :], X0zw[:, :], EPS, AL.max)
                    nc.vector.reciprocal_approx_fast(izw[:, :], t1w[:, :])
                    u0w = tw("u0"); v0w = tw("v0")
                    TT(u0w[:, :], X0xw[:, :], izw[:, :], op=AL.mult)
                    STT(u0w[:, :], u0w[:, :], rtwS(12), rqw(14), AL.mult, AL.add)
                    TT(v0w[:, :], X0yw[:, :], izw[:, :], op=AL.mult)
                    STT(v0w[:, :], v0w[:, :], rtwS(13), rqw(15), AL.mult, AL.add)
                    TS2(u0w[:, :], u0w[:, :], -0.5 * (W - 1), 1.5 * (W - 1), AL.max, AL.min)
                    TS2(v0w[:, :], v0w[:, :], -0.5 * (H - 1), 1.5 * (H - 1), AL.max, AL.min)
                    x0fw = tw("x0f"); y0fw = tw("y0f")
                    fi32w = sc.tile([128, M], i32, name=f"fi32w_{s}", tag="fi32w")
                    TS(t1w[:, :], u0w[:, :], 0.5, AL.subtract)
                    nc.vector.tensor_copy(fi32w[:, :], t1w[:, :])
                    nc.vector.tensor_copy(x0fw[:, :], fi32w[:, :])
                    TS(t1w[:, :], v0w[:, :], 0.5, AL.subtract)
                    nc.vector.tensor_copy(fi32w[:, :], t1w[:, :])
                    nc.vector.tensor_copy(y0fw[:, :], fi32w[:, :])
                    xfw = t1w; kxw = izw; yrw = X0zw
                    ktw = X0xw; kbw = X0yw
                    STT(xfw[:, :], x0fw[:, :], bwS(s, 1), bwq(s, 2), AL.max, AL.min)
                    nc.vector.tensor_scalar(kxw[:, :], xfw[:, :], bwS(s, 3), None, AL.add)
                    nc.vector.tensor_scalar(yrw[:, :], y0fw[:, :], bwS(s, 0), 0.0, AL.subtract, AL.max)
                    TS2(ktw[:, :], yrw[:, :], float(TR - 1), float(TC), AL.min, AL.mult)
                    TT(ktw[:, :], ktw[:, :], kxw[:, :], op=AL.add)
                    TS2(kbw[:, :], yrw[:, :], 1.0, float(TR - 1), AL.add, AL.min)
                    TS(kbw[:, :], kbw[:, :], float(TC), AL.mult)
                    TT(kbw[:, :], kbw[:, :], kxw[:, :], op=AL.add)
                    kidx = sc.tile([128, 2 * M], i16, name=f"kidx_{s}", tag="kidx")
                    nc.vector.tensor_copy(kidx[:, :M], ktw[:, :])
                    nc.vector.tensor_copy(kidx[:, M:], kbw[:, :])

                    gt2 = gp.tile([128, 2 * N * 2], fp16, tag="gt2")
                    nc.gpsimd.ap_gather(gt2[:, :], tbl[:, :], kidx[:, :],
                                        channels=128, num_elems=NELEM, d=2, num_idxs=2 * N)

                    # ---------- mod-128 warp pipeline ----------
                    def xm(k):
                        sl = x1m[:, :]
                        return bass.AP(sl.tensor, sl.offset + k, [list(sl.ap[0]), [3, 8 * A]])

                    def tm(name):
                        return sc.tile([128, 8 * A], fp32, name="m_" + name + f"_{s}", tag="m_" + name)

                    def matvec(dst, aps, qis, t1):
                        TT(dst[:, :], aps[0], qis[0], op=AL.mult)
                        TT(t1[:, :], aps[1], qis[1], op=AL.mult)
                        TT(dst[:, :], dst[:, :], t1[:, :], op=AL.add)
                        TT(t1[:, :], aps[2], qis[2], op=AL.mult)
                        TT(dst[:, :], dst[:, :], t1[:, :], op=AL.add)
                        TT(dst[:, :], dst[:, :], qis[3], op=AL.add)

                    # ---- on-device A6/B6/T6 at the initial pose (rtm0) ----
                    abt = sc.tile([128, 8 * A * 18], fp16, name=f"abt_{s}", tag="abt")

                    def acol(k):
                        sl = abt[:, :]
                        return bass.AP(sl.tensor, sl.offset + k, [list(sl.ap[0]), [18, 224]])

                    def rq0(qi):
                        sl = rtm0[:, qi * 8:(qi + 1) * 8]
                        return bass.AP(sl.tensor, sl.offset, [list(sl.ap[0]), [1, 8], [0, A]])

                    j1 = tm("j1"); j2 = tm("j2")
                    jx = tm("jx"); jy = tm("jy"); jz = tm("jz"); jiz = tm("jiz")
                    matvec(jz, [xm(0), xm(1), xm(2)], [rq0(6), rq0(7), rq0(8), rq0(11)], j1)
                    matvec(jx, [xm(0), xm(1), xm(2)], [rq0(0), rq0(1), rq0(2), rq0(9)], j1)
                    matvec(jy, [xm(0), xm(1), xm(2)], [rq0(3), rq0(4), rq0(5), rq0(10)], j1)
                    TS(j1[:, :], jz[:, :], EPS, AL.max)
                    nc.vector.reciprocal_approx_fast(jiz[:, :], j1[:, :])
                    fxiz = tm("fxiz"); fyiz = tm("fyiz"); zizt = tm("zizt")
                    A2t = tm("A2t"); B2t = tm("B2t")
                    TT(fxiz[:, :], jiz[:, :], rq0(12), op=AL.mult)
                    TT(fyiz[:, :], jiz[:, :], rq0(13), op=AL.mult)
                    TT(zizt[:, :], jz[:, :], jiz[:, :], op=AL.mult)
                    TT(j1[:, :], jx[:, :], jiz[:, :], op=AL.mult)
                    TT(A2t[:, :], fxiz[:, :], j1[:, :], op=AL.mult)
                    TT(j1[:, :], jy[:, :], jiz[:, :], op=AL.mult)
                    TT(B2t[:, :], fyiz[:, :], j1[:, :], op=AL.mult)
                    TS(acol(0), fxiz[:, :], -1.0, AL.mult)
                    TS(acol(1), fxiz[:, :], 0.0, AL.mult)
                    nc.vector.tensor_copy(acol(2), A2t[:, :])
                    TT(acol(3), A2t[:, :], jy[:, :], op=AL.mult)
                    TT(j1[:, :], zizt[:, :], rq0(12), op=AL.mult)
                    TT(j2[:, :], A2t[:, :], jx[:, :], op=AL.mult)
                    TT(j1[:, :], j1[:, :], j2[:, :], op=AL.add)
                    TS(acol(4), j1[:, :], -1.0, AL.mult)
                    TT(acol(5), fxiz[:, :], jy[:, :], op=AL.mult)
                    TS(acol(6), fxiz[:, :], 0.0, AL.mult)
                    TS(acol(7), fyiz[:, :], -1.0, AL.mult)
                    nc.vector.tensor_copy(acol(8), B2t[:, :])
                    TT(j1[:, :], zizt[:, :], rq0(13), op=AL.mult)
                    TT(j2[:, :], B2t[:, :], jy[:, :], op=AL.mult)
                    TT(acol(9), j1[:, :], j2[:, :], op=AL.add)
                    TT(j1[:, :], B2t[:, :], jx[:, :], op=AL.mult)
                    TS(acol(10), j1[:, :], -1.0, AL.mult)
                    TT(j1[:, :], fyiz[:, :], jx[:, :], op=AL.mult)
                    TS(acol(11), j1[:, :], -1.0, AL.mult)
                    TS(acol(12), fxiz[:, :], 0.0, AL.mult)
                    TS(acol(13), fxiz[:, :], 0.0, AL.mult)
                    TS2(acol(14), fxiz[:, :], 0.0, 1.0, AL.mult, AL.add)
                    nc.vector.tensor_copy(acol(15), jy[:, :])
                    TS(acol(16), jx[:, :], -1.0, AL.mult)
                    TS(acol(17), fxiz[:, :], 0.0, AL.mult)

                    m1 = j1; m2 = j2
                    X0z = jz
                    matvec(X0z, [xm(0), xm(1), xm(2)], [rq(6), rq(7), rq(8), rq(11)], m1)
                    X0x = jx
                    matvec(X0x, [xm(0), xm(1), xm(2)], [rq(0), rq(1), rq(2), rq(9)], m1)
                    X0y = jy
                    matvec(X0y, [xm(0), xm(1), xm(2)], [rq(3), rq(4), rq(5), rq(10)], m1)
                    iz = jiz
                    TS(m1[:, :], X0z[:, :], EPS, AL.max)
                    nc.vector.reciprocal_approx_fast(iz[:, :], m1[:, :])
                    u0 = fxiz; v0 = fyiz
                    TT(u0[:, :], X0x[:, :], iz[:, :], op=AL.mult)
                    TT(u0[:, :], u0[:, :], rq(12), op=AL.mult)
                    TT(u0[:, :], u0[:, :], rq(14), op=AL.add)
                    TT(v0[:, :], X0y[:, :], iz[:, :], op=AL.mult)
                    TT(v0[:, :], v0[:, :], rq(13), op=AL.mult)
                    TT(v0[:, :], v0[:, :], rq(15), op=AL.add)
                    vmask = zizt
                    TS(vmask[:, :], X0z[:, :], EPS, AL.is_gt)
                    STT(vmask[:, :], u0[:, :], 0.0, vmask[:, :], AL.is_gt, AL.mult)
                    STT(vmask[:, :], u0[:, :], float(W - 1), vmask[:, :], AL.is_lt, AL.mult)
                    STT(vmask[:, :], v0[:, :], 0.0, vmask[:, :], AL.is_gt, AL.mult)
                    STT(vmask[:, :], v0[:, :], float(H - 1), vmask[:, :], AL.is_lt, AL.mult)
                    TS2(u0[:, :], u0[:, :], -0.5 * (W - 1), 1.5 * (W - 1), AL.max, AL.min)
                    TS2(v0[:, :], v0[:, :], -0.5 * (H - 1), 1.5 * (H - 1), AL.max, AL.min)
                    wx = A2t; wy = B2t; x0f = tm("x0f"); y0f = tm("y0f")
                    fi32m = sc.tile([128, 8 * A], i32, name=f"fi32m_{s}", tag="fi32m")
                    TS(m1[:, :], u0[:, :], 0.5, AL.subtract)
                    nc.vector.tensor_copy(fi32m[:, :], m1[:, :])
                    nc.vector.tensor_copy(x0f[:, :], fi32m[:, :])
                    TT(wx[:, :], u0[:, :], x0f[:, :], op=AL.subtract)
                    TS(m1[:, :], v0[:, :], 0.5, AL.subtract)
                    nc.vector.tensor_copy(fi32m[:, :], m1[:, :])
                    nc.vector.tensor_copy(y0f[:, :], fi32m[:, :])
                    TT(wy[:, :], v0[:, :], y0f[:, :], op=AL.subtract)
                    mx0 = tm("mx0"); mx1 = tm("mx1"); my0 = tm("my0"); my1 = tm("my1")
                    TS(mx0[:, :], x0f[:, :], -0.5, AL.is_gt)
                    STT(mx0[:, :], x0f[:, :], float(W - 1) + 0.5, mx0[:, :], AL.is_lt, AL.mult)
                    TS(mx1[:, :], x0f[:, :], -1.5, AL.is_gt)
                    STT(mx1[:, :], x0f[:, :], float(W - 2) + 0.5, mx1[:, :], AL.is_lt, AL.mult)
                    TS(my0[:, :], y0f[:, :], -0.5, AL.is_gt)
                    STT(my0[:, :], y0f[:, :], float(H - 1) + 0.5, my0[:, :], AL.is_lt, AL.mult)
                    TS(my1[:, :], y0f[:, :], -1.5, AL.is_gt)
                    STT(my1[:, :], y0f[:, :], float(H - 2) + 0.5, my1[:, :], AL.is_lt, AL.mult)
                    W00 = tm("W00"); W01 = tm("W01"); W10 = tm("W10"); W11 = tm("W11")
                    TS2(m1[:, :], wx[:, :], 1.0, -1.0, AL.subtract, AL.mult)  # 1-wx
                    TS2(m2[:, :], wy[:, :], 1.0, -1.0, AL.subtract, AL.mult)  # 1-wy
                    TT(W00[:, :], m1[:, :], m2[:, :], op=AL.mult)
                    TT(W00[:, :], W00[:, :], mx0[:, :], op=AL.mult)
                    TT(W00[:, :], W00[:, :], my0[:, :], op=AL.mult)
                    TT(W01[:, :], wx[:, :], m2[:, :], op=AL.mult)
                    TT(W01[:, :], W01[:, :], mx1[:, :], op=AL.mult)
                    TT(W01[:, :], W01[:, :], my0[:, :], op=AL.mult)
                    TT(W10[:, :], m1[:, :], wy[:, :], op=AL.mult)
                    TT(W10[:, :], W10[:, :], mx0[:, :], op=AL.mult)
                    TT(W10[:, :], W10[:, :], my1[:, :], op=AL.mult)
                    TT(W11[:, :], wx[:, :], wy[:, :], op=AL.mult)
                    TT(W11[:, :], W11[:, :], mx1[:, :], op=AL.mult)
                    TT(W11[:, :], W11[:, :], my1[:, :], op=AL.mult)

                    # ---------- PE transpose + combine ----------
                    samp = sc.tile([128, A * 128], fp16, tag="samp")
                    ctmp = sc.tile([128, 512], fp16, tag="ctmp")
                    for a4 in range(A // 4):
                        ptall = pp.tile([128, 2048], fp16, tag="ptall")
                        for ci, base in enumerate((0, 1, 2 * N, 2 * N + 1)):
                            for aa in range(4):
                                a = a4 * 4 + aa
                                src = bass.AP(gt2.tensor, gt2.offset + base + a * 256,
                                              [list(gt2.ap[0]), [2, 128]])
                                nc.tensor.transpose(
                                    ptall[:, ci * 512 + aa * 128:ci * 512 + (aa + 1) * 128],
                                    src, idn[:, :])
                        for ci, wt_ in ((0, W00), (1, W01), (2, W10), (3, W11)):
                            pap = bass.AP(ptall.tensor, ptall.offset + ci * 512,
                                          [list(ptall.ap[0]), [128, 4], [16, 8], [1, 16]])
                            woff = wt_.offset + a4 * 4
                            wap = bass.AP(wt_.tensor, woff, [list(wt_.ap[0]), [1, 4], [A, 8], [0, 16]])
                            dst_off = samp.offset + a4 * 4 * 128
                            dap = bass.AP(samp.tensor, dst_off, [list(samp.ap[0]), [128, 4], [16, 8], [1, 16]])
                            if ci == 0:
                                TT(dap, pap, wap, op=AL.mult)
                            else:
                                tap = bass.AP(ctmp.tensor, ctmp.offset, [list(ctmp.ap[0]), [128, 4], [16, 8], [1, 16]])
                                TT(tap, pap, wap, op=AL.mult)
                                TT(dap, dap, tap, op=AL.add)

                    # ---------- residuals, huber weights, G build ----------
                    def sq(q):
                        sl = samp[:, :]
                        return bass.AP(sl.tensor, sl.offset + q, [list(sl.ap[0]), [16, 8], [128, A]])

                    def i1q(c):
                        sl = i1[:, :]
                        return bass.AP(sl.tensor, sl.offset + c, [list(sl.ap[0]), [3 * A, 8], [3, A]])

                    Gt = sc.tile([128, 28 * 224], fp16, tag="Gt")
                    g6a = sc.tile([128, 6 * 224], fp16, tag="g6a")
                    g6b = sc.tile([128, 6 * 224], fp16, tag="g6b")
                    one_m = tm("one_m")
                    TS2(one_m[:, :], vmask[:, :], 1.0, -1e-6, AL.subtract, AL.mult)  # (1-vm)*1e-6
                    rr = tm("rr"); bb_ = tm("bb"); ss = tm("ss")
                    ppv = tm("ppv"); qqv = tm("qqv")

                    def abt6(k0):  # [x(6) outer, chunk(224) inner], stride 18 per chunk
                        sl = abt[:, :]
                        return bass.AP(sl.tensor, sl.offset + k0, [list(sl.ap[0]), [1, 6], [18, 224]])

                    def gcols(c):  # G cols c*7 .. c*7+5: [x outer, chunk inner]
                        sl = Gt[:, :]
                        return bass.AP(sl.tensor, sl.offset + c * 7 * 224, [list(sl.ap[0]), [224, 6], [1, 224]])

                    def bc6(t):    # broadcast [128,224] over 6 x-cols
                        sl = t[:, :]
                        return bass.AP(sl.tensor, sl.offset, [list(sl.ap[0]), [0, 6], [1, 224]])

                    for c in range(3):
                        TT(rr[:, :], i1q(c), sq(8 + c), op=AL.subtract)
                        TT(rr[:, :], rr[:, :], vmask[:, :], op=AL.mult)
                        TT(rr[:, :], rr[:, :], one_m[:, :], op=AL.add)
                        nc.scalar.activation(bb_[:, :], rr[:, :], ACT.Abs)
                        TS(bb_[:, :], bb_[:, :], HUBER_DELTA, AL.max)
                        nc.vector.reciprocal_approx_fast(bb_[:, :], bb_[:, :])
                        nc.scalar.activation(ss[:, :], bb_[:, :], ACT.Sqrt, scale=HUBER_DELTA)
                        TT(ppv[:, :], ss[:, :], sq(0 + c), op=AL.mult)
                        TT(qqv[:, :], ss[:, :], sq(3 + c), op=AL.mult)
                        TT(g6a[:, :], abt6(0), bc6(ppv), op=AL.mult)
                        TT(g6b[:, :], abt6(6), bc6(qqv), op=AL.mult)
                        TT(gcols(c), g6a[:, :], g6b[:, :], op=AL.add)
                        TT(Gt[:, (c * 7 + 6) * 224:(c * 7 + 7) * 224], ss[:, :], rr[:, :], op=AL.mult)
                    # depth channel
                    TT(rr[:, :], iz[:, :], sq(11), op=AL.subtract)
                    TT(rr[:, :], rr[:, :], vmask[:, :], op=AL.mult)
                    TT(rr[:, :], rr[:, :], one_m[:, :], op=AL.add)
                    nc.scalar.activation(bb_[:, :], rr[:, :], ACT.Abs, scale=LAMBDA)
                    TS(bb_[:, :], bb_[:, :], HUBER_DELTA, AL.max)
                    nc.vector.reciprocal_approx_fast(bb_[:, :], bb_[:, :])
                    nc.scalar.activation(ss[:, :], bb_[:, :], ACT.Sqrt,
                                         scale=HUBER_DELTA * LAMBDA * LAMBDA)
                    TT(ppv[:, :], ss[:, :], sq(6), op=AL.mult)
                    TT(qqv[:, :], ss[:, :], sq(7), op=AL.mult)
                    TT(g6a[:, :], abt6(0), bc6(ppv), op=AL.mult)
                    TT(g6b[:, :], abt6(6), bc6(qqv), op=AL.mult)
                    TT(g6a[:, :], g6a[:, :], g6b[:, :], op=AL.add)
                    TT(g6b[:, :], abt6(12), bc6(ss), op=AL.mult)
                    TT(gcols(3), g6a[:, :], g6b[:, :], op=AL.add)
                    TT(Gt[:, (3 * 7 + 6) * 224:(3 * 7 + 7) * 224], ss[:, :], rr[:, :], op=AL.mult)

                    # ---------- PE: JtWJ accumulation ----------
                    for g in range(8):
                        b = g // 4
                        for a in range(A):
                            off = Gt.offset + g * A + a
                            gap = bass.AP(Gt.tensor, off, [list(Gt.ap[0]), [224, 28]])
                            first = (s == 0 and (g % 4) == 0 and a == 0)
                            last = (s == NS - 1 and (g % 4) == 3 and a == A - 1)
                            nc.tensor.matmul(psJ[b][:, :], gap, gap,
                                             start=first, stop=last,
                                             skip_group_check=True)

                # ---------- per-iteration: extract JtWJ/Rhs, solve, update pose ----------
                S28 = sc.tile([28, 56], fp32, tag="S28")
                for b in range(2):
                    nc.vector.tensor_copy(S28[:, b * 28:(b + 1) * 28], psJ[b][:, :])
                D28 = sc.tile([7, 56], fp32, tag="D28")
                for b in range(2):
                    for c in range(4):
                        src = S28[c * 7:(c + 1) * 7, b * 28 + c * 7:b * 28 + c * 7 + 7]
                        dsl = D28[:, b * 28 + c * 7:b * 28 + c * 7 + 7]
                        nc.sync.dma_start(out=dsl, in_=src)
                M7 = sc.tile([7, 14], fp32, tag="M7")
                for b in range(2):
                    din = bass.AP(D28.tensor, D28.offset + b * 28,
                                  [list(D28.ap[0]), [1, 7], [7, 4]])
                    nc.vector.tensor_reduce(M7[:, b * 7:(b + 1) * 7], din, axis=AX.X, op=AL.add)
                # Mb [2, 49]: row b = M7 block b flattened (x-major)
                Mb = sc.tile([2, 49], fp32, tag="Mb")
                for b in range(2):
                    msrc = bass.AP(M7.tensor, M7.offset + b * 7, [[M7.ap[0][0], 7], [1, 7]])
                    mdsl = Mb[b:b + 1, 0:1]
                    mdst = bass.AP(mdsl.tensor, mdsl.offset, [[Mb.ap[0][0], 1], [7, 7], [1, 7]])
                    nc.sync.dma_start(out=mdst, in_=msrc)
                # tr = sum diag(JtWJ); LM ridge on diag
                trt = sc.tile([2, 1], fp32, tag="trt")
                diag = bass.AP(Mb.tensor, Mb.offset, [list(Mb.ap[0]), [8, 6]])
                nc.vector.tensor_reduce(trt[:, :], diag, axis=AX.X, op=AL.add)
                trb = bass.AP(trt.tensor, trt.offset, [list(trt.ap[0]), [0, 6]])
                STT(diag, trb, 1e-6, diag, AL.mult, AL.add)

                # Cholesky LL^T = Hm (6x6, both batches in 2 partitions)
                Lt = sc.tile([2, 36], fp32, tag="Lt")
                lsrc = bass.AP(Mb.tensor, Mb.offset, [list(Mb.ap[0]), [7, 6], [1, 6]])
                nc.vector.tensor_copy(Lt[:, :], lsrc)
                rhs = sc.tile([2, 6], fp32, tag="rhs")
                rsrc = bass.AP(Mb.tensor, Mb.offset + 6, [list(Mb.ap[0]), [7, 6]])
                nc.vector.tensor_copy(rhs[:, :], rsrc)
                idg = sc.tile([2, 6], fp32, tag="idg")
                tmpj = sc.tile([2, 36], fp32, tag="tmpj")
                red = sc.tile([2, 6], fp32, tag="redj")
                for j in range(6):
                    jj = Lt[:, 6 * j + j:6 * j + j + 1]
                    if j > 0:
                        ljk = Lt[:, 6 * j:6 * j + j]
                        TT(tmpj[:, :j], ljk, ljk, op=AL.mult)
                        nc.vector.tensor_reduce(red[:, 0:1], tmpj[:, :j], axis=AX.X, op=AL.add)
                        TT(jj, jj, red[:, 0:1], op=AL.subtract)
                    nc.scalar.activation(jj, jj, ACT.Sqrt)
                    nc.vector.reciprocal(idg[:, j:j + 1], jj)
                    nr = 5 - j
                    if nr > 0:
                        colap = bass.AP(Lt.tensor, Lt.offset + 6 * (j + 1) + j, [list(Lt.ap[0]), [6, nr]])
                        if j > 0:
                            lik = bass.AP(Lt.tensor, Lt.offset + 6 * (j + 1), [list(Lt.ap[0]), [6, nr], [1, j]])
                            ljkb = bass.AP(Lt.tensor, Lt.offset + 6 * j, [list(Lt.ap[0]), [0, nr], [1, j]])
                            TT(tmpj[:, :nr * j], lik, ljkb, op=AL.mult)
                            tin = bass.AP(tmpj.tensor, tmpj.offset, [list(tmpj.ap[0]), [j, nr], [1, j]])
                            nc.vector.tensor_reduce(red[:, :nr], tin, axis=AX.X, op=AL.add)
                            TT(colap, colap, red[:, :nr], op=AL.subtract)
                        nc.vector.tensor_scalar(colap, colap, idg[:, j:j + 1], None, AL.mult)
                # forward substitution: L y = rhs (in place on rhs)
                for j in range(6):
                    yj = rhs[:, j:j + 1]
                    if j > 0:
                        ljk = Lt[:, 6 * j:6 * j + j]
                        TT(tmpj[:, :j], ljk, rhs[:, :j], op=AL.mult)
                        nc.vector.tensor_reduce(red[:, 0:1], tmpj[:, :j], axis=AX.X, op=AL.add)
                        TT(yj, yj, red[:, 0:1], op=AL.subtract)
                    nc.vector.tensor_scalar(yj, yj, idg[:, j:j + 1], None, AL.mult)
                # back substitution: L^T x = y -> xi = -x stored in xi tile
                for j in range(5, -1, -1):
                    xj = rhs[:, j:j + 1]
                    nk = 5 - j
                    if nk > 0:
                        lkj = bass.AP(Lt.tensor, Lt.offset + 6 * (j + 1) + j, [list(Lt.ap[0]), [6, nk]])
                        TT(tmpj[:, :nk], lkj, rhs[:, j + 1:6], op=AL.mult)
                        nc.vector.tensor_reduce(red[:, 0:1], tmpj[:, :nk], axis=AX.X, op=AL.add)
                        TT(xj, xj, red[:, 0:1], op=AL.subtract)
                    nc.vector.tensor_scalar(xj, xj, idg[:, j:j + 1], None, AL.mult)
                xi = sc.tile([2, 6], fp32, tag="xi")
                TS(xi[:, :], rhs[:, :], -1.0, AL.mult)

                # se3_exp(xi) via Taylor series (|w| << 1 in this regime)
                w3 = xi[:, 3:6]
                wsq = sc.tile([2, 3], fp32, tag="wsq")
                TT(wsq[:, :], w3, w3, op=AL.mult)
                th2 = sc.tile([2, 1], fp32, tag="th2")
                nc.vector.tensor_reduce(th2[:, :], wsq[:, :], axis=AX.X, op=AL.add)
                coA = sc.tile([2, 1], fp32, tag="coA")
                coB = sc.tile([2, 1], fp32, tag="coB")
                coC = sc.tile([2, 1], fp32, tag="coC")
                hh = sc.tile([2, 1], fp32, tag="hh")
                TS2(hh[:, :], th2[:, :], 1.0 / 120.0, -1.0 / 6.0, AL.mult, AL.add)
                nc.vector.tensor_scalar(coA[:, :], th2[:, :], hh[:, :], 1.0, AL.mult, AL.add)
                TS2(hh[:, :], th2[:, :], 1.0 / 720.0, -1.0 / 24.0, AL.mult, AL.add)
                nc.vector.tensor_scalar(coB[:, :], th2[:, :], hh[:, :], 0.5, AL.mult, AL.add)
                TS2(hh[:, :], th2[:, :], 1.0 / 5040.0, -1.0 / 120.0, AL.mult, AL.add)
                nc.vector.tensor_scalar(coC[:, :], th2[:, :], hh[:, :], 1.0 / 6.0, AL.mult, AL.add)
                # K, K2
                Kt = sc.tile([2, 9], fp32, tag="Kt")
                nc.vector.memset(Kt[:, :], 0.0)
                TS(Kt[:, 1:2], xi[:, 5:6], -1.0, AL.mult)   # -z
                nc.vector.tensor_copy(Kt[:, 2:3], xi[:, 4:5])  # y
                nc.vector.tensor_copy(Kt[:, 3:4], xi[:, 5:6])  # z
                TS(Kt[:, 5:6], xi[:, 3:4], -1.0, AL.mult)   # -x
                TS(Kt[:, 6:7], xi[:, 4:5], -1.0, AL.mult)   # -y
                nc.vector.tensor_copy(Kt[:, 7:8], xi[:, 3:4])  # x
                K2t = sc.tile([2, 9], fp32, tag="K2t")
                wiap = bass.AP(xi.tensor, xi.offset + 3, [list(xi.ap[0]), [1, 3], [0, 3]])
                wjap = bass.AP(xi.tensor, xi.offset + 3, [list(xi.ap[0]), [0, 3], [1, 3]])
                TT(K2t[:, :], wiap, wjap, op=AL.mult)
                k2diag = bass.AP(K2t.tensor, K2t.offset, [list(K2t.ap[0]), [4, 3]])
                nc.vector.tensor_scalar(k2diag, k2diag, th2[:, :], None, AL.subtract)
                Rt = sc.tile([2, 9], fp32, tag="Rt")
                Vt = sc.tile([2, 9], fp32, tag="Vt")
                t9 = sc.tile([2, 9], fp32, tag="t9")
                nc.vector.tensor_scalar(Rt[:, :], Kt[:, :], coA[:, :], None, AL.mult)
                nc.vector.tensor_scalar(t9[:, :], K2t[:, :], coB[:, :], None, AL.mult)
                TT(Rt[:, :], Rt[:, :], t9[:, :], op=AL.add)
                rdiag = bass.AP(Rt.tensor, Rt.offset, [list(Rt.ap[0]), [4, 3]])
                TS(rdiag, rdiag, 1.0, AL.add)
                nc.vector.tensor_scalar(Vt[:, :], Kt[:, :], coB[:, :], None, AL.mult)
                nc.vector.tensor_scalar(t9[:, :], K2t[:, :], coC[:, :], None, AL.mult)
                TT(Vt[:, :], Vt[:, :], t9[:, :], op=AL.add)
                vdiag = bass.AP(Vt.tensor, Vt.offset, [list(Vt.ap[0]), [4, 3]])
                TS(vdiag, vdiag, 1.0, AL.add)
                # t = V @ v
                vbc = bass.AP(xi.tensor, xi.offset, [list(xi.ap[0]), [0, 3], [1, 3]])
                TT(t9[:, :], Vt[:, :], vbc, op=AL.mult)
                tv = sc.tile([2, 3], fp32, tag="tv")
                t9v = bass.AP(t9.tensor, t9.offset, [list(t9.ap[0]), [3, 3], [1, 3]])
                nc.vector.tensor_reduce(tv[:, :], t9v, axis=AX.X, op=AL.add)
                # E = [[R, t],[0,0,0,1]] as [2,16]
                Et = sc.tile([2, 16], fp32, tag="Et")
                nc.vector.memset(Et[:, :], 0.0)
                edst = bass.AP(Et.tensor, Et.offset, [list(Et.ap[0]), [4, 3], [1, 3]])
                esrc = bass.AP(Rt.tensor, Rt.offset, [list(Rt.ap[0]), [3, 3], [1, 3]])
                nc.vector.tensor_copy(edst, esrc)
                edst2 = bass.AP(Et.tensor, Et.offset + 3, [list(Et.ap[0]), [4, 3]])
                nc.vector.tensor_copy(edst2, tv[:, :])
                TS(Et[:, 15:16], Et[:, 15:16], 1.0, AL.add)
                # newT = T @ E
                nT = sc.tile([2, 16], fp32, tag="nT")
                for k in range(4):
                    tcol = bass.AP(Tq.tensor, Tq.offset + k, [list(Tq.ap[0]), [4, 4], [0, 4]])
                    erow = bass.AP(Et.tensor, Et.offset + 4 * k, [list(Et.ap[0]), [0, 4], [1, 4]])
                    if k == 0:
                        TT(nT[:, :], tcol, erow, op=AL.mult)
                    else:
                        TT(tmpj[:, :16], tcol, erow, op=AL.mult)
                        TT(nT[:, :], nT[:, :], tmpj[:, :16], op=AL.add)
                nc.vector.tensor_copy(Tq[:, :], nT[:, :])
                # rebuild q = [R(9) | t(3) | intr(4)] and broadcast to rtm/rtw
                qt = sc.tile([2, 16], fp32, tag="qt")
                qr = bass.AP(Tq.tensor, Tq.offset, [list(Tq.ap[0]), [4, 3], [1, 3]])
                nc.vector.tensor_copy(qt[:, 0:9], qr)
                qtcol = bass.AP(Tq.tensor, Tq.offset + 3, [list(Tq.ap[0]), [4, 3]])
                nc.vector.tensor_copy(qt[:, 9:12], qtcol)
                nc.vector.tensor_copy(qt[:, 12:16], intr[:, :])
                nc.sync.dma_start(out=qscr.ap(), in_=qt[:, :])
                qsap = qscr.ap()
                for b in range(2):
                    qsrc = bass.AP(qsap.tensor, qsap.offset + b * 16, [[0, 64], [1, 16]])
                    nc.sync.dma_start(out=rtw[b * 64:(b + 1) * 64, :], in_=qsrc)
                for g in range(8):
                    b = g // 4
                    qsrc = bass.AP(qsap.tensor, qsap.offset + b * 16, [[0, 128], [1, 16]])
                    rdst = bass.AP(rtm.tensor, rtm.offset + g, [list(rtm.ap[0]), [8, 16]])
                    nc.sync.dma_start(out=rdst, in_=qsrc)

            nc.sync.dma_start(out=tout_ext.ap(), in_=Tq[:, :])

    nc.finalize()
    return nc


def _get_runner():
    """Build the sharded jitted executable ONCE (per-call jit(shard_map(...))
    in run_bass_via_pjrt retraces every call)."""
    if "runner" in _NC_CACHE:
        return _NC_CACHE["runner"]
    import jax
    import numpy as _np
    from concourse import mybir
    from concourse import bass2jax
    from jax.experimental.shard_map import shard_map
    from jax.sharding import Mesh, PartitionSpec

    nc = _NC_CACHE.get("nc")
    if nc is None:
        nc = build_nc()
        _NC_CACHE["nc"] = nc
    bass2jax.install_neuronx_cc_hook()

    partition_name = nc.partition_id_tensor.name if nc.partition_id_tensor else None
    in_names, out_names, out_avals, zero_outs = [], [], [], []
    for alloc in nc.m.functions[0].allocations:
        if not isinstance(alloc, mybir.MemoryLocationSet):
            continue
        name = alloc.memorylocations[0].name
        if alloc.kind == "ExternalInput":
            if name != partition_name:
                in_names.append(name)
        elif alloc.kind == "ExternalOutput":
            out_names.append(name)
            shape = tuple(alloc.tensor_shape)
            dtype = mybir.dt.np(alloc.dtype)
            out_avals.append(jax.core.ShapedArray(shape, dtype))
            zero_outs.append(_np.zeros(shape, dtype))
    n_params = len(in_names)
    n_outs = len(out_names)
    all_names = list(in_names) + out_names
    if partition_name is not None:
        all_names.append(partition_name)
    donate = tuple(range(n_params, n_params + n_outs))

    def _body(*args):
        operands = list(args)
        if partition_name is not None:
            operands.append(bass2jax.partition_id_tensor())
        outs = bass2jax._bass_exec_p.bind(
            *operands,
            out_avals=tuple(out_avals),
            in_names=tuple(all_names),
            out_names=tuple(out_names),
            lowering_input_output_aliases=(),
            sim_require_finite=True,
            sim_require_nnan=True,
            nc=nc,
        )
        return tuple(outs)

    devices = jax.devices()[:8]
    mesh = Mesh(_np.asarray(devices), ("core",))
    in_specs = (PartitionSpec("core"),) * (n_params + n_outs)
    out_specs = (PartitionSpec("core"),) * n_outs
    sharded = jax.jit(
        shard_map(_body, mesh=mesh, in_specs=in_specs, out_specs=out_specs,
                  check_rep=False),
        donate_argnums=donate, keep_unused=True)
    _NC_CACHE["runner"] = (sharded, in_names, out_names, out_avals, zero_outs)
    return _NC_CACHE["runner"]


def kernel(pose_twist, I0, I1, invD0, invD1, intrinsics):
    from concourse.bass_utils import run_bass_kernel_spmd

    nc = _NC_CACHE.get("nc")
    if nc is None:
        nc = build_nc()
        _NC_CACHE["nc"] = nc

    pose_twist = np.asarray(pose_twist, np.float32)
    I0 = np.asarray(I0, np.float32); I1 = np.asarray(I1, np.float32)
    invD0 = np.asarray(invD0, np.float32); invD1 = np.asarray(invD1, np.float32)
    intrinsics = np.asarray(intrinsics, np.float32)

    import time as _time
    LAST_WALL.clear(); LAST_EXEC_NS.clear(); LAST_TRACES.clear()
    t0 = _time.time()
    in_maps, _ = host_precompute_all(pose_twist, I0, I1, invD0, invD1, intrinsics)
    t1 = _time.time()
    if PROFILE:
        res = run_bass_kernel_spmd(nc, in_maps, list(range(8)), trace=True)
        if res.exec_time_ns is not None:
            LAST_EXEC_NS.append(res.exec_time_ns)
        if res.instructions_and_trace is not None:
            LAST_TRACES.append(res.instructions_and_trace[1])
        touts = [res.results[core]["tout"] for core in range(8)]
    else:
        sharded, in_names, out_names, out_avals, zero_outs = _get_runner()
        concat_in = [np.concatenate([in_maps[c][nm] for c in range(8)], axis=0)
                     for nm in in_names]
        concat_zeros = [np.zeros((8 * z.shape[0], *z.shape[1:]), z.dtype)
                        for z in zero_outs]
        out_arrs = sharded(*concat_in, *concat_zeros)
        oi = out_names.index("tout")
        tall = np.asarray(out_arrs[oi]).reshape(8, *out_avals[oi].shape)
        touts = [tall[c] for c in range(8)]
    t2 = _time.time()
    LAST_WALL.extend([round(t1 - t0, 3), round(t2 - t1, 3)])

    outs = [t.reshape(2, 4, 4) for t in touts]
    return np.concatenate(outs, axis=0).astype(np.float32)


# revision 48
# speedup vs baseline: 2.4689x; 1.6058x over previous
"""Trainium2 Bass kernel for nn_InvDirectImageAlign (inverse-compositional image alignment).

v3: ONE compiled NEFF runs all 5 Gauss-Newton iterations on device
(hardware For_i loop). Per core: 2 batch elements. Device does warp,
bilinear grid_sample (GPSIMD ap_gather from fp16 pair-dup band tables),
the JtWJ/Rhs normal equations via TensorEngine matmuls of a per-pixel
fp16 factor matrix G (JtWJ = sum_c G_c^T G_c), the 6x6 Cholesky solve,
se3_exp (Taylor series - angles are <<1 here) and the pose composition.
Inputs upload once; output is just the final 4x4 poses.

Chunking: (batch, 16-row y-band, 224-col x-half) = 80 chunks/core; the 8
GPSIMD partition-groups each own one chunk per superstep; 10 supersteps.
Two pixel layouts, bridged only by PE transposes of gathered data:
  mod-128:    pixel j of chunk(g,s) at partition j%128, free col (g, j//128)
  wrapped-16: pixel j at partition 16g + j%16, free col j//16   (ap_gather's
              index layout)
"""
import numpy as np

B, C, H, W = 16, 3, 320, 448
HW = H * W
N_ITERS = 5
LAMBDA = 0.01
HUBER_DELTA = 0.1
EPS = 1e-6

BH = 16            # band rows per chunk
CW = 224           # band cols per chunk
N = BH * CW        # 3584 px per chunk
A = N // 128       # 28
M = N // 16        # 224
NS = 10            # supersteps
TR = 67            # table rows (16 + 25 + 26)
TC = 266           # table cols (224 + 20 + 21 + 1)
NELEM = TR * TC    # 17822 pairs
YPAD = 25
XPAD = 20


def skew3(w):
    x, y, z = w[..., 0], w[..., 1], w[..., 2]
    O = np.zeros_like(x)
    return np.stack([np.stack([O, -z, y], -1),
                     np.stack([z, O, -x], -1),
                     np.stack([-y, x, O], -1)], -2)


def se3_exp(xi):
    xi = np.asarray(xi, np.float64)
    v, w = xi[:, :3], xi[:, 3:]
    th2 = np.sum(w * w, -1)[:, None, None]
    th2c = np.maximum(th2, 1e-16)
    th = np.sqrt(th2c)
    small = th2 < 1e-10
    Aa = np.where(small, 1.0 - th2 / 6.0, np.sin(th) / th)
    Bc = np.where(small, 0.5 - th2 / 24.0, (1.0 - np.cos(th)) / th2c)
    Cc = np.where(small, 1.0 / 6.0 - th2 / 120.0, (1.0 - Aa) / th2c)
    K = skew3(w)
    K2 = K @ K
    I = np.eye(3)
    R = I + Aa * K + Bc * K2
    V = I + Bc * K + Cc * K2
    t = np.einsum('bij,bj->bi', V, v)
    T = np.zeros((xi.shape[0], 4, 4))
    T[:, :3, :3] = R
    T[:, :3, 3] = t
    T[:, 3, 3] = 1.0
    return T.astype(np.float32)


def feature_gradient(img):
    p = np.pad(img, ((0, 0), (0, 0), (0, 0), (1, 1)), mode='edge')
    dx = 0.5 * (p[..., 2:] - p[..., :-2])
    p = np.pad(img, ((0, 0), (0, 0), (1, 1), (0, 0)), mode='edge')
    dy = 0.5 * (p[..., 2:, :] - p[..., :-2, :])
    return dx.astype(np.float32), dy.astype(np.float32)


def chunk_of(g, s):
    b = g // 4
    local = (g % 4) * 10 + s
    return b, local // 2, local % 2


def bases_of(yb, xh):
    r0, c0 = yb * BH, xh * CW
    rbase = int(np.clip(r0 - YPAD, 0, H - TR))
    cbase = int(np.clip(c0 - XPAD, 0, W - (TC - 1)))
    return rbase, cbase


def mod128_cols_batch(x):
    """[2,K,H,W] -> [128, NS*8*A*K] vectorized (one core's 2 batches)."""
    K = x.shape[1]
    # chunk (b, yb, xh): local = yb*2+xh; g = b*4 + local//10; s = local%10
    a = x.reshape(2, K, 20, BH, 2, CW)          # b K yb row xh col
    a = a.transpose(0, 2, 4, 1, 3, 5)           # b yb xh K row col
    a = a.reshape(2, 40, K, N)                  # local = yb*2+xh
    a = a.reshape(2, 4, 10, K, A, 128)          # b g4 s K a p
    a = a.transpose(5, 2, 0, 1, 4, 3)           # p s b g4 a K
    return np.ascontiguousarray(a.reshape(128, NS, 8, A, K).reshape(128, -1))


def wrap16_cols_batch(x, K):
    """[2,K,H,W] -> [128, NS*M*K] (partition 16g + j%16, col (j//16)*K + k)."""
    a = x.reshape(2, K, 20, BH, 2, CW)
    a = a.transpose(0, 2, 4, 1, 3, 5).reshape(2, 40, K, N)
    a = a.reshape(2, 4, 10, K, M, 16)           # b g4 s K m plo
    a = a.transpose(0, 1, 5, 2, 4, 3)           # b g4 plo s m K
    # partition = 16*(b*4+g4) + plo
    return np.ascontiguousarray(a.reshape(128, NS, M, K).reshape(128, -1))


def host_precompute_all(pose_twist, I0, I1, invD0, invD1, intr):
    """Vectorized over all B=16; returns per-core input dicts + T0 per core."""
    T0 = se3_exp(pose_twist)
    fx = intr[:, 0][:, None, None]; fy = intr[:, 1][:, None, None]
    cx = intr[:, 2][:, None, None]; cy = intr[:, 3][:, None, None]
    uu = np.arange(W, dtype=np.float32)[None, None, :]
    vv = np.arange(H, dtype=np.float32)[None, :, None]
    iD = np.maximum(invD1[:, 0], EPS).astype(np.float32)
    z1 = (1.0 / iD).astype(np.float32)
    xn = ((uu - cx) / fx).astype(np.float32)     # [B,1,W]
    yn = ((vv - cy) / fy).astype(np.float32)     # [B,H,1]
    x1 = xn * z1
    y1 = yn * z1
    # edge-replicated padded raw planes; device computes the 12 table planes
    rawp = np.empty((B, 4, H + 2, W + 2), np.float16)
    rawp[:, :3, 1:H + 1, 1:W + 1] = I0
    rawp[:, 3:, 1:H + 1, 1:W + 1] = invD0
    rawp[:, :, 0] = rawp[:, :, 1]
    rawp[:, :, H + 1] = rawp[:, :, H]
    rawp[:, :, :, 0] = rawp[:, :, :, 1]
    rawp[:, :, :, W + 1] = rawp[:, :, :, W]

    X1 = np.stack([x1, y1, z1], 1).astype(np.float16)       # [B, 3, H, W]
    I1f = np.asarray(I1, np.float32)

    bw = np.zeros((128, NS, 4), np.float32)
    for g in range(8):
        for s in range(NS):
            _, yb, xh2 = chunk_of(g, s)
            rbase, cbase = bases_of(yb, xh2)
            bw[16 * g:16 * g + 16, s, 0] = rbase
            bw[16 * g:16 * g + 16, s, 1] = cbase - 1          # xf min
            bw[16 * g:16 * g + 16, s, 2] = cbase + (TC - 2)   # xf max
            bw[16 * g:16 * g + 16, s, 3] = 1 - cbase          # kx offset
    bw = np.ascontiguousarray(bw.reshape(128, NS * 4))
    idn = np.eye(128, dtype=np.float16)

    I1h = I1f.astype(np.float16)
    core_inputs, T0s = [], []
    for core in range(8):
        sl = slice(2 * core, 2 * core + 2)
        inp = {}
        inp["rawp"] = np.ascontiguousarray(rawp[sl].reshape(2, 4, (H + 2) * (W + 2)))
        inp["x1m"] = mod128_cols_batch(X1[sl])
        inp["x1w"] = wrap16_cols_batch(X1[sl], 3)
        inp["i1m"] = mod128_cols_batch(I1h[sl])
        inp["bw"] = bw
        inp["idn"] = idn
        q = np.zeros((2, 16), np.float32)
        q[:, :9] = T0[sl, :3, :3].reshape(2, 9)
        q[:, 9:12] = T0[sl, :3, 3]
        q[:, 12:16] = intr[sl]
        rtm = np.zeros((128, 16, 8), np.float32)
        rtw = np.zeros((128, 16), np.float32)
        for g in range(8):
            bb = g // 4
            rtm[:, :, g] = q[bb][None, :]
            rtw[16 * g:16 * g + 16, :] = q[bb][None, :]
        inp["rtm"] = np.ascontiguousarray(rtm.reshape(128, 16 * 8))
        inp["rtw"] = rtw
        inp["t0q"] = np.ascontiguousarray(T0[sl].reshape(2, 16).astype(np.float32))
        inp["intr2"] = np.ascontiguousarray(intr[sl].astype(np.float32))
        core_inputs.append(inp)
        T0s.append(T0[sl])
    return core_inputs, T0s


_NC_CACHE = {}
PROFILE = False
LAST_EXEC_NS = []
LAST_TRACES = []
LAST_WALL = []


def build_nc():
    import concourse.bacc as bacc
    import concourse.bass as bass
    import concourse.tile as tile
    from concourse import mybir

    fp32 = mybir.dt.float32
    fp16 = mybir.dt.float16
    i16 = mybir.dt.int16
    i32 = mybir.dt.int32
    AL = mybir.AluOpType
    ACT = mybir.ActivationFunctionType
    AX = mybir.AxisListType

    nc = bacc.Bacc("TRN2", target_bir_lowering=False, debug=False, num_devices=8)

    rawp_in = nc.dram_tensor("rawp", [2, 4, (H + 2) * (W + 2)], fp16, kind="ExternalInput")
    pd_in = nc.dram_tensor("pds12", [2, 12, HW + 2], fp16, kind="Internal")
    x1m_in = nc.dram_tensor("x1m", [128, NS * 8 * A * 3], fp16, kind="ExternalInput")
    x1w_in = nc.dram_tensor("x1w", [128, NS * M * 3], fp16, kind="ExternalInput")
    i1m_in = nc.dram_tensor("i1m", [128, NS * 8 * A * 3], fp16, kind="ExternalInput")
    bw_in = nc.dram_tensor("bw", [128, NS * 4], fp32, kind="ExternalInput")
    idn_in = nc.dram_tensor("idn", [128, 128], fp16, kind="ExternalInput")
    rtm_in = nc.dram_tensor("rtm", [128, 16 * 8], fp32, kind="ExternalInput")
    rtw_in = nc.dram_tensor("rtw", [128, 16], fp32, kind="ExternalInput")
    t0_in = nc.dram_tensor("t0q", [2, 16], fp32, kind="ExternalInput")
    intr_in = nc.dram_tensor("intr2", [2, 4], fp32, kind="ExternalInput")
    tout_ext = nc.dram_tensor("tout", [2, 16], fp32, kind="ExternalOutput")
    qscr = nc.dram_tensor("qscr", [2, 16], fp32, kind="Internal")

    with tile.TileContext(nc) as tc:
        with tc.tile_pool(name="cst", bufs=1) as cpool, \
             tc.tile_pool(name="tblp", bufs=1) as tpool, \
             tc.tile_pool(name="strm", bufs=2) as sp, \
             tc.tile_pool(name="scr", bufs=1) as sc, \
             tc.tile_pool(name="gath", bufs=1) as gp, \
             tc.tile_pool(name="ps", bufs=2, space="PSUM") as pp, \
             tc.tile_pool(name="jp", bufs=1, space="PSUM") as jp:

            rtm = cpool.tile([128, 16 * 8], fp32, tag="rtm")
            rtm0 = cpool.tile([128, 16 * 8], fp32, tag="rtm0")
            rtw = cpool.tile([128, 16], fp32, tag="rtw")
            bwc = cpool.tile([128, NS * 4], fp32, tag="bw")
            idn = cpool.tile([128, 128], fp16, tag="idn")
            Tq = cpool.tile([2, 16], fp32, tag="Tq")
            intr = cpool.tile([2, 4], fp32, tag="intr")
            nc.sync.dma_start(out=rtm[:, :], in_=rtm_in.ap())
            nc.sync.dma_start(out=rtm0[:, :], in_=rtm_in.ap())
            nc.sync.dma_start(out=rtw[:, :], in_=rtw_in.ap())
            nc.sync.dma_start(out=bwc[:, :], in_=bw_in.ap())
            nc.sync.dma_start(out=idn[:, :], in_=idn_in.ap())
            nc.sync.dma_start(out=Tq[:, :], in_=t0_in.ap())
            nc.sync.dma_start(out=intr[:, :], in_=intr_in.ap())

            psJ = [jp.tile([28, 28], fp32, name=f"psJ{b}", tag=f"psJ{b}") for b in range(2)]

            # one-time: 12 table planes (grads + raw) from padded raw planes
            WP = W + 2
            zt1 = cpool.tile([1, 1], fp16, tag="zt1")
            nc.vector.memset(zt1[:, :], 0.0)
            pda = pd_in.ap()
            for b2 in range(2):
                for pl in range(12):
                    for zo in (0, HW + 1):
                        zdst = bass.AP(pda.tensor,
                                       pda.offset + (b2 * 12 + pl) * (HW + 2) + zo,
                                       [[1, 1], [1, 1]])
                        nc.sync.dma_start(out=zdst, in_=zt1[:, :])
            tA0 = cpool.tile([128, WP], fp16, tag="tA0")
            tU0 = cpool.tile([128, WP], fp16, tag="tU0")
            tD0 = cpool.tile([128, WP], fp16, tag="tD0")
            go0 = cpool.tile([128, W], fp16, tag="go0")
            rpa = rawp_in.ap()
            for b2 in range(2):
                for c4 in range(4):
                    pbase = rpa.offset + (b2 * 4 + c4) * (H + 2) * WP
                    for r0, nr in ((0, 128), (128, 128), (256, 64)):
                        for t_, roff in ((tA0, r0 + 1), (tU0, r0), (tD0, r0 + 2)):
                            srcap = bass.AP(rpa.tensor, pbase + roff * WP,
                                            [[WP, nr], [1, WP]])
                            nc.sync.dma_start(out=t_[:nr, :], in_=srcap)
                        pdx = c4 if c4 < 3 else 6
                        pdy = 3 + c4 if c4 < 3 else 7
                        pcp = 8 + c4 if c4 < 3 else 11
                        dbase = pda.offset + 1 + r0 * W
                        # dx = 0.5*(A[:,2:] - A[:,:2-])
                        nc.vector.tensor_tensor(go0[:nr, :], tA0[:nr, 2:WP], tA0[:nr, 0:W], op=AL.subtract)
                        nc.vector.tensor_scalar(go0[:nr, :], go0[:nr, :], 0.5, None, AL.mult)
                        ddst = bass.AP(pda.tensor, dbase + (b2 * 12 + pdx) * (HW + 2),
                                       [[W, nr], [1, W]])
                        nc.sync.dma_start(out=ddst, in_=go0[:nr, :])
                        # dy = 0.5*(D[:,1:W+1] - U[:,1:W+1])
                        nc.vector.tensor_tensor(go0[:nr, :], tD0[:nr, 1:W + 1], tU0[:nr, 1:W + 1], op=AL.subtract)
                        nc.vector.tensor_scalar(go0[:nr, :], go0[:nr, :], 0.5, None, AL.mult)
                        ddst = bass.AP(pda.tensor, dbase + (b2 * 12 + pdy) * (HW + 2),
                                       [[W, nr], [1, W]])
                        nc.sync.dma_start(out=ddst, in_=go0[:nr, :])
                        # raw copy plane
                        ddst = bass.AP(pda.tensor, dbase + (b2 * 12 + pcp) * (HW + 2),
                                       [[W, nr], [1, W]])
                        nc.sync.dma_start(out=ddst, in_=tA0[:nr, 1:W + 1])

            tbl0 = tpool.tile([128, NELEM * 2], fp16, tag="tbl")
            nc.vector.memset(tbl0[:, :], 0.0)
            stbl0 = tpool.tile([128, 34 * (TC + 1)], fp16, tag="stbl")
            nc.vector.memset(stbl0[:, :], 0.0)

            def rq(qi):   # mod-128 per-group broadcast: dims (g x8, a x A step0)
                sl = rtm[:, qi * 8:(qi + 1) * 8]
                return bass.AP(sl.tensor, sl.offset, [list(sl.ap[0]), [1, 8], [0, A]])

            def rqw(qi):  # wrapped per-partition scalar bcast over M
                sl = rtw[:, qi:qi + 1]
                return bass.AP(sl.tensor, sl.offset, [list(sl.ap[0]), [0, M]])

            def rtwS(qi):  # wrapped per-partition scalar [128,1]
                return rtw[:, qi:qi + 1]

            def bwq(s, j):
                sl = bwc[:, s * 4 + j:s * 4 + j + 1]
                return bass.AP(sl.tensor, sl.offset, [list(sl.ap[0]), [0, M]])

            def bwS(s, j):
                return bwc[:, s * 4 + j:s * 4 + j + 1]

            TT = nc.vector.tensor_tensor
            TS = lambda out, in0, s1, op: nc.vector.tensor_scalar(out, in0, s1, None, op)
            TS2 = lambda out, in0, s1, s2, op0, op1: nc.vector.tensor_scalar(out, in0, s1, s2, op0, op1)
            STT = nc.vector.scalar_tensor_tensor

            with tc.For_i(0, N_ITERS) as _it:
                for s in range(NS):
                    tbl = tbl0
                    for r0, nr in ((0, 34), (34, 33)):
                        for g in range(8):
                            b, yb, xh = chunk_of(g, s)
                            rbase, cbase = bases_of(yb, xh)
                            start = (rbase + r0) * W + cbase
                            src0 = pd_in.ap()
                            src = bass.AP(src0.tensor,
                                          src0.offset + b * 12 * (HW + 2) + start,
                                          [[HW + 2, 12], [W, nr], [1, TC + 1]])
                            dsl = stbl0[16 * g:16 * g + 12, :]
                            dst = bass.AP(dsl.tensor, dsl.offset,
                                          [[dsl.ap[0][0], 12], [TC + 1, nr], [1, TC + 1]])
                            nc.sync.dma_start(out=dst, in_=src)
                        for e in range(2):
                            pout = bass.AP(tbl.tensor, tbl.offset + e + r0 * 2 * TC,
                                           [list(tbl.ap[0]), [2 * TC, nr], [2, TC]])
                            pin = bass.AP(stbl0.tensor, stbl0.offset + e,
                                          [list(stbl0.ap[0]), [TC + 1, nr], [1, TC]])
                            nc.scalar.activation(pout, pin, ACT.Copy)

                    x1w = sp.tile([128, M * 3], fp16, tag="x1w")
                    nc.sync.dma_start(out=x1w[:, :], in_=x1w_in.ap()[:, s * M * 3:(s + 1) * M * 3])
                    x1m = sp.tile([128, 8 * A * 3], fp16, tag="x1m")
                    nc.sync.dma_start(out=x1m[:, :], in_=x1m_in.ap()[:, s * 8 * A * 3:(s + 1) * 8 * A * 3])
                    i1 = sp.tile([128, 8 * A * 3], fp16, tag="i1")
                    nc.sync.dma_start(out=i1[:, :], in_=i1m_in.ap()[:, s * 8 * A * 3:(s + 1) * 8 * A * 3])

                    # ---------- wrapped-16 idx pipeline ----------
                    def xw(k):
                        sl = x1w[:, :]
                        return bass.AP(sl.tensor, sl.offset + k, [list(sl.ap[0]), [3, M]])

                    def tw(name):
                        return sc.tile([128, M], fp32, name="w_" + name + f"_{s}", tag="w_" + name)

                    t1w = tw("t1")
                    X0zw = tw("X0z")
                    STT(X0zw[:, :], xw(0), rtwS(6), rqw(11), AL.mult, AL.add)
                    STT(X0zw[:, :], xw(1), rtwS(7), X0zw[:, :], AL.mult, AL.add)
                    STT(X0zw[:, :], xw(2), rtwS(8), X0zw[:, :], AL.mult, AL.add)
                    X0xw = tw("X0x")
                    STT(X0xw[:, :], xw(0), rtwS(0), rqw(9), AL.mult, AL.add)
                    STT(X0xw[:, :], xw(1), rtwS(1), X0xw[:, :], AL.mult, AL.add)
                    STT(X0xw[:, :], xw(2), rtwS(2), X0xw[:, :], AL.mult, AL.add)
                    X0yw = tw("X0y")
                    STT(X0yw[:, :], xw(0), rtwS(3), rqw(10), AL.mult, AL.add)
                    STT(X0yw[:, :], xw(1), rtwS(4), X0yw[:, :], AL.mult, AL.add)
                    STT(X0yw[:, :], xw(2), rtwS(5), X0yw[:, :], AL.mult, AL.add)

                    izw = tw("iz")
                    TS(t1w[:, :], X0zw[:, :], EPS, AL.max)
                    nc.vector.reciprocal_approx_fast(izw[:, :], t1w[:, :])
                    u0w = tw("u0"); v0w = tw("v0")
                    TT(u0w[:, :], X0xw[:, :], izw[:, :], op=AL.mult)
                    STT(u0w[:, :], u0w[:, :], rtwS(12), rqw(14), AL.mult, AL.add)
                    TT(v0w[:, :], X0yw[:, :], izw[:, :], op=AL.mult)
                    STT(v0w[:, :], v0w[:, :], rtwS(13), rqw(15), AL.mult, AL.add)
                    TS2(u0w[:, :], u0w[:, :], -0.5 * (W - 1), 1.5 * (W - 1), AL.max, AL.min)
                    TS2(v0w[:, :], v0w[:, :], -0.5 * (H - 1), 1.5 * (H - 1), AL.max, AL.min)
                    x0fw = tw("x0f"); y0fw = tw("y0f")
                    fi32w = sc.tile([128, M], i32, name=f"fi32w_{s}", tag="fi32w")
                    TS(t1w[:, :], u0w[:, :], 0.5, AL.subtract)
                    nc.vector.tensor_copy(fi32w[:, :], t1w[:, :])
                    nc.vector.tensor_copy(x0fw[:, :], fi32w[:, :])
                    TS(t1w[:, :], v0w[:, :], 0.5, AL.subtract)
                    nc.vector.tensor_copy(fi32w[:, :], t1w[:, :])
                    nc.vector.tensor_copy(y0fw[:, :], fi32w[:, :])
                    xfw = t1w; kxw = izw; yrw = X0zw
                    ktw = X0xw; kbw = X0yw
                    STT(xfw[:, :], x0fw[:, :], bwS(s, 1), bwq(s, 2), AL.max, AL.min)
                    nc.vector.tensor_scalar(kxw[:, :], xfw[:, :], bwS(s, 3), None, AL.add)
                    nc.vector.tensor_scalar(yrw[:, :], y0fw[:, :], bwS(s, 0), 0.0, AL.subtract, AL.max)
                    TS2(ktw[:, :], yrw[:, :], float(TR - 1), float(TC), AL.min, AL.mult)
                    TT(ktw[:, :], ktw[:, :], kxw[:, :], op=AL.add)
                    TS2(kbw[:, :], yrw[:, :], 1.0, float(TR - 1), AL.add, AL.min)
                    TS(kbw[:, :], kbw[:, :], float(TC), AL.mult)
                    TT(kbw[:, :], kbw[:, :], kxw[:, :], op=AL.add)
                    kidx = sc.tile([128, 2 * M], i16, name=f"kidx_{s}", tag="kidx")
                    nc.vector.tensor_copy(kidx[:, :M], ktw[:, :])
                    nc.vector.tensor_copy(kidx[:, M:], kbw[:, :])

                    gt2 = gp.tile([128, 2 * N * 2], fp16, tag="gt2")
                    nc.gpsimd.ap_gather(gt2[:, :], tbl[:, :], kidx[:, :],
                                        channels=128, num_elems=NELEM, d=2, num_idxs=2 * N)

                    # ---------- mod-128 warp pipeline ----------
                    def xm(k):
                        sl = x1m[:, :]
                        return bass.AP(sl.tensor, sl.offset + k, [list(sl.ap[0]), [3, 8 * A]])

                    def tm(name):
                        return sc.tile([128, 8 * A], fp32, name="m_" + name + f"_{s}", tag="m_" + name)

                    def matvec(dst, aps, qis, t1):
                        TT(dst[:, :], aps[0], qis[0], op=AL.mult)
                        TT(t1[:, :], aps[1], qis[1], op=AL.mult)
                        TT(dst[:, :], dst[:, :], t1[:, :], op=AL.add)
                        TT(t1[:, :], aps[2], qis[2], op=AL.mult)
                        TT(dst[:, :], dst[:, :], t1[:, :], op=AL.add)
                        TT(dst[:, :], dst[:, :], qis[3], op=AL.add)

                    # ---- on-device A6/B6/T6 at the initial pose (rtm0) ----
                    abt = sc.tile([128, 8 * A * 18], fp16, name=f"abt_{s}", tag="abt")

                    def acol(k):
                        sl = abt[:, :]
                        return bass.AP(sl.tensor, sl.offset + k, [list(sl.ap[0]), [18, 224]])

                    def rq0(qi):
                        sl = rtm0[:, qi * 8:(qi + 1) * 8]
                        return bass.AP(sl.tensor, sl.offset, [list(sl.ap[0]), [1, 8], [0, A]])

                    j1 = tm("j1"); j2 = tm("j2")
                    jx = tm("jx"); jy = tm("jy"); jz = tm("jz"); jiz = tm("jiz")
                    matvec(jz, [xm(0), xm(1), xm(2)], [rq0(6), rq0(7), rq0(8), rq0(11)], j1)
                    matvec(jx, [xm(0), xm(1), xm(2)], [rq0(0), rq0(1), rq0(2), rq0(9)], j1)
                    matvec(jy, [xm(0), xm(1), xm(2)], [rq0(3), rq0(4), rq0(5), rq0(10)], j1)
                    TS(j1[:, :], jz[:, :], EPS, AL.max)
                    nc.vector.reciprocal_approx_fast(jiz[:, :], j1[:, :])
                    fxiz = tm("fxiz"); fyiz = tm("fyiz"); zizt = tm("zizt")
                    A2t = tm("A2t"); B2t = tm("B2t")
                    TT(fxiz[:, :], jiz[:, :], rq0(12), op=AL.mult)
                    TT(fyiz[:, :], jiz[:, :], rq0(13), op=AL.mult)
                    TT(zizt[:, :], jz[:, :], jiz[:, :], op=AL.mult)
                    TT(j1[:, :], jx[:, :], jiz[:, :], op=AL.mult)
                    TT(A2t[:, :], fxiz[:, :], j1[:, :], op=AL.mult)
                    TT(j1[:, :], jy[:, :], jiz[:, :], op=AL.mult)
                    TT(B2t[:, :], fyiz[:, :], j1[:, :], op=AL.mult)
                    TS(acol(0), fxiz[:, :], -1.0, AL.mult)
                    TS(acol(1), fxiz[:, :], 0.0, AL.mult)
                    nc.vector.tensor_copy(acol(2), A2t[:, :])
                    TT(acol(3), A2t[:, :], jy[:, :], op=AL.mult)
                    TT(j1[:, :], zizt[:, :], rq0(12), op=AL.mult)
                    TT(j2[:, :], A2t[:, :], jx[:, :], op=AL.mult)
                    TT(j1[:, :], j1[:, :], j2[:, :], op=AL.add)
                    TS(acol(4), j1[:, :], -1.0, AL.mult)
                    TT(acol(5), fxiz[:, :], jy[:, :], op=AL.mult)
                    TS(acol(6), fxiz[:, :], 0.0, AL.mult)
                    TS(acol(7), fyiz[:, :], -1.0, AL.mult)
                    nc.vector.tensor_copy(acol(8), B2t[:, :])
                    TT(j1[:, :], zizt[:, :], rq0(13), op=AL.mult)
                    TT(j2[:, :], B2t[:, :], jy[:, :], op=AL.mult)
                    TT(acol(9), j1[:, :], j2[:, :], op=AL.add)
                    TT(j1[:, :], B2t[:, :], jx[:, :], op=AL.mult)
                    TS(acol(10), j1[:, :], -1.0, AL.mult)
                    TT(j1[:, :], fyiz[:, :], jx[:, :], op=AL.mult)
                    TS(acol(11), j1[:, :], -1.0, AL.mult)
                    TS(acol(12), fxiz[:, :], 0.0, AL.mult)
                    TS(acol(13), fxiz[:, :], 0.0, AL.mult)
                    TS2(acol(14), fxiz[:, :], 0.0, 1.0, AL.mult, AL.add)
                    nc.vector.tensor_copy(acol(15), jy[:, :])
                    TS(acol(16), jx[:, :], -1.0, AL.mult)
                    TS(acol(17), fxiz[:, :], 0.0, AL.mult)

                    m1 = j1; m2 = j2
                    X0z = jz
                    matvec(X0z, [xm(0), xm(1), xm(2)], [rq(6), rq(7), rq(8), rq(11)], m1)
                    X0x = jx
                    matvec(X0x, [xm(0), xm(1), xm(2)], [rq(0), rq(1), rq(2), rq(9)], m1)
                    X0y = jy
                    matvec(X0y, [xm(0), xm(1), xm(2)], [rq(3), rq(4), rq(5), rq(10)], m1)
                    iz = jiz
                    TS(m1[:, :], X0z[:, :], EPS, AL.max)
                    nc.vector.reciprocal_approx_fast(iz[:, :], m1[:, :])
                    u0 = fxiz; v0 = fyiz
                    TT(u0[:, :], X0x[:, :], iz[:, :], op=AL.mult)
                    TT(u0[:, :], u0[:, :], rq(12), op=AL.mult)
                    TT(u0[:, :], u0[:, :], rq(14), op=AL.add)
                    TT(v0[:, :], X0y[:, :], iz[:, :], op=AL.mult)
                    TT(v0[:, :], v0[:, :], rq(13), op=AL.mult)
                    TT(v0[:, :], v0[:, :], rq(15), op=AL.add)
                    vmask = zizt
                    TS(vmask[:, :], X0z[:, :], EPS, AL.is_gt)
                    STT(vmask[:, :], u0[:, :], 0.0, vmask[:, :], AL.is_gt, AL.mult)
                    STT(vmask[:, :], u0[:, :], float(W - 1), vmask[:, :], AL.is_lt, AL.mult)
                    STT(vmask[:, :], v0[:, :], 0.0, vmask[:, :], AL.is_gt, AL.mult)
                    STT(vmask[:, :], v0[:, :], float(H - 1), vmask[:, :], AL.is_lt, AL.mult)
                    TS2(u0[:, :], u0[:, :], -0.5 * (W - 1), 1.5 * (W - 1), AL.max, AL.min)
                    TS2(v0[:, :], v0[:, :], -0.5 * (H - 1), 1.5 * (H - 1), AL.max, AL.min)
                    wx = A2t; wy = B2t; x0f = tm("x0f"); y0f = tm("y0f")
                    fi32m = sc.tile([128, 8 * A], i32, name=f"fi32m_{s}", tag="fi32m")
                    TS(m1[:, :], u0[:, :], 0.5, AL.subtract)
                    nc.vector.tensor_copy(fi32m[:, :], m1[:, :])
                    nc.vector.tensor_copy(x0f[:, :], fi32m[:, :])
                    TT(wx[:, :], u0[:, :], x0f[:, :], op=AL.subtract)
                    TS(m1[:, :], v0[:, :], 0.5, AL.subtract)
                    nc.vector.tensor_copy(fi32m[:, :], m1[:, :])
                    nc.vector.tensor_copy(y0f[:, :], fi32m[:, :])
                    TT(wy[:, :], v0[:, :], y0f[:, :], op=AL.subtract)
                    mx0 = tm("mx0"); mx1 = tm("mx1"); my0 = tm("my0"); my1 = tm("my1")
                    TS(mx0[:, :], x0f[:, :], -0.5, AL.is_gt)
                    STT(mx0[:, :], x0f[:, :], float(W - 1) + 0.5, mx0[:, :], AL.is_lt, AL.mult)
                    TS(mx1[:, :], x0f[:, :], -1.5, AL.is_gt)
                    STT(mx1[:, :], x0f[:, :], float(W - 2) + 0.5, mx1[:, :], AL.is_lt, AL.mult)
                    TS(my0[:, :], y0f[:, :], -0.5, AL.is_gt)
                    STT(my0[:, :], y0f[:, :], float(H - 1) + 0.5, my0[:, :], AL.is_lt, AL.mult)
                    TS(my1[:, :], y0f[:, :], -1.5, AL.is_gt)
                    STT(my1[:, :], y0f[:, :], float(H - 2) + 0.5, my1[:, :], AL.is_lt, AL.mult)
                    W00 = tm("W00"); W01 = tm("W01"); W10 = tm("W10"); W11 = tm("W11")
                    TS2(m1[:, :], wx[:, :], 1.0, -1.0, AL.subtract, AL.mult)  # 1-wx
                    TS2(m2[:, :], wy[:, :], 1.0, -1.0, AL.subtract, AL.mult)  # 1-wy
                    TT(W00[:, :], m1[:, :], m2[:, :], op=AL.mult)
                    TT(W00[:, :], W00[:, :], mx0[:, :], op=AL.mult)
                    TT(W00[:, :], W00[:, :], my0[:, :], op=AL.mult)
                    TT(W01[:, :], wx[:, :], m2[:, :], op=AL.mult)
                    TT(W01[:, :], W01[:, :], mx1[:, :], op=AL.mult)
                    TT(W01[:, :], W01[:, :], my0[:, :], op=AL.mult)
                    TT(W10[:, :], m1[:, :], wy[:, :], op=AL.mult)
                    TT(W10[:, :], W10[:, :], mx0[:, :], op=AL.mult)
                    TT(W10[:, :], W10[:, :], my1[:, :], op=AL.mult)
                    TT(W11[:, :], wx[:, :], wy[:, :], op=AL.mult)
                    TT(W11[:, :], W11[:, :], mx1[:, :], op=AL.mult)
                    TT(W11[:, :], W11[:, :], my1[:, :], op=AL.mult)

                    # ---------- PE transpose + combine ----------
                    samp = sc.tile([128, A * 128], fp16, tag="samp")
                    ctmp = sc.tile([128, 512], fp16, tag="ctmp")
                    for a4 in range(A // 4):
                        ptall = pp.tile([128, 2048], fp16, tag="ptall")
                        for ci, base in enumerate((0, 1, 2 * N, 2 * N + 1)):
                            for aa in range(4):
                                a = a4 * 4 + aa
                                src = bass.AP(gt2.tensor, gt2.offset + base + a * 256,
                                              [list(gt2.ap[0]), [2, 128]])
                                nc.tensor.transpose(
                                    ptall[:, ci * 512 + aa * 128:ci * 512 + (aa + 1) * 128],
                                    src, idn[:, :])
                        for ci, wt_ in ((0, W00), (1, W01), (2, W10), (3, W11)):
                            pap = bass.AP(ptall.tensor, ptall.offset + ci * 512,
                                          [list(ptall.ap[0]), [128, 4], [16, 8], [1, 16]])
                            woff = wt_.offset + a4 * 4
                            wap = bass.AP(wt_.tensor, woff, [list(wt_.ap[0]), [1, 4], [A, 8], [0, 16]])
                            dst_off = samp.offset + a4 * 4 * 128
                            dap = bass.AP(samp.tensor, dst_off, [list(samp.ap[0]), [128, 4], [16, 8], [1, 16]])
                            if ci == 0:
                                TT(dap, pap, wap, op=AL.mult)
                            else:
                                tap = bass.AP(ctmp.tensor, ctmp.offset, [list(ctmp.ap[0]), [128, 4], [16, 8], [1, 16]])
                                TT(tap, pap, wap, op=AL.mult)
                                TT(dap, dap, tap, op=AL.add)

                    # ---------- residuals, huber weights, G build ----------
                    def sq(q):
                        sl = samp[:, :]
                        return bass.AP(sl.tensor, sl.offset + q, [list(sl.ap[0]), [16, 8], [128, A]])

                    def i1q(c):
                        sl = i1[:, :]
                        return bass.AP(sl.tensor, sl.offset + c, [list(sl.ap[0]), [3 * A, 8], [3, A]])

                    Gt = sc.tile([128, 28 * 224], fp16, tag="Gt")
                    g6a = sc.tile([128, 6 * 224], fp16, tag="g6a")
                    g6b = sc.tile([128, 6 * 224], fp16, tag="g6b")
                    one_m = tm("one_m")
                    TS2(one_m[:, :], vmask[:, :], 1.0, -1e-6, AL.subtract, AL.mult)  # (1-vm)*1e-6
                    rr = tm("rr"); bb_ = tm("bb"); ss = tm("ss")
                    ppv = tm("ppv"); qqv = tm("qqv")

                    def abt6(k0):  # [x(6) outer, chunk(224) inner], stride 18 per chunk
                        sl = abt[:, :]
                        return bass.AP(sl.tensor, sl.offset + k0, [list(sl.ap[0]), [1, 6], [18, 224]])

                    def gcols(c):  # G cols c*7 .. c*7+5: [x outer, chunk inner]
                        sl = Gt[:, :]
                        return bass.AP(sl.tensor, sl.offset + c * 7 * 224, [list(sl.ap[0]), [224, 6], [1, 224]])

                    def bc6(t):    # broadcast [128,224] over 6 x-cols
                        sl = t[:, :]
                        return bass.AP(sl.tensor, sl.offset, [list(sl.ap[0]), [0, 6], [1, 224]])

                    for c in range(3):
                        TT(rr[:, :], i1q(c), sq(8 + c), op=AL.subtract)
                        TT(rr[:, :], rr[:, :], vmask[:, :], op=AL.mult)
                        TT(rr[:, :], rr[:, :], one_m[:, :], op=AL.add)
                        nc.scalar.activation(bb_[:, :], rr[:, :], ACT.Abs)
                        TS(bb_[:, :], bb_[:, :], HUBER_DELTA, AL.max)
                        nc.vector.reciprocal_approx_fast(bb_[:, :], bb_[:, :])
                        nc.scalar.activation(ss[:, :], bb_[:, :], ACT.Sqrt, scale=HUBER_DELTA)
                        TT(ppv[:, :], ss[:, :], sq(0 + c), op=AL.mult)
                        TT(qqv[:, :], ss[:, :], sq(3 + c), op=AL.mult)
                        TT(g6a[:, :], abt6(0), bc6(ppv), op=AL.mult)
                        TT(g6b[:, :], abt6(6), bc6(qqv), op=AL.mult)
                        TT(gcols(c), g6a[:, :], g6b[:, :], op=AL.add)
                        TT(Gt[:, (c * 7 + 6) * 224:(c * 7 + 7) * 224], ss[:, :], rr[:, :], op=AL.mult)
                    # depth channel
                    TT(rr[:, :], iz[:, :], sq(11), op=AL.subtract)
                    TT(rr[:, :], rr[:, :], vmask[:, :], op=AL.mult)
                    TT(rr[:, :], rr[:, :], one_m[:, :], op=AL.add)
                    nc.scalar.activation(bb_[:, :], rr[:, :], ACT.Abs, scale=LAMBDA)
                    TS(bb_[:, :], bb_[:, :], HUBER_DELTA, AL.max)
                    nc.vector.reciprocal_approx_fast(bb_[:, :], bb_[:, :])
                    nc.scalar.activation(ss[:, :], bb_[:, :], ACT.Sqrt,
                                         scale=HUBER_DELTA * LAMBDA * LAMBDA)
                    TT(ppv[:, :], ss[:, :], sq(6), op=AL.mult)
                    TT(qqv[:, :], ss[:, :], sq(7), op=AL.mult)
                    TT(g6a[:, :], abt6(0), bc6(ppv), op=AL.mult)
                    TT(g6b[:, :], abt6(6), bc6(qqv), op=AL.mult)
                    TT(g6a[:, :], g6a[:, :], g6b[:, :], op=AL.add)
                    TT(g6b[:, :], abt6(12), bc6(ss), op=AL.mult)
                    TT(gcols(3), g6a[:, :], g6b[:, :], op=AL.add)
                    TT(Gt[:, (3 * 7 + 6) * 224:(3 * 7 + 7) * 224], ss[:, :], rr[:, :], op=AL.mult)

                    # ---------- PE: JtWJ accumulation ----------
                    for g in range(8):
                        b = g // 4
                        for a in range(A):
                            off = Gt.offset + g * A + a
                            gap = bass.AP(Gt.tensor, off, [list(Gt.ap[0]), [224, 28]])
                            first = (s == 0 and (g % 4) == 0 and a == 0)
                            last = (s == NS - 1 and (g % 4) == 3 and a == A - 1)
                            nc.tensor.matmul(psJ[b][:, :], gap, gap,
                                             start=first, stop=last,
                                             skip_group_check=True)

                # ---------- per-iteration: extract JtWJ/Rhs, solve, update pose ----------
                S28 = sc.tile([28, 56], fp32, tag="S28")
                for b in range(2):
                    nc.vector.tensor_copy(S28[:, b * 28:(b + 1) * 28], psJ[b][:, :])
                D28 = sc.tile([7, 56], fp32, tag="D28")
                for b in range(2):
                    for c in range(4):
                        src = S28[c * 7:(c + 1) * 7, b * 28 + c * 7:b * 28 + c * 7 + 7]
                        dsl = D28[:, b * 28 + c * 7:b * 28 + c * 7 + 7]
                        nc.sync.dma_start(out=dsl, in_=src)
                M7 = sc.tile([7, 14], fp32, tag="M7")
                for b in range(2):
                    din = bass.AP(D28.tensor, D28.offset + b * 28,
                                  [list(D28.ap[0]), [1, 7], [7, 4]])
                    nc.vector.tensor_reduce(M7[:, b * 7:(b + 1) * 7], din, axis=AX.X, op=AL.add)
                # Mb [2, 49]: row b = M7 block b flattened (x-major)
                Mb = sc.tile([2, 49], fp32, tag="Mb")
                for b in range(2):
                    msrc = bass.AP(M7.tensor, M7.offset + b * 7, [[M7.ap[0][0], 7], [1, 7]])
                    mdsl = Mb[b:b + 1, 0:1]
                    mdst = bass.AP(mdsl.tensor, mdsl.offset, [[Mb.ap[0][0], 1], [7, 7], [1, 7]])
                    nc.sync.dma_start(out=mdst, in_=msrc)
                # tr = sum diag(JtWJ); LM ridge on diag
                trt = sc.tile([2, 1], fp32, tag="trt")
                diag = bass.AP(Mb.tensor, Mb.offset, [list(Mb.ap[0]), [8, 6]])
                nc.vector.tensor_reduce(trt[:, :], diag, axis=AX.X, op=AL.add)
                trb = bass.AP(trt.tensor, trt.offset, [list(trt.ap[0]), [0, 6]])
                STT(diag, trb, 1e-6, diag, AL.mult, AL.add)

                # Cholesky LL^T = Hm (6x6, both batches in 2 partitions)
                Lt = sc.tile([2, 36], fp32, tag="Lt")
                lsrc = bass.AP(Mb.tensor, Mb.offset, [list(Mb.ap[0]), [7, 6], [1, 6]])
                nc.vector.tensor_copy(Lt[:, :], lsrc)
                rhs = sc.tile([2, 6], fp32, tag="rhs")
                rsrc = bass.AP(Mb.tensor, Mb.offset + 6, [list(Mb.ap[0]), [7, 6]])
                nc.vector.tensor_copy(rhs[:, :], rsrc)
                idg = sc.tile([2, 6], fp32, tag="idg")
                tmpj = sc.tile([2, 36], fp32, tag="tmpj")
                red = sc.tile([2, 6], fp32, tag="redj")
                for j in range(6):
                    jj = Lt[:, 6 * j + j:6 * j + j + 1]
                    if j > 0:
                        ljk = Lt[:, 6 * j:6 * j + j]
                        TT(tmpj[:, :j], ljk, ljk, op=AL.mult)
                        nc.vector.tensor_reduce(red[:, 0:1], tmpj[:, :j], axis=AX.X, op=AL.add)
                        TT(jj, jj, red[:, 0:1], op=AL.subtract)
                    nc.scalar.activation(jj, jj, ACT.Sqrt)
                    nc.vector.reciprocal(idg[:, j:j + 1], jj)
                    nr = 5 - j
                    if nr > 0:
                        colap = bass.AP(Lt.tensor, Lt.offset + 6 * (j + 1) + j, [list(Lt.ap[0]), [6, nr]])
                        if j > 0:
                            lik = bass.AP(Lt.tensor, Lt.offset + 6 * (j + 1), [list(Lt.ap[0]), [6, nr], [1, j]])
                            ljkb = bass.AP(Lt.tensor, Lt.offset + 6 * j, [list(Lt.ap[0]), [0, nr], [1, j]])
                            TT(tmpj[:, :nr * j], lik, ljkb, op=AL.mult)
                            tin = bass.AP(tmpj.tensor, tmpj.offset, [list(tmpj.ap[0]), [j, nr], [1, j]])
                            nc.vector.tensor_reduce(red[:, :nr], tin, axis=AX.X, op=AL.add)
                            TT(colap, colap, red[:, :nr], op=AL.subtract)
                        nc.vector.tensor_scalar(colap, colap, idg[:, j:j + 1], None, AL.mult)
                # forward substitution: L y = rhs (in place on rhs)
                for j in range(6):
                    yj = rhs[:, j:j + 1]
                    if j > 0:
                        ljk = Lt[:, 6 * j:6 * j + j]
                        TT(tmpj[:, :j], ljk, rhs[:, :j], op=AL.mult)
                        nc.vector.tensor_reduce(red[:, 0:1], tmpj[:, :j], axis=AX.X, op=AL.add)
                        TT(yj, yj, red[:, 0:1], op=AL.subtract)
                    nc.vector.tensor_scalar(yj, yj, idg[:, j:j + 1], None, AL.mult)
                # back substitution: L^T x = y -> xi = -x stored in xi tile
                for j in range(5, -1, -1):
                    xj = rhs[:, j:j + 1]
                    nk = 5 - j
                    if nk > 0:
                        lkj = bass.AP(Lt.tensor, Lt.offset + 6 * (j + 1) + j, [list(Lt.ap[0]), [6, nk]])
                        TT(tmpj[:, :nk], lkj, rhs[:, j + 1:6], op=AL.mult)
                        nc.vector.tensor_reduce(red[:, 0:1], tmpj[:, :nk], axis=AX.X, op=AL.add)
                        TT(xj, xj, red[:, 0:1], op=AL.subtract)
                    nc.vector.tensor_scalar(xj, xj, idg[:, j:j + 1], None, AL.mult)
                xi = sc.tile([2, 6], fp32, tag="xi")
                TS(xi[:, :], rhs[:, :], -1.0, AL.mult)

                # se3_exp(xi) via Taylor series (|w| << 1 in this regime)
                w3 = xi[:, 3:6]
                wsq = sc.tile([2, 3], fp32, tag="wsq")
                TT(wsq[:, :], w3, w3, op=AL.mult)
                th2 = sc.tile([2, 1], fp32, tag="th2")
                nc.vector.tensor_reduce(th2[:, :], wsq[:, :], axis=AX.X, op=AL.add)
                coA = sc.tile([2, 1], fp32, tag="coA")
                coB = sc.tile([2, 1], fp32, tag="coB")
                coC = sc.tile([2, 1], fp32, tag="coC")
                hh = sc.tile([2, 1], fp32, tag="hh")
                TS2(hh[:, :], th2[:, :], 1.0 / 120.0, -1.0 / 6.0, AL.mult, AL.add)
                nc.vector.tensor_scalar(coA[:, :], th2[:, :], hh[:, :], 1.0, AL.mult, AL.add)
                TS2(hh[:, :], th2[:, :], 1.0 / 720.0, -1.0 / 24.0, AL.mult, AL.add)
                nc.vector.tensor_scalar(coB[:, :], th2[:, :], hh[:, :], 0.5, AL.mult, AL.add)
                TS2(hh[:, :], th2[:, :], 1.0 / 5040.0, -1.0 / 120.0, AL.mult, AL.add)
                nc.vector.tensor_scalar(coC[:, :], th2[:, :], hh[:, :], 1.0 / 6.0, AL.mult, AL.add)
                # K, K2
                Kt = sc.tile([2, 9], fp32, tag="Kt")
                nc.vector.memset(Kt[:, :], 0.0)
                TS(Kt[:, 1:2], xi[:, 5:6], -1.0, AL.mult)   # -z
                nc.vector.tensor_copy(Kt[:, 2:3], xi[:, 4:5])  # y
                nc.vector.tensor_copy(Kt[:, 3:4], xi[:, 5:6])  # z
                TS(Kt[:, 5:6], xi[:, 3:4], -1.0, AL.mult)   # -x
                TS(Kt[:, 6:7], xi[:, 4:5], -1.0, AL.mult)   # -y
                nc.vector.tensor_copy(Kt[:, 7:8], xi[:, 3:4])  # x
                K2t = sc.tile([2, 9], fp32, tag="K2t")
                wiap = bass.AP(xi.tensor, xi.offset + 3, [list(xi.ap[0]), [1, 3], [0, 3]])
                wjap = bass.AP(xi.tensor, xi.offset + 3, [list(xi.ap[0]), [0, 3], [1, 3]])
                TT(K2t[:, :], wiap, wjap, op=AL.mult)
                k2diag = bass.AP(K2t.tensor, K2t.offset, [list(K2t.ap[0]), [4, 3]])
                nc.vector.tensor_scalar(k2diag, k2diag, th2[:, :], None, AL.subtract)
                Rt = sc.tile([2, 9], fp32, tag="Rt")
                Vt = sc.tile([2, 9], fp32, tag="Vt")
                t9 = sc.tile([2, 9], fp32, tag="t9")
                nc.vector.tensor_scalar(Rt[:, :], Kt[:, :], coA[:, :], None, AL.mult)
                nc.vector.tensor_scalar(t9[:, :], K2t[:, :], coB[:, :], None, AL.mult)
                TT(Rt[:, :], Rt[:, :], t9[:, :], op=AL.add)
                rdiag = bass.AP(Rt.tensor, Rt.offset, [list(Rt.ap[0]), [4, 3]])
                TS(rdiag, rdiag, 1.0, AL.add)
                nc.vector.tensor_scalar(Vt[:, :], Kt[:, :], coB[:, :], None, AL.mult)
                nc.vector.tensor_scalar(t9[:, :], K2t[:, :], coC[:, :], None, AL.mult)
                TT(Vt[:, :], Vt[:, :], t9[:, :], op=AL.add)
                vdiag = bass.AP(Vt.tensor, Vt.offset, [list(Vt.ap[0]), [4, 3]])
                TS(vdiag, vdiag, 1.0, AL.add)
                # t = V @ v
                vbc = bass.AP(xi.tensor, xi.offset, [list(xi.ap[0]), [0, 3], [1, 3]])
                TT(t9[:, :], Vt[:, :], vbc, op=AL.mult)
                tv = sc.tile([2, 3], fp32, tag="tv")
                t9v = bass.AP(t9.tensor, t9.offset, [list(t9.ap[0]), [3, 3], [1, 3]])
                nc.vector.tensor_reduce(tv[:, :], t9v, axis=AX.X, op=AL.add)
                # E = [[R, t],[0,0,0,1]] as [2,16]
                Et = sc.tile([2, 16], fp32, tag="Et")
                nc.vector.memset(Et[:, :], 0.0)
                edst = bass.AP(Et.tensor, Et.offset, [list(Et.ap[0]), [4, 3], [1, 3]])
                esrc = bass.AP(Rt.tensor, Rt.offset, [list(Rt.ap[0]), [3, 3], [1, 3]])
                nc.vector.tensor_copy(edst, esrc)
                edst2 = bass.AP(Et.tensor, Et.offset + 3, [list(Et.ap[0]), [4, 3]])
                nc.vector.tensor_copy(edst2, tv[:, :])
                TS(Et[:, 15:16], Et[:, 15:16], 1.0, AL.add)
                # newT = T @ E
                nT = sc.tile([2, 16], fp32, tag="nT")
                for k in range(4):
                    tcol = bass.AP(Tq.tensor, Tq.offset + k, [list(Tq.ap[0]), [4, 4], [0, 4]])
                    erow = bass.AP(Et.tensor, Et.offset + 4 * k, [list(Et.ap[0]), [0, 4], [1, 4]])
                    if k == 0:
                        TT(nT[:, :], tcol, erow, op=AL.mult)
                    else:
                        TT(tmpj[:, :16], tcol, erow, op=AL.mult)
                        TT(nT[:, :], nT[:, :], tmpj[:, :16], op=AL.add)
                nc.vector.tensor_copy(Tq[:, :], nT[:, :])
                # rebuild q = [R(9) | t(3) | intr(4)] and broadcast to rtm/rtw
                qt = sc.tile([2, 16], fp32, tag="qt")
                qr = bass.AP(Tq.tensor, Tq.offset, [list(Tq.ap[0]), [4, 3], [1, 3]])
                nc.vector.tensor_copy(qt[:, 0:9], qr)
                qtcol = bass.AP(Tq.tensor, Tq.offset + 3, [list(Tq.ap[0]), [4, 3]])
                nc.vector.tensor_copy(qt[:, 9:12], qtcol)
                nc.vector.tensor_copy(qt[:, 12:16], intr[:, :])
                nc.sync.dma_start(out=qscr.ap(), in_=qt[:, :])
                qsap = qscr.ap()
                for b in range(2):
                    qsrc = bass.AP(qsap.tensor, qsap.offset + b * 16, [[0, 64], [1, 16]])
                    nc.sync.dma_start(out=rtw[b * 64:(b + 1) * 64, :], in_=qsrc)
                for g in range(8):
                    b = g // 4
                    qsrc = bass.AP(qsap.tensor, qsap.offset + b * 16, [[0, 128], [1, 16]])
                    rdst = bass.AP(rtm.tensor, rtm.offset + g, [list(rtm.ap[0]), [8, 16]])
                    nc.sync.dma_start(out=rdst, in_=qsrc)

            nc.sync.dma_start(out=tout_ext.ap(), in_=Tq[:, :])

    nc.finalize()
    return nc


def _get_runner():
    """Build the sharded jitted executable ONCE (per-call jit(shard_map(...))
    in run_bass_via_pjrt retraces every call)."""
    if "runner" in _NC_CACHE:
        return _NC_CACHE["runner"]
    import jax
    import numpy as _np
    from concourse import mybir
    from concourse import bass2jax
    from jax.experimental.shard_map import shard_map
    from jax.sharding import Mesh, PartitionSpec

    nc = _NC_CACHE.get("nc")
    if nc is None:
        nc = build_nc()
        _NC_CACHE["nc"] = nc
    bass2jax.install_neuronx_cc_hook()

    partition_name = nc.partition_id_tensor.name if nc.partition_id_tensor else None
    in_names, out_names, out_avals, zero_outs = [], [], [], []
    for alloc in nc.m.functions[0].allocations:
        if not isinstance(alloc, mybir.MemoryLocationSet):
            continue
        name = alloc.memorylocations[0].name
        if alloc.kind == "ExternalInput":
            if name != partition_name:
                in_names.append(name)
        elif alloc.kind == "ExternalOutput":
            out_names.append(name)
            shape = tuple(alloc.tensor_shape)
            dtype = mybir.dt.np(alloc.dtype)
            out_avals.append(jax.core.ShapedArray(shape, dtype))
            zero_outs.append(_np.zeros(shape, dtype))
    n_params = len(in_names)
    n_outs = len(out_names)
    all_names = list(in_names) + out_names
    if partition_name is not None:
        all_names.append(partition_name)
    donate = tuple(range(n_params, n_params + n_outs))

    def _body(*args):
        operands = list(args)
        if partition_name is not None:
            operands.append(bass2jax.partition_id_tensor())
        outs = bass2jax._bass_exec_p.bind(
            *operands,
            out_avals=tuple(out_avals),
            in_names=tuple(all_names),
            out_names=tuple(out_names),
            lowering_input_output_aliases=(),
            sim_require_finite=True,
            sim_require_nnan=True,
            nc=nc,
        )
        return tuple(outs)

    devices = jax.devices()[:8]
    mesh = Mesh(_np.asarray(devices), ("core",))
    in_specs = (PartitionSpec("core"),) * (n_params + n_outs)
    out_specs = (PartitionSpec("core"),) * n_outs
    sharded = jax.jit(
        shard_map(_body, mesh=mesh, in_specs=in_specs, out_specs=out_specs,
                  check_rep=False),
        donate_argnums=donate, keep_unused=True)
    _NC_CACHE["runner"] = (sharded, in_names, out_names, out_avals, zero_outs)
    return _NC_CACHE["runner"]


def kernel(pose_twist, I0, I1, invD0, invD1, intrinsics):
    from concourse.bass_utils import run_bass_kernel_spmd

    nc = _NC_CACHE.get("nc")
    if nc is None:
        nc = build_nc()
        _NC_CACHE["nc"] = nc

    pose_twist = np.asarray(pose_twist, np.float32)
    I0 = np.asarray(I0, np.float32); I1 = np.asarray(I1, np.float32)
    invD0 = np.asarray(invD0, np.float32); invD1 = np.asarray(invD1, np.float32)
    intrinsics = np.asarray(intrinsics, np.float32)

    import time as _time
    LAST_WALL.clear(); LAST_EXEC_NS.clear(); LAST_TRACES.clear()
    t0 = _time.time()
    in_maps, _ = host_precompute_all(pose_twist, I0, I1, invD0, invD1, intrinsics)
    t1 = _time.time()
    if PROFILE:
        res = run_bass_kernel_spmd(nc, in_maps, list(range(8)), trace=True)
        if res.exec_time_ns is not None:
            LAST_EXEC_NS.append(res.exec_time_ns)
        if res.instructions_and_trace is not None:
            LAST_TRACES.append(res.instructions_and_trace[1])
        touts = [res.results[core]["tout"] for core in range(8)]
    else:
        sharded, in_names, out_names, out_avals, zero_outs = _get_runner()
        concat_in = [np.concatenate([in_maps[c][nm] for c in range(8)], axis=0)
                     for nm in in_names]
        concat_zeros = [np.zeros((8 * z.shape[0], *z.shape[1:]), z.dtype)
                        for z in zero_outs]
        out_arrs = sharded(*concat_in, *concat_zeros)
        oi = out_names.index("tout")
        tall = np.asarray(out_arrs[oi]).reshape(8, *out_avals[oi].shape)
        touts = [tall[c] for c in range(8)]
    t2 = _time.time()
    LAST_WALL.extend([round(t1 - t0, 3), round(t2 - t1, 3)])

    outs = [t.reshape(2, 4, 4) for t in touts]
    return np.concatenate(outs, axis=0).astype(np.float32)


# revision 49
# speedup vs baseline: 2.5241x; 1.0224x over previous
"""Trainium2 Bass kernel for nn_InvDirectImageAlign (inverse-compositional image alignment).

v3: ONE compiled NEFF runs all 5 Gauss-Newton iterations on device
(hardware For_i loop). Per core: 2 batch elements. Device does warp,
bilinear grid_sample (GPSIMD ap_gather from fp16 pair-dup band tables),
the JtWJ/Rhs normal equations via TensorEngine matmuls of a per-pixel
fp16 factor matrix G (JtWJ = sum_c G_c^T G_c), the 6x6 Cholesky solve,
se3_exp (Taylor series - angles are <<1 here) and the pose composition.
Inputs upload once; output is just the final 4x4 poses.

Chunking: (batch, 16-row y-band, 224-col x-half) = 80 chunks/core; the 8
GPSIMD partition-groups each own one chunk per superstep; 10 supersteps.
Two pixel layouts, bridged only by PE transposes of gathered data:
  mod-128:    pixel j of chunk(g,s) at partition j%128, free col (g, j//128)
  wrapped-16: pixel j at partition 16g + j%16, free col j//16   (ap_gather's
              index layout)
"""
import numpy as np

B, C, H, W = 16, 3, 320, 448
HW = H * W
N_ITERS = 5
LAMBDA = 0.01
HUBER_DELTA = 0.1
EPS = 1e-6

BH = 16            # band rows per chunk
CW = 224           # band cols per chunk
N = BH * CW        # 3584 px per chunk
A = N // 128       # 28
M = N // 16        # 224
NS = 10            # supersteps
TR = 67            # table rows (16 + 25 + 26)
TC = 266           # table cols (224 + 20 + 21 + 1)
NELEM = TR * TC    # 17822 pairs
YPAD = 25
XPAD = 20


def skew3(w):
    x, y, z = w[..., 0], w[..., 1], w[..., 2]
    O = np.zeros_like(x)
    return np.stack([np.stack([O, -z, y], -1),
                     np.stack([z, O, -x], -1),
                     np.stack([-y, x, O], -1)], -2)


def se3_exp(xi):
    xi = np.asarray(xi, np.float64)
    v, w = xi[:, :3], xi[:, 3:]
    th2 = np.sum(w * w, -1)[:, None, None]
    th2c = np.maximum(th2, 1e-16)
    th = np.sqrt(th2c)
    small = th2 < 1e-10
    Aa = np.where(small, 1.0 - th2 / 6.0, np.sin(th) / th)
    Bc = np.where(small, 0.5 - th2 / 24.0, (1.0 - np.cos(th)) / th2c)
    Cc = np.where(small, 1.0 / 6.0 - th2 / 120.0, (1.0 - Aa) / th2c)
    K = skew3(w)
    K2 = K @ K
    I = np.eye(3)
    R = I + Aa * K + Bc * K2
    V = I + Bc * K + Cc * K2
    t = np.einsum('bij,bj->bi', V, v)
    T = np.zeros((xi.shape[0], 4, 4))
    T[:, :3, :3] = R
    T[:, :3, 3] = t
    T[:, 3, 3] = 1.0
    return T.astype(np.float32)


def feature_gradient(img):
    p = np.pad(img, ((0, 0), (0, 0), (0, 0), (1, 1)), mode='edge')
    dx = 0.5 * (p[..., 2:] - p[..., :-2])
    p = np.pad(img, ((0, 0), (0, 0), (1, 1), (0, 0)), mode='edge')
    dy = 0.5 * (p[..., 2:, :] - p[..., :-2, :])
    return dx.astype(np.float32), dy.astype(np.float32)


def chunk_of(g, s):
    b = g // 4
    local = (g % 4) * 10 + s
    return b, local // 2, local % 2


def bases_of(yb, xh):
    r0, c0 = yb * BH, xh * CW
    rbase = int(np.clip(r0 - YPAD, 0, H - TR))
    cbase = int(np.clip(c0 - XPAD, 0, W - (TC - 1)))
    return rbase, cbase


def mod128_cols_batch(x):
    """[2,K,H,W] -> [128, NS*8*A*K] vectorized (one core's 2 batches)."""
    K = x.shape[1]
    # chunk (b, yb, xh): local = yb*2+xh; g = b*4 + local//10; s = local%10
    a = x.reshape(2, K, 20, BH, 2, CW)          # b K yb row xh col
    a = a.transpose(0, 2, 4, 1, 3, 5)           # b yb xh K row col
    a = a.reshape(2, 40, K, N)                  # local = yb*2+xh
    a = a.reshape(2, 4, 10, K, A, 128)          # b g4 s K a p
    a = a.transpose(5, 2, 0, 1, 4, 3)           # p s b g4 a K
    return np.ascontiguousarray(a.reshape(128, NS, 8, A, K).reshape(128, -1))


def wrap16_cols_batch(x, K):
    """[2,K,H,W] -> [128, NS*M*K] (partition 16g + j%16, col (j//16)*K + k)."""
    a = x.reshape(2, K, 20, BH, 2, CW)
    a = a.transpose(0, 2, 4, 1, 3, 5).reshape(2, 40, K, N)
    a = a.reshape(2, 4, 10, K, M, 16)           # b g4 s K m plo
    a = a.transpose(0, 1, 5, 2, 4, 3)           # b g4 plo s m K
    # partition = 16*(b*4+g4) + plo
    return np.ascontiguousarray(a.reshape(128, NS, M, K).reshape(128, -1))


def host_precompute_all(pose_twist, I0, I1, invD0, invD1, intr):
    """Vectorized over all B=16; returns per-core input dicts + T0 per core."""
    T0 = se3_exp(pose_twist)
    fx = intr[:, 0][:, None, None]; fy = intr[:, 1][:, None, None]
    cx = intr[:, 2][:, None, None]; cy = intr[:, 3][:, None, None]
    uu = np.arange(W, dtype=np.float32)[None, None, :]
    vv = np.arange(H, dtype=np.float32)[None, :, None]
    iD = np.maximum(invD1[:, 0], EPS).astype(np.float32)
    z1 = (1.0 / iD).astype(np.float32)
    xn = ((uu - cx) / fx).astype(np.float32)     # [B,1,W]
    yn = ((vv - cy) / fy).astype(np.float32)     # [B,H,1]
    x1 = xn * z1
    y1 = yn * z1
    # edge-replicated padded raw planes; device computes the 12 table planes
    rawp = np.empty((B, 4, H + 2, W + 2), np.float16)
    rawp[:, :3, 1:H + 1, 1:W + 1] = I0
    rawp[:, 3:, 1:H + 1, 1:W + 1] = invD0
    rawp[:, :, 0] = rawp[:, :, 1]
    rawp[:, :, H + 1] = rawp[:, :, H]
    rawp[:, :, :, 0] = rawp[:, :, :, 1]
    rawp[:, :, :, W + 1] = rawp[:, :, :, W]

    X1 = np.stack([x1, y1, z1], 1).astype(np.float16)       # [B, 3, H, W]
    I1f = np.asarray(I1, np.float32)

    bw = np.zeros((128, NS, 4), np.float32)
    for g in range(8):
        for s in range(NS):
            _, yb, xh2 = chunk_of(g, s)
            rbase, cbase = bases_of(yb, xh2)
            bw[16 * g:16 * g + 16, s, 0] = rbase
            bw[16 * g:16 * g + 16, s, 1] = cbase - 1          # xf min
            bw[16 * g:16 * g + 16, s, 2] = cbase + (TC - 2)   # xf max
            bw[16 * g:16 * g + 16, s, 3] = 1 - cbase          # kx offset
    bw = np.ascontiguousarray(bw.reshape(128, NS * 4))
    idn = np.eye(128, dtype=np.float16)

    I1h = I1f.astype(np.float16)
    core_inputs, T0s = [], []
    for core in range(8):
        sl = slice(2 * core, 2 * core + 2)
        inp = {}
        inp["rawp"] = np.ascontiguousarray(rawp[sl].reshape(2, 4, (H + 2) * (W + 2)))
        inp["x1m"] = mod128_cols_batch(X1[sl])
        inp["x1w"] = wrap16_cols_batch(X1[sl], 3)
        inp["i1m"] = mod128_cols_batch(I1h[sl])
        inp["bw"] = bw
        inp["idn"] = idn
        q = np.zeros((2, 16), np.float32)
        q[:, :9] = T0[sl, :3, :3].reshape(2, 9)
        q[:, 9:12] = T0[sl, :3, 3]
        q[:, 12:16] = intr[sl]
        rtm = np.zeros((128, 16, 8), np.float32)
        rtw = np.zeros((128, 16), np.float32)
        for g in range(8):
            bb = g // 4
            rtm[:, :, g] = q[bb][None, :]
            rtw[16 * g:16 * g + 16, :] = q[bb][None, :]
        inp["rtm"] = np.ascontiguousarray(rtm.reshape(128, 16 * 8))
        inp["rtw"] = rtw
        inp["t0q"] = np.ascontiguousarray(T0[sl].reshape(2, 16).astype(np.float32))
        inp["intr2"] = np.ascontiguousarray(intr[sl].astype(np.float32))
        core_inputs.append(inp)
        T0s.append(T0[sl])
    return core_inputs, T0s


_NC_CACHE = {}
PROFILE = False
LAST_EXEC_NS = []
LAST_TRACES = []
LAST_WALL = []


def build_nc():
    import concourse.bacc as bacc
    import concourse.bass as bass
    import concourse.tile as tile
    from concourse import mybir

    fp32 = mybir.dt.float32
    fp16 = mybir.dt.float16
    i16 = mybir.dt.int16
    i32 = mybir.dt.int32
    AL = mybir.AluOpType
    ACT = mybir.ActivationFunctionType
    AX = mybir.AxisListType

    nc = bacc.Bacc("TRN2", target_bir_lowering=False, debug=False, num_devices=8)

    rawp_in = nc.dram_tensor("rawp", [2, 4, (H + 2) * (W + 2)], fp16, kind="ExternalInput")
    pd_in = nc.dram_tensor("pds12", [2, 12, HW + 2], fp16, kind="Internal")
    x1m_in = nc.dram_tensor("x1m", [128, NS * 8 * A * 3], fp16, kind="ExternalInput")
    x1w_in = nc.dram_tensor("x1w", [128, NS * M * 3], fp16, kind="ExternalInput")
    i1m_in = nc.dram_tensor("i1m", [128, NS * 8 * A * 3], fp16, kind="ExternalInput")
    bw_in = nc.dram_tensor("bw", [128, NS * 4], fp32, kind="ExternalInput")
    idn_in = nc.dram_tensor("idn", [128, 128], fp16, kind="ExternalInput")
    rtm_in = nc.dram_tensor("rtm", [128, 16 * 8], fp32, kind="ExternalInput")
    rtw_in = nc.dram_tensor("rtw", [128, 16], fp32, kind="ExternalInput")
    t0_in = nc.dram_tensor("t0q", [2, 16], fp32, kind="ExternalInput")
    intr_in = nc.dram_tensor("intr2", [2, 4], fp32, kind="ExternalInput")
    tout_ext = nc.dram_tensor("tout", [2, 16], fp32, kind="ExternalOutput")
    qscr = nc.dram_tensor("qscr", [2, 16], fp32, kind="Internal")

    with tile.TileContext(nc) as tc:
        with tc.tile_pool(name="cst", bufs=1) as cpool, \
             tc.tile_pool(name="tblp", bufs=1) as tpool, \
             tc.tile_pool(name="strm", bufs=2) as sp, \
             tc.tile_pool(name="scr", bufs=1) as sc, \
             tc.tile_pool(name="gath", bufs=1) as gp, \
             tc.tile_pool(name="ps", bufs=2, space="PSUM") as pp, \
             tc.tile_pool(name="jp", bufs=1, space="PSUM") as jp:

            rtm = cpool.tile([128, 16 * 8], fp32, tag="rtm")
            rtm0 = cpool.tile([128, 16 * 8], fp32, tag="rtm0")
            rtw = cpool.tile([128, 16], fp32, tag="rtw")
            bwc = cpool.tile([128, NS * 4], fp32, tag="bw")
            idn = cpool.tile([128, 128], fp16, tag="idn")
            Tq = cpool.tile([2, 16], fp32, tag="Tq")
            intr = cpool.tile([2, 4], fp32, tag="intr")
            nc.sync.dma_start(out=rtm[:, :], in_=rtm_in.ap())
            nc.sync.dma_start(out=rtm0[:, :], in_=rtm_in.ap())
            nc.sync.dma_start(out=rtw[:, :], in_=rtw_in.ap())
            nc.sync.dma_start(out=bwc[:, :], in_=bw_in.ap())
            nc.sync.dma_start(out=idn[:, :], in_=idn_in.ap())
            nc.sync.dma_start(out=Tq[:, :], in_=t0_in.ap())
            nc.sync.dma_start(out=intr[:, :], in_=intr_in.ap())

            psJ = [jp.tile([28, 28], fp32, name=f"psJ{b}", tag=f"psJ{b}") for b in range(2)]

            # one-time: 12 table planes (grads + raw) from padded raw planes
            WP = W + 2
            zt1 = cpool.tile([1, 1], fp16, tag="zt1")
            nc.vector.memset(zt1[:, :], 0.0)
            pda = pd_in.ap()
            for b2 in range(2):
                for pl in range(12):
                    for zo in (0, HW + 1):
                        zdst = bass.AP(pda.tensor,
                                       pda.offset + (b2 * 12 + pl) * (HW + 2) + zo,
                                       [[1, 1], [1, 1]])
                        nc.sync.dma_start(out=zdst, in_=zt1[:, :])
            tA0 = cpool.tile([128, WP], fp16, tag="tA0")
            tU0 = cpool.tile([128, WP], fp16, tag="tU0")
            tD0 = cpool.tile([128, WP], fp16, tag="tD0")
            go0 = cpool.tile([128, W], fp16, tag="go0")
            rpa = rawp_in.ap()
            for b2 in range(2):
                for c4 in range(4):
                    pbase = rpa.offset + (b2 * 4 + c4) * (H + 2) * WP
                    for r0, nr in ((0, 128), (128, 128), (256, 64)):
                        for t_, roff in ((tA0, r0 + 1), (tU0, r0), (tD0, r0 + 2)):
                            srcap = bass.AP(rpa.tensor, pbase + roff * WP,
                                            [[WP, nr], [1, WP]])
                            nc.sync.dma_start(out=t_[:nr, :], in_=srcap)
                        pdx = c4 if c4 < 3 else 6
                        pdy = 3 + c4 if c4 < 3 else 7
                        pcp = 8 + c4 if c4 < 3 else 11
                        dbase = pda.offset + 1 + r0 * W
                        # dx = 0.5*(A[:,2:] - A[:,:2-])
                        nc.vector.tensor_tensor(go0[:nr, :], tA0[:nr, 2:WP], tA0[:nr, 0:W], op=AL.subtract)
                        nc.vector.tensor_scalar(go0[:nr, :], go0[:nr, :], 0.5, None, AL.mult)
                        ddst = bass.AP(pda.tensor, dbase + (b2 * 12 + pdx) * (HW + 2),
                                       [[W, nr], [1, W]])
                        nc.sync.dma_start(out=ddst, in_=go0[:nr, :])
                        # dy = 0.5*(D[:,1:W+1] - U[:,1:W+1])
                        nc.vector.tensor_tensor(go0[:nr, :], tD0[:nr, 1:W + 1], tU0[:nr, 1:W + 1], op=AL.subtract)
                        nc.vector.tensor_scalar(go0[:nr, :], go0[:nr, :], 0.5, None, AL.mult)
                        ddst = bass.AP(pda.tensor, dbase + (b2 * 12 + pdy) * (HW + 2),
                                       [[W, nr], [1, W]])
                        nc.sync.dma_start(out=ddst, in_=go0[:nr, :])
                        # raw copy plane
                        ddst = bass.AP(pda.tensor, dbase + (b2 * 12 + pcp) * (HW + 2),
                                       [[W, nr], [1, W]])
                        nc.sync.dma_start(out=ddst, in_=tA0[:nr, 1:W + 1])

            tbl0 = tpool.tile([128, NELEM * 2], fp16, tag="tbl")
            nc.vector.memset(tbl0[:, :], 0.0)
            stbl0 = tpool.tile([128, 34 * (TC + 1)], fp16, tag="stbl")
            nc.vector.memset(stbl0[:, :], 0.0)

            def rq(qi):   # mod-128 per-group broadcast: dims (g x8, a x A step0)
                sl = rtm[:, qi * 8:(qi + 1) * 8]
                return bass.AP(sl.tensor, sl.offset, [list(sl.ap[0]), [1, 8], [0, A]])

            def rqw(qi):  # wrapped per-partition scalar bcast over M
                sl = rtw[:, qi:qi + 1]
                return bass.AP(sl.tensor, sl.offset, [list(sl.ap[0]), [0, M]])

            def rtwS(qi):  # wrapped per-partition scalar [128,1]
                return rtw[:, qi:qi + 1]

            def bwq(s, j):
                sl = bwc[:, s * 4 + j:s * 4 + j + 1]
                return bass.AP(sl.tensor, sl.offset, [list(sl.ap[0]), [0, M]])

            def bwS(s, j):
                return bwc[:, s * 4 + j:s * 4 + j + 1]

            TT = nc.vector.tensor_tensor
            TS = lambda out, in0, s1, op: nc.vector.tensor_scalar(out, in0, s1, None, op)
            TS2 = lambda out, in0, s1, s2, op0, op1: nc.vector.tensor_scalar(out, in0, s1, s2, op0, op1)
            STT = nc.vector.scalar_tensor_tensor

            with tc.For_i(0, N_ITERS) as _it:
                for s in range(NS):
                    tbl = tbl0
                    for r0, nr in ((0, 34), (34, 33)):
                        for g in range(8):
                            b, yb, xh = chunk_of(g, s)
                            rbase, cbase = bases_of(yb, xh)
                            start = (rbase + r0) * W + cbase
                            src0 = pd_in.ap()
                            src = bass.AP(src0.tensor,
                                          src0.offset + b * 12 * (HW + 2) + start,
                                          [[HW + 2, 12], [W, nr], [1, TC + 1]])
                            dsl = stbl0[16 * g:16 * g + 12, :]
                            dst = bass.AP(dsl.tensor, dsl.offset,
                                          [[dsl.ap[0][0], 12], [TC + 1, nr], [1, TC + 1]])
                            nc.sync.dma_start(out=dst, in_=src)
                        for e in range(2):
                            pout = bass.AP(tbl.tensor, tbl.offset + e + r0 * 2 * TC,
                                           [list(tbl.ap[0]), [2 * TC, nr], [2, TC]])
                            pin = bass.AP(stbl0.tensor, stbl0.offset + e,
                                          [list(stbl0.ap[0]), [TC + 1, nr], [1, TC]])
                            nc.scalar.activation(pout, pin, ACT.Copy)

                    x1w = sp.tile([128, M * 3], fp16, tag="x1w")
                    nc.sync.dma_start(out=x1w[:, :], in_=x1w_in.ap()[:, s * M * 3:(s + 1) * M * 3])
                    x1m = sp.tile([128, 8 * A * 3], fp16, tag="x1m")
                    nc.sync.dma_start(out=x1m[:, :], in_=x1m_in.ap()[:, s * 8 * A * 3:(s + 1) * 8 * A * 3])
                    i1 = sp.tile([128, 8 * A * 3], fp16, tag="i1")
                    nc.sync.dma_start(out=i1[:, :], in_=i1m_in.ap()[:, s * 8 * A * 3:(s + 1) * 8 * A * 3])

                    # ---------- wrapped-16 idx pipeline ----------
                    def xw(k):
                        sl = x1w[:, :]
                        return bass.AP(sl.tensor, sl.offset + k, [list(sl.ap[0]), [3, M]])

                    def tw(name):
                        return sc.tile([128, M], fp32, name="w_" + name + f"_{s}", tag="w_" + name)

                    t1w = tw("t1")
                    X0zw = tw("X0z")
                    STT(X0zw[:, :], xw(0), rtwS(6), rqw(11), AL.mult, AL.add)
                    STT(X0zw[:, :], xw(1), rtwS(7), X0zw[:, :], AL.mult, AL.add)
                    STT(X0zw[:, :], xw(2), rtwS(8), X0zw[:, :], AL.mult, AL.add)
                    X0xw = tw("X0x")
                    STT(X0xw[:, :], xw(0), rtwS(0), rqw(9), AL.mult, AL.add)
                    STT(X0xw[:, :], xw(1), rtwS(1), X0xw[:, :], AL.mult, AL.add)
                    STT(X0xw[:, :], xw(2), rtwS(2), X0xw[:, :], AL.mult, AL.add)
                    X0yw = tw("X0y")
                    STT(X0yw[:, :], xw(0), rtwS(3), rqw(10), AL.mult, AL.add)
                    STT(X0yw[:, :], xw(1), rtwS(4), X0yw[:, :], AL.mult, AL.add)
                    STT(X0yw[:, :], xw(2), rtwS(5), X0yw[:, :], AL.mult, AL.add)

                    izw = tw("iz")
                    TS(t1w[:, :], X0zw[:, :], EPS, AL.max)
                    nc.vector.reciprocal_approx_fast(izw[:, :], t1w[:, :])
                    u0w = tw("u0"); v0w = tw("v0")
                    TT(u0w[:, :], X0xw[:, :], izw[:, :], op=AL.mult)
                    STT(u0w[:, :], u0w[:, :], rtwS(12), rqw(14), AL.mult, AL.add)
                    TT(v0w[:, :], X0yw[:, :], izw[:, :], op=AL.mult)
                    STT(v0w[:, :], v0w[:, :], rtwS(13), rqw(15), AL.mult, AL.add)
                    TS2(u0w[:, :], u0w[:, :], -0.5 * (W - 1), 1.5 * (W - 1), AL.max, AL.min)
                    TS2(v0w[:, :], v0w[:, :], -0.5 * (H - 1), 1.5 * (H - 1), AL.max, AL.min)
                    x0fw = tw("x0f"); y0fw = tw("y0f")
                    fi32w = sc.tile([128, M], i32, name=f"fi32w_{s}", tag="fi32w")
                    TS(t1w[:, :], u0w[:, :], 0.5, AL.subtract)
                    nc.vector.tensor_copy(fi32w[:, :], t1w[:, :])
                    nc.vector.tensor_copy(x0fw[:, :], fi32w[:, :])
                    TS(t1w[:, :], v0w[:, :], 0.5, AL.subtract)
                    nc.vector.tensor_copy(fi32w[:, :], t1w[:, :])
                    nc.vector.tensor_copy(y0fw[:, :], fi32w[:, :])
                    xfw = t1w; kxw = izw; yrw = X0zw
                    ktw = X0xw; kbw = X0yw
                    STT(xfw[:, :], x0fw[:, :], bwS(s, 1), bwq(s, 2), AL.max, AL.min)
                    nc.vector.tensor_scalar(kxw[:, :], xfw[:, :], bwS(s, 3), None, AL.add)
                    nc.vector.tensor_scalar(yrw[:, :], y0fw[:, :], bwS(s, 0), 0.0, AL.subtract, AL.max)
                    TS2(ktw[:, :], yrw[:, :], float(TR - 1), float(TC), AL.min, AL.mult)
                    TT(ktw[:, :], ktw[:, :], kxw[:, :], op=AL.add)
                    TS2(kbw[:, :], yrw[:, :], 1.0, float(TR - 1), AL.add, AL.min)
                    TS(kbw[:, :], kbw[:, :], float(TC), AL.mult)
                    TT(kbw[:, :], kbw[:, :], kxw[:, :], op=AL.add)
                    kidx = sc.tile([128, 2 * M], i16, name=f"kidx_{s}", tag="kidx")
                    nc.vector.tensor_copy(kidx[:, :M], ktw[:, :])
                    nc.vector.tensor_copy(kidx[:, M:], kbw[:, :])

                    gt2 = gp.tile([128, 2 * N * 2], fp16, tag="gt2")
                    nc.gpsimd.ap_gather(gt2[:, :], tbl[:, :], kidx[:, :],
                                        channels=128, num_elems=NELEM, d=2, num_idxs=2 * N)

                    # ---------- mod-128 warp pipeline ----------
                    def xm(k):
                        sl = x1m[:, :]
                        return bass.AP(sl.tensor, sl.offset + k, [list(sl.ap[0]), [3, 8 * A]])

                    def tm(name):
                        return sc.tile([128, 8 * A], fp32, name="m_" + name + f"_{s}", tag="m_" + name)

                    def matvec(dst, aps, qis, t1):
                        TT(dst[:, :], aps[0], qis[0], op=AL.mult)
                        TT(t1[:, :], aps[1], qis[1], op=AL.mult)
                        TT(dst[:, :], dst[:, :], t1[:, :], op=AL.add)
                        TT(t1[:, :], aps[2], qis[2], op=AL.mult)
                        TT(dst[:, :], dst[:, :], t1[:, :], op=AL.add)
                        TT(dst[:, :], dst[:, :], qis[3], op=AL.add)

                    # ---- on-device A6/B6/T6 at the initial pose (rtm0) ----
                    abt = sc.tile([128, 8 * A * 18], fp16, name=f"abt_{s}", tag="abt")

                    def acol(k):
                        sl = abt[:, :]
                        return bass.AP(sl.tensor, sl.offset + k, [list(sl.ap[0]), [18, 224]])

                    def rq0(qi):
                        sl = rtm0[:, qi * 8:(qi + 1) * 8]
                        return bass.AP(sl.tensor, sl.offset, [list(sl.ap[0]), [1, 8], [0, A]])

                    j1 = tm("j1"); j2 = tm("j2")
                    jx = tm("jx"); jy = tm("jy"); jz = tm("jz"); jiz = tm("jiz")
                    matvec(jz, [xm(0), xm(1), xm(2)], [rq0(6), rq0(7), rq0(8), rq0(11)], j1)
                    matvec(jx, [xm(0), xm(1), xm(2)], [rq0(0), rq0(1), rq0(2), rq0(9)], j1)
                    matvec(jy, [xm(0), xm(1), xm(2)], [rq0(3), rq0(4), rq0(5), rq0(10)], j1)
                    TS(j1[:, :], jz[:, :], EPS, AL.max)
                    nc.vector.reciprocal_approx_fast(jiz[:, :], j1[:, :])
                    fxiz = tm("fxiz"); fyiz = tm("fyiz"); zizt = tm("zizt")
                    A2t = tm("A2t"); B2t = tm("B2t")
                    TT(fxiz[:, :], jiz[:, :], rq0(12), op=AL.mult)
                    TT(fyiz[:, :], jiz[:, :], rq0(13), op=AL.mult)
                    TT(zizt[:, :], jz[:, :], jiz[:, :], op=AL.mult)
                    TT(j1[:, :], jx[:, :], jiz[:, :], op=AL.mult)
                    TT(A2t[:, :], fxiz[:, :], j1[:, :], op=AL.mult)
                    TT(j1[:, :], jy[:, :], jiz[:, :], op=AL.mult)
                    TT(B2t[:, :], fyiz[:, :], j1[:, :], op=AL.mult)
                    TS(acol(0), fxiz[:, :], -1.0, AL.mult)
                    TS(acol(1), fxiz[:, :], 0.0, AL.mult)
                    nc.vector.tensor_copy(acol(2), A2t[:, :])
                    TT(acol(3), A2t[:, :], jy[:, :], op=AL.mult)
                    TT(j1[:, :], zizt[:, :], rq0(12), op=AL.mult)
                    TT(j2[:, :], A2t[:, :], jx[:, :], op=AL.mult)
                    TT(j1[:, :], j1[:, :], j2[:, :], op=AL.add)
                    TS(acol(4), j1[:, :], -1.0, AL.mult)
                    TT(acol(5), fxiz[:, :], jy[:, :], op=AL.mult)
                    TS(acol(6), fxiz[:, :], 0.0, AL.mult)
                    TS(acol(7), fyiz[:, :], -1.0, AL.mult)
                    nc.vector.tensor_copy(acol(8), B2t[:, :])
                    TT(j1[:, :], zizt[:, :], rq0(13), op=AL.mult)
                    TT(j2[:, :], B2t[:, :], jy[:, :], op=AL.mult)
                    TT(acol(9), j1[:, :], j2[:, :], op=AL.add)
                    TT(j1[:, :], B2t[:, :], jx[:, :], op=AL.mult)
                    TS(acol(10), j1[:, :], -1.0, AL.mult)
                    TT(j1[:, :], fyiz[:, :], jx[:, :], op=AL.mult)
                    TS(acol(11), j1[:, :], -1.0, AL.mult)
                    TS(acol(12), fxiz[:, :], 0.0, AL.mult)
                    TS(acol(13), fxiz[:, :], 0.0, AL.mult)
                    TS2(acol(14), fxiz[:, :], 0.0, 1.0, AL.mult, AL.add)
                    nc.vector.tensor_copy(acol(15), jy[:, :])
                    TS(acol(16), jx[:, :], -1.0, AL.mult)
                    TS(acol(17), fxiz[:, :], 0.0, AL.mult)

                    m1 = j1; m2 = j2
                    X0z = jz
                    matvec(X0z, [xm(0), xm(1), xm(2)], [rq(6), rq(7), rq(8), rq(11)], m1)
                    X0x = jx
                    matvec(X0x, [xm(0), xm(1), xm(2)], [rq(0), rq(1), rq(2), rq(9)], m1)
                    X0y = jy
                    matvec(X0y, [xm(0), xm(1), xm(2)], [rq(3), rq(4), rq(5), rq(10)], m1)
                    iz = jiz
                    TS(m1[:, :], X0z[:, :], EPS, AL.max)
                    nc.vector.reciprocal_approx_fast(iz[:, :], m1[:, :])
                    u0 = fxiz; v0 = fyiz
                    TT(u0[:, :], X0x[:, :], iz[:, :], op=AL.mult)
                    TT(u0[:, :], u0[:, :], rq(12), op=AL.mult)
                    TT(u0[:, :], u0[:, :], rq(14), op=AL.add)
                    TT(v0[:, :], X0y[:, :], iz[:, :], op=AL.mult)
                    TT(v0[:, :], v0[:, :], rq(13), op=AL.mult)
                    TT(v0[:, :], v0[:, :], rq(15), op=AL.add)
                    vmask = zizt
                    TS(vmask[:, :], X0z[:, :], EPS, AL.is_gt)
                    STT(vmask[:, :], u0[:, :], 0.0, vmask[:, :], AL.is_gt, AL.mult)
                    STT(vmask[:, :], u0[:, :], float(W - 1), vmask[:, :], AL.is_lt, AL.mult)
                    STT(vmask[:, :], v0[:, :], 0.0, vmask[:, :], AL.is_gt, AL.mult)
                    STT(vmask[:, :], v0[:, :], float(H - 1), vmask[:, :], AL.is_lt, AL.mult)
                    TS2(u0[:, :], u0[:, :], -0.5 * (W - 1), 1.5 * (W - 1), AL.max, AL.min)
                    TS2(v0[:, :], v0[:, :], -0.5 * (H - 1), 1.5 * (H - 1), AL.max, AL.min)
                    wx = A2t; wy = B2t; x0f = tm("x0f"); y0f = tm("y0f")
                    fi32m = sc.tile([128, 8 * A], i32, name=f"fi32m_{s}", tag="fi32m")
                    TS(m1[:, :], u0[:, :], 0.5, AL.subtract)
                    nc.vector.tensor_copy(fi32m[:, :], m1[:, :])
                    nc.vector.tensor_copy(x0f[:, :], fi32m[:, :])
                    TT(wx[:, :], u0[:, :], x0f[:, :], op=AL.subtract)
                    TS(m1[:, :], v0[:, :], 0.5, AL.subtract)
                    nc.vector.tensor_copy(fi32m[:, :], m1[:, :])
                    nc.vector.tensor_copy(y0f[:, :], fi32m[:, :])
                    TT(wy[:, :], v0[:, :], y0f[:, :], op=AL.subtract)
                    mx0 = tm("mx0"); mx1 = tm("mx1"); my0 = tm("my0"); my1 = tm("my1")
                    TS(mx0[:, :], x0f[:, :], -0.5, AL.is_gt)
                    STT(mx0[:, :], x0f[:, :], float(W - 1) + 0.5, mx0[:, :], AL.is_lt, AL.mult)
                    TS(mx1[:, :], x0f[:, :], -1.5, AL.is_gt)
                    STT(mx1[:, :], x0f[:, :], float(W - 2) + 0.5, mx1[:, :], AL.is_lt, AL.mult)
                    TS(my0[:, :], y0f[:, :], -0.5, AL.is_gt)
                    STT(my0[:, :], y0f[:, :], float(H - 1) + 0.5, my0[:, :], AL.is_lt, AL.mult)
                    TS(my1[:, :], y0f[:, :], -1.5, AL.is_gt)
                    STT(my1[:, :], y0f[:, :], float(H - 2) + 0.5, my1[:, :], AL.is_lt, AL.mult)
                    W00 = tm("W00"); W01 = tm("W01"); W10 = tm("W10"); W11 = tm("W11")
                    TS2(m1[:, :], wx[:, :], 1.0, -1.0, AL.subtract, AL.mult)  # 1-wx
                    TS2(m2[:, :], wy[:, :], 1.0, -1.0, AL.subtract, AL.mult)  # 1-wy
                    TT(W00[:, :], m1[:, :], m2[:, :], op=AL.mult)
                    TT(W00[:, :], W00[:, :], mx0[:, :], op=AL.mult)
                    TT(W00[:, :], W00[:, :], my0[:, :], op=AL.mult)
                    TT(W01[:, :], wx[:, :], m2[:, :], op=AL.mult)
                    TT(W01[:, :], W01[:, :], mx1[:, :], op=AL.mult)
                    TT(W01[:, :], W01[:, :], my0[:, :], op=AL.mult)
                    TT(W10[:, :], m1[:, :], wy[:, :], op=AL.mult)
                    TT(W10[:, :], W10[:, :], mx0[:, :], op=AL.mult)
                    TT(W10[:, :], W10[:, :], my1[:, :], op=AL.mult)
                    TT(W11[:, :], wx[:, :], wy[:, :], op=AL.mult)
                    TT(W11[:, :], W11[:, :], mx1[:, :], op=AL.mult)
                    TT(W11[:, :], W11[:, :], my1[:, :], op=AL.mult)

                    # ---------- PE transpose + combine ----------
                    samp = sc.tile([128, A * 128], fp16, tag="samp")
                    ctmp = sc.tile([128, 512], fp16, tag="ctmp")
                    for a4 in range(A // 4):
                        ptall = pp.tile([128, 2048], fp16, tag="ptall")
                        for ci, base in enumerate((0, 1, 2 * N, 2 * N + 1)):
                            for aa in range(4):
                                a = a4 * 4 + aa
                                src = bass.AP(gt2.tensor, gt2.offset + base + a * 256,
                                              [list(gt2.ap[0]), [2, 128]])
                                nc.tensor.transpose(
                                    ptall[:, ci * 512 + aa * 128:ci * 512 + (aa + 1) * 128],
                                    src, idn[:, :])
                        for ci, wt_ in ((0, W00), (1, W01), (2, W10), (3, W11)):
                            pap = bass.AP(ptall.tensor, ptall.offset + ci * 512,
                                          [list(ptall.ap[0]), [128, 4], [16, 8], [1, 16]])
                            woff = wt_.offset + a4 * 4
                            wap = bass.AP(wt_.tensor, woff, [list(wt_.ap[0]), [1, 4], [A, 8], [0, 16]])
                            dst_off = samp.offset + a4 * 4 * 128
                            dap = bass.AP(samp.tensor, dst_off, [list(samp.ap[0]), [128, 4], [16, 8], [1, 16]])
                            if ci == 0:
                                TT(dap, pap, wap, op=AL.mult)
                            else:
                                tap = bass.AP(ctmp.tensor, ctmp.offset, [list(ctmp.ap[0]), [128, 4], [16, 8], [1, 16]])
                                TT(tap, pap, wap, op=AL.mult)
                                TT(dap, dap, tap, op=AL.add)

                    # ---------- residuals, huber weights, G build ----------
                    def sq(q):
                        sl = samp[:, :]
                        return bass.AP(sl.tensor, sl.offset + q, [list(sl.ap[0]), [16, 8], [128, A]])

                    def i1q(c):
                        sl = i1[:, :]
                        return bass.AP(sl.tensor, sl.offset + c, [list(sl.ap[0]), [3 * A, 8], [3, A]])

                    Gt = sc.tile([128, 28 * 224], fp16, tag="Gt")
                    g6a = sc.tile([128, 6 * 224], fp16, tag="g6a")
                    g6b = sc.tile([128, 6 * 224], fp16, tag="g6b")
                    one_m = tm("one_m")
                    TS2(one_m[:, :], vmask[:, :], 1.0, -1e-6, AL.subtract, AL.mult)  # (1-vm)*1e-6
                    rr = tm("rr"); bb_ = tm("bb"); ss = tm("ss")
                    ppv = tm("ppv"); qqv = tm("qqv")

                    def abt6(k0):  # [x(6) outer, chunk(224) inner], stride 18 per chunk
                        sl = abt[:, :]
                        return bass.AP(sl.tensor, sl.offset + k0, [list(sl.ap[0]), [1, 6], [18, 224]])

                    def gcols(c):  # G cols c*7 .. c*7+5: [x outer, chunk inner]
                        sl = Gt[:, :]
                        return bass.AP(sl.tensor, sl.offset + c * 7 * 224, [list(sl.ap[0]), [224, 6], [1, 224]])

                    def bc6(t):    # broadcast [128,224] over 6 x-cols
                        sl = t[:, :]
                        return bass.AP(sl.tensor, sl.offset, [list(sl.ap[0]), [0, 6], [1, 224]])

                    for c in range(3):
                        TT(rr[:, :], i1q(c), sq(8 + c), op=AL.subtract)
                        TT(rr[:, :], rr[:, :], vmask[:, :], op=AL.mult)
                        TT(rr[:, :], rr[:, :], one_m[:, :], op=AL.add)
                        nc.scalar.activation(bb_[:, :], rr[:, :], ACT.Abs)
                        TS(bb_[:, :], bb_[:, :], HUBER_DELTA, AL.max)
                        nc.vector.reciprocal_approx_fast(bb_[:, :], bb_[:, :])
                        nc.scalar.activation(ss[:, :], bb_[:, :], ACT.Sqrt, scale=HUBER_DELTA)
                        TT(ppv[:, :], ss[:, :], sq(0 + c), op=AL.mult)
                        TT(qqv[:, :], ss[:, :], sq(3 + c), op=AL.mult)
                        TT(g6a[:, :], abt6(0), bc6(ppv), op=AL.mult)
                        TT(g6b[:, :], abt6(6), bc6(qqv), op=AL.mult)
                        TT(gcols(c), g6a[:, :], g6b[:, :], op=AL.add)
                        TT(Gt[:, (c * 7 + 6) * 224:(c * 7 + 7) * 224], ss[:, :], rr[:, :], op=AL.mult)
                    # depth channel
                    TT(rr[:, :], iz[:, :], sq(11), op=AL.subtract)
                    TT(rr[:, :], rr[:, :], vmask[:, :], op=AL.mult)
                    TT(rr[:, :], rr[:, :], one_m[:, :], op=AL.add)
                    nc.scalar.activation(bb_[:, :], rr[:, :], ACT.Abs, scale=LAMBDA)
                    TS(bb_[:, :], bb_[:, :], HUBER_DELTA, AL.max)
                    nc.vector.reciprocal_approx_fast(bb_[:, :], bb_[:, :])
                    nc.scalar.activation(ss[:, :], bb_[:, :], ACT.Sqrt,
                                         scale=HUBER_DELTA * LAMBDA * LAMBDA)
                    TT(ppv[:, :], ss[:, :], sq(6), op=AL.mult)
                    TT(qqv[:, :], ss[:, :], sq(7), op=AL.mult)
                    TT(g6a[:, :], abt6(0), bc6(ppv), op=AL.mult)
                    TT(g6b[:, :], abt6(6), bc6(qqv), op=AL.mult)
                    TT(g6a[:, :], g6a[:, :], g6b[:, :], op=AL.add)
                    TT(g6b[:, :], abt6(12), bc6(ss), op=AL.mult)
                    TT(gcols(3), g6a[:, :], g6b[:, :], op=AL.add)
                    TT(Gt[:, (3 * 7 + 6) * 224:(3 * 7 + 7) * 224], ss[:, :], rr[:, :], op=AL.mult)

                    # ---------- PE: JtWJ accumulation ----------
                    for g in range(8):
                        b = g // 4
                        for a in range(A):
                            off = Gt.offset + g * A + a
                            gap = bass.AP(Gt.tensor, off, [list(Gt.ap[0]), [224, 28]])
                            first = (s == 0 and (g % 4) == 0 and a == 0)
                            last = (s == NS - 1 and (g % 4) == 3 and a == A - 1)
                            nc.tensor.matmul(psJ[b][:, :], gap, gap,
                                             start=first, stop=last,
                                             skip_group_check=True)

                # ---------- per-iteration: extract JtWJ/Rhs, solve, update pose ----------
                S28 = sc.tile([28, 56], fp32, tag="S28")
                for b in range(2):
                    nc.vector.tensor_copy(S28[:, b * 28:(b + 1) * 28], psJ[b][:, :])
                D28 = sc.tile([7, 56], fp32, tag="D28")
                for b in range(2):
                    for c in range(4):
                        src = S28[c * 7:(c + 1) * 7, b * 28 + c * 7:b * 28 + c * 7 + 7]
                        dsl = D28[:, b * 28 + c * 7:b * 28 + c * 7 + 7]
                        nc.sync.dma_start(out=dsl, in_=src)
                M7 = sc.tile([7, 14], fp32, tag="M7")
                for b in range(2):
                    din = bass.AP(D28.tensor, D28.offset + b * 28,
                                  [list(D28.ap[0]), [1, 7], [7, 4]])
                    nc.vector.tensor_reduce(M7[:, b * 7:(b + 1) * 7], din, axis=AX.X, op=AL.add)
                # Mb [2, 49]: row b = M7 block b flattened (x-major)
                Mb = sc.tile([2, 49], fp32, tag="Mb")
                for b in range(2):
                    msrc = bass.AP(M7.tensor, M7.offset + b * 7, [[M7.ap[0][0], 7], [1, 7]])
                    mdsl = Mb[b:b + 1, 0:1]
                    mdst = bass.AP(mdsl.tensor, mdsl.offset, [[Mb.ap[0][0], 1], [7, 7], [1, 7]])
                    nc.sync.dma_start(out=mdst, in_=msrc)
                # tr = sum diag(JtWJ); LM ridge on diag
                trt = sc.tile([2, 1], fp32, tag="trt")
                diag = bass.AP(Mb.tensor, Mb.offset, [list(Mb.ap[0]), [8, 6]])
                nc.vector.tensor_reduce(trt[:, :], diag, axis=AX.X, op=AL.add)
                trb = bass.AP(trt.tensor, trt.offset, [list(trt.ap[0]), [0, 6]])
                STT(diag, trb, 1e-6, diag, AL.mult, AL.add)

                # Cholesky LL^T = Hm (6x6, both batches in 2 partitions)
                Lt = sc.tile([2, 36], fp32, tag="Lt")
                lsrc = bass.AP(Mb.tensor, Mb.offset, [list(Mb.ap[0]), [7, 6], [1, 6]])
                nc.vector.tensor_copy(Lt[:, :], lsrc)
                rhs = sc.tile([2, 6], fp32, tag="rhs")
                rsrc = bass.AP(Mb.tensor, Mb.offset + 6, [list(Mb.ap[0]), [7, 6]])
                nc.vector.tensor_copy(rhs[:, :], rsrc)
                idg = sc.tile([2, 6], fp32, tag="idg")
                tmpj = sc.tile([2, 36], fp32, tag="tmpj")
                red = sc.tile([2, 6], fp32, tag="redj")
                for j in range(6):
                    jj = Lt[:, 6 * j + j:6 * j + j + 1]
                    if j > 0:
                        ljk = Lt[:, 6 * j:6 * j + j]
                        TT(tmpj[:, :j], ljk, ljk, op=AL.mult)
                        nc.vector.tensor_reduce(red[:, 0:1], tmpj[:, :j], axis=AX.X, op=AL.add)
                        TT(jj, jj, red[:, 0:1], op=AL.subtract)
                    nc.scalar.activation(jj, jj, ACT.Sqrt)
                    nc.vector.reciprocal(idg[:, j:j + 1], jj)
                    nr = 5 - j
                    if nr > 0:
                        colap = bass.AP(Lt.tensor, Lt.offset + 6 * (j + 1) + j, [list(Lt.ap[0]), [6, nr]])
                        if j > 0:
                            lik = bass.AP(Lt.tensor, Lt.offset + 6 * (j + 1), [list(Lt.ap[0]), [6, nr], [1, j]])
                            ljkb = bass.AP(Lt.tensor, Lt.offset + 6 * j, [list(Lt.ap[0]), [0, nr], [1, j]])
                            TT(tmpj[:, :nr * j], lik, ljkb, op=AL.mult)
                            tin = bass.AP(tmpj.tensor, tmpj.offset, [list(tmpj.ap[0]), [j, nr], [1, j]])
                            nc.vector.tensor_reduce(red[:, :nr], tin, axis=AX.X, op=AL.add)
                            TT(colap, colap, red[:, :nr], op=AL.subtract)
                        nc.vector.tensor_scalar(colap, colap, idg[:, j:j + 1], None, AL.mult)
                # forward substitution: L y = rhs (in place on rhs)
                for j in range(6):
                    yj = rhs[:, j:j + 1]
                    if j > 0:
                        ljk = Lt[:, 6 * j:6 * j + j]
                        TT(tmpj[:, :j], ljk, rhs[:, :j], op=AL.mult)
                        nc.vector.tensor_reduce(red[:, 0:1], tmpj[:, :j], axis=AX.X, op=AL.add)
                        TT(yj, yj, red[:, 0:1], op=AL.subtract)
                    nc.vector.tensor_scalar(yj, yj, idg[:, j:j + 1], None, AL.mult)
                # back substitution: L^T x = y -> xi = -x stored in xi tile
                for j in range(5, -1, -1):
                    xj = rhs[:, j:j + 1]
                    nk = 5 - j
                    if nk > 0:
                        lkj = bass.AP(Lt.tensor, Lt.offset + 6 * (j + 1) + j, [list(Lt.ap[0]), [6, nk]])
                        TT(tmpj[:, :nk], lkj, rhs[:, j + 1:6], op=AL.mult)
                        nc.vector.tensor_reduce(red[:, 0:1], tmpj[:, :nk], axis=AX.X, op=AL.add)
                        TT(xj, xj, red[:, 0:1], op=AL.subtract)
                    nc.vector.tensor_scalar(xj, xj, idg[:, j:j + 1], None, AL.mult)
                xi = sc.tile([2, 6], fp32, tag="xi")
                TS(xi[:, :], rhs[:, :], -1.0, AL.mult)

                # se3_exp(xi) via Taylor series (|w| << 1 in this regime)
                w3 = xi[:, 3:6]
                wsq = sc.tile([2, 3], fp32, tag="wsq")
                TT(wsq[:, :], w3, w3, op=AL.mult)
                th2 = sc.tile([2, 1], fp32, tag="th2")
                nc.vector.tensor_reduce(th2[:, :], wsq[:, :], axis=AX.X, op=AL.add)
                coA = sc.tile([2, 1], fp32, tag="coA")
                coB = sc.tile([2, 1], fp32, tag="coB")
                coC = sc.tile([2, 1], fp32, tag="coC")
                hh = sc.tile([2, 1], fp32, tag="hh")
                TS2(hh[:, :], th2[:, :], 1.0 / 120.0, -1.0 / 6.0, AL.mult, AL.add)
                nc.vector.tensor_scalar(coA[:, :], th2[:, :], hh[:, :], 1.0, AL.mult, AL.add)
                TS2(hh[:, :], th2[:, :], 1.0 / 720.0, -1.0 / 24.0, AL.mult, AL.add)
                nc.vector.tensor_scalar(coB[:, :], th2[:, :], hh[:, :], 0.5, AL.mult, AL.add)
                TS2(hh[:, :], th2[:, :], 1.0 / 5040.0, -1.0 / 120.0, AL.mult, AL.add)
                nc.vector.tensor_scalar(coC[:, :], th2[:, :], hh[:, :], 1.0 / 6.0, AL.mult, AL.add)
                # K, K2
                Kt = sc.tile([2, 9], fp32, tag="Kt")
                nc.vector.memset(Kt[:, :], 0.0)
                TS(Kt[:, 1:2], xi[:, 5:6], -1.0, AL.mult)   # -z
                nc.vector.tensor_copy(Kt[:, 2:3], xi[:, 4:5])  # y
                nc.vector.tensor_copy(Kt[:, 3:4], xi[:, 5:6])  # z
                TS(Kt[:, 5:6], xi[:, 3:4], -1.0, AL.mult)   # -x
                TS(Kt[:, 6:7], xi[:, 4:5], -1.0, AL.mult)   # -y
                nc.vector.tensor_copy(Kt[:, 7:8], xi[:, 3:4])  # x
                K2t = sc.tile([2, 9], fp32, tag="K2t")
                wiap = bass.AP(xi.tensor, xi.offset + 3, [list(xi.ap[0]), [1, 3], [0, 3]])
                wjap = bass.AP(xi.tensor, xi.offset + 3, [list(xi.ap[0]), [0, 3], [1, 3]])
                TT(K2t[:, :], wiap, wjap, op=AL.mult)
                k2diag = bass.AP(K2t.tensor, K2t.offset, [list(K2t.ap[0]), [4, 3]])
                nc.vector.tensor_scalar(k2diag, k2diag, th2[:, :], None, AL.subtract)
                Rt = sc.tile([2, 9], fp32, tag="Rt")
                Vt = sc.tile([2, 9], fp32, tag="Vt")
                t9 = sc.tile([2, 9], fp32, tag="t9")
                nc.vector.tensor_scalar(Rt[:, :], Kt[:, :], coA[:, :], None, AL.mult)
                nc.vector.tensor_scalar(t9[:, :], K2t[:, :], coB[:, :], None, AL.mult)
                TT(Rt[:, :], Rt[:, :], t9[:, :], op=AL.add)
                rdiag = bass.AP(Rt.tensor, Rt.offset, [list(Rt.ap[0]), [4, 3]])
                TS(rdiag, rdiag, 1.0, AL.add)
                nc.vector.tensor_scalar(Vt[:, :], Kt[:, :], coB[:, :], None, AL.mult)
                nc.vector.tensor_scalar(t9[:, :], K2t[:, :], coC[:, :], None, AL.mult)
                TT(Vt[:, :], Vt[:, :], t9[:, :], op=AL.add)
                vdiag = bass.AP(Vt.tensor, Vt.offset, [list(Vt.ap[0]), [4, 3]])
                TS(vdiag, vdiag, 1.0, AL.add)
                # t = V @ v
                vbc = bass.AP(xi.tensor, xi.offset, [list(xi.ap[0]), [0, 3], [1, 3]])
                TT(t9[:, :], Vt[:, :], vbc, op=AL.mult)
                tv = sc.tile([2, 3], fp32, tag="tv")
                t9v = bass.AP(t9.tensor, t9.offset, [list(t9.ap[0]), [3, 3], [1, 3]])
                nc.vector.tensor_reduce(tv[:, :], t9v, axis=AX.X, op=AL.add)
                # E = [[R, t],[0,0,0,1]] as [2,16]
                Et = sc.tile([2, 16], fp32, tag="Et")
                nc.vector.memset(Et[:, :], 0.0)
                edst = bass.AP(Et.tensor, Et.offset, [list(Et.ap[0]), [4, 3], [1, 3]])
                esrc = bass.AP(Rt.tensor, Rt.offset, [list(Rt.ap[0]), [3, 3], [1, 3]])
                nc.vector.tensor_copy(edst, esrc)
                edst2 = bass.AP(Et.tensor, Et.offset + 3, [list(Et.ap[0]), [4, 3]])
                nc.vector.tensor_copy(edst2, tv[:, :])
                TS(Et[:, 15:16], Et[:, 15:16], 1.0, AL.add)
                # newT = T @ E
                nT = sc.tile([2, 16], fp32, tag="nT")
                for k in range(4):
                    tcol = bass.AP(Tq.tensor, Tq.offset + k, [list(Tq.ap[0]), [4, 4], [0, 4]])
                    erow = bass.AP(Et.tensor, Et.offset + 4 * k, [list(Et.ap[0]), [0, 4], [1, 4]])
                    if k == 0:
                        TT(nT[:, :], tcol, erow, op=AL.mult)
                    else:
                        TT(tmpj[:, :16], tcol, erow, op=AL.mult)
                        TT(nT[:, :], nT[:, :], tmpj[:, :16], op=AL.add)
                nc.vector.tensor_copy(Tq[:, :], nT[:, :])
                # rebuild q = [R(9) | t(3) | intr(4)] and broadcast to rtm/rtw
                qt = sc.tile([2, 16], fp32, tag="qt")
                qr = bass.AP(Tq.tensor, Tq.offset, [list(Tq.ap[0]), [4, 3], [1, 3]])
                nc.vector.tensor_copy(qt[:, 0:9], qr)
                qtcol = bass.AP(Tq.tensor, Tq.offset + 3, [list(Tq.ap[0]), [4, 3]])
                nc.vector.tensor_copy(qt[:, 9:12], qtcol)
                nc.vector.tensor_copy(qt[:, 12:16], intr[:, :])
                nc.sync.dma_start(out=qscr.ap(), in_=qt[:, :])
                qsap = qscr.ap()
                for b in range(2):
                    qsrc = bass.AP(qsap.tensor, qsap.offset + b * 16, [[0, 64], [1, 16]])
                    nc.sync.dma_start(out=rtw[b * 64:(b + 1) * 64, :], in_=qsrc)
                for g in range(8):
                    b = g // 4
                    qsrc = bass.AP(qsap.tensor, qsap.offset + b * 16, [[0, 128], [1, 16]])
                    rdst = bass.AP(rtm.tensor, rtm.offset + g, [list(rtm.ap[0]), [8, 16]])
                    nc.sync.dma_start(out=rdst, in_=qsrc)

            nc.sync.dma_start(out=tout_ext.ap(), in_=Tq[:, :])

    nc.finalize()
    return nc


def _get_runner():
    """Build the sharded jitted executable ONCE (per-call jit(shard_map(...))
    in run_bass_via_pjrt retraces every call)."""
    if "runner" in _NC_CACHE:
        return _NC_CACHE["runner"]
    import jax
    import numpy as _np
    from concourse import mybir
    from concourse import bass2jax
    from jax.experimental.shard_map import shard_map
    from jax.sharding import Mesh, PartitionSpec

    nc = _NC_CACHE.get("nc")
    if nc is None:
        nc = build_nc()
        _NC_CACHE["nc"] = nc
    bass2jax.install_neuronx_cc_hook()

    partition_name = nc.partition_id_tensor.name if nc.partition_id_tensor else None
    in_names, out_names, out_avals, zero_outs = [], [], [], []
    for alloc in nc.m.functions[0].allocations:
        if not isinstance(alloc, mybir.MemoryLocationSet):
            continue
        name = alloc.memorylocations[0].name
        if alloc.kind == "ExternalInput":
            if name != partition_name:
                in_names.append(name)
        elif alloc.kind == "ExternalOutput":
            out_names.append(name)
            shape = tuple(alloc.tensor_shape)
            dtype = mybir.dt.np(alloc.dtype)
            out_avals.append(jax.core.ShapedArray(shape, dtype))
            zero_outs.append(_np.zeros(shape, dtype))
    n_params = len(in_names)
    n_outs = len(out_names)
    all_names = list(in_names) + out_names
    if partition_name is not None:
        all_names.append(partition_name)
    donate = tuple(range(n_params, n_params + n_outs))

    def _body(*args):
        operands = list(args)
        if partition_name is not None:
            operands.append(bass2jax.partition_id_tensor())
        outs = bass2jax._bass_exec_p.bind(
            *operands,
            out_avals=tuple(out_avals),
            in_names=tuple(all_names),
            out_names=tuple(out_names),
            lowering_input_output_aliases=(),
            sim_require_finite=True,
            sim_require_nnan=True,
            nc=nc,
        )
        return tuple(outs)

    devices = jax.devices()[:8]
    mesh = Mesh(_np.asarray(devices), ("core",))
    in_specs = (PartitionSpec("core"),) * (n_params + n_outs)
    out_specs = (PartitionSpec("core"),) * n_outs
    sharded = jax.jit(
        shard_map(_body, mesh=mesh, in_specs=in_specs, out_specs=out_specs,
                  check_rep=False),
        donate_argnums=donate, keep_unused=True)
    from jax.sharding import NamedSharding
    shd = NamedSharding(mesh, PartitionSpec("core"))
    _NC_CACHE["runner"] = (sharded, in_names, out_names, out_avals, zero_outs, shd)
    return _NC_CACHE["runner"]


def kernel(pose_twist, I0, I1, invD0, invD1, intrinsics):
    from concourse.bass_utils import run_bass_kernel_spmd

    nc = _NC_CACHE.get("nc")
    if nc is None:
        nc = build_nc()
        _NC_CACHE["nc"] = nc

    pose_twist = np.asarray(pose_twist, np.float32)
    I0 = np.asarray(I0, np.float32); I1 = np.asarray(I1, np.float32)
    invD0 = np.asarray(invD0, np.float32); invD1 = np.asarray(invD1, np.float32)
    intrinsics = np.asarray(intrinsics, np.float32)

    import time as _time
    LAST_WALL.clear(); LAST_EXEC_NS.clear(); LAST_TRACES.clear()
    t0 = _time.time()
    in_maps, _ = host_precompute_all(pose_twist, I0, I1, invD0, invD1, intrinsics)
    t1 = _time.time()
    if PROFILE:
        res = run_bass_kernel_spmd(nc, in_maps, list(range(8)), trace=True)
        if res.exec_time_ns is not None:
            LAST_EXEC_NS.append(res.exec_time_ns)
        if res.instructions_and_trace is not None:
            LAST_TRACES.append(res.instructions_and_trace[1])
        touts = [res.results[core]["tout"] for core in range(8)]
    else:
        sharded, in_names, out_names, out_avals, zero_outs, shd = _get_runner()
        import jax as _jax
        # async per-tensor device_put: transfer of tensor k overlaps the host
        # concatenation of tensor k+1
        concat_in = []
        for nm in in_names:
            arr = np.concatenate([in_maps[c][nm] for c in range(8)], axis=0)
            concat_in.append(_jax.device_put(arr, shd))
        concat_zeros = [_jax.device_put(
            np.zeros((8 * z.shape[0], *z.shape[1:]), z.dtype), shd)
            for z in zero_outs]
        out_arrs = sharded(*concat_in, *concat_zeros)
        oi = out_names.index("tout")
        tall = np.asarray(out_arrs[oi]).reshape(8, *out_avals[oi].shape)
        touts = [tall[c] for c in range(8)]
    t2 = _time.time()
    LAST_WALL.extend([round(t1 - t0, 3), round(t2 - t1, 3)])

    outs = [t.reshape(2, 4, 4) for t in touts]
    return np.concatenate(outs, axis=0).astype(np.float32)
